# revision 12
# baseline (speedup 1.0000x reference)
"""AirObject GNN kernel for 8 Trainium2 NeuronCores (Bass/Tile).

Data-parallel over the T=8 graphs (one graph per core). Feature-major
activations. Attention uses the exact identity
  exp(leaky_0.2(e)) = max(exp(0.2 e), exp(e)),  e[j,i] = f1[i]+f2[j]
which is rank-1 in exp space: Q = adj * max(a_j*b_i, c_j*d_i); per map
only 2 tensor_scalar + 2 tensor_tensor passes, spread over DVE/ACT/GPS.
Softmax denominator rides the PE matmul as a ones-column. The TCN+mean
collapses exactly to a matvec against sliding column-sums, so only
[1024,5] column-sum/edge data crosses cores (AllReduce), each core does
a 256-row slice of the final matvec, AllReduce-gathers y, l2norms.
"""
import numpy as np
import ml_dtypes

import concourse.bacc as bacc
import concourse.tile as tile
import concourse.mybir as mybir
from concourse.bass_utils import run_bass_kernel_spmd

F32 = mybir.dt.float32
BF16 = mybir.dt.bfloat16
AL = mybir.AluOpType
AF = mybir.ActivationFunctionType

T, N = 8, 1024
NF = 512
NHID, NH = 64, 8
NOUT = 1024
TCN_OUT, TCN_K = 2048, 3
L_OUT = T * N - TCN_K + 1
NCH = N // 128
BF = ml_dtypes.bfloat16

_CACHE = {}


def _engine_seq():
    pat = ["DVE", "DVE", "ACT", "DVE"]
    i = 0
    while True:
        yield pat[i % len(pat)]
        i += 1


def _build():
    if "nc" in _CACHE:
        return _CACHE["nc"]
    nc = bacc.Bacc("TRN2", debug=False, num_devices=8)
    D = {}

    def din(name, shape, dt):
        D[name] = nc.dram_tensor(name, shape, dt, kind="ExternalInput")

    din("ptsT", [2, N], F32)
    din("descsT", [256, N], BF16)
    din("adjb", [N, N], BF16)
    din("edgemask", [1, 4], F32)
    din("onehot", [8, 1], F32)
    din("k2t", [3 * NOUT, 256], F32)
    din("tcnb", [1, 256], F32)
    din("pw1", [2, 64], F32); din("pb1", [64, 1], F32)
    din("pw2", [64, 128], F32); din("pb2", [128, 1], F32)
    din("pw3", [128, 256], F32); din("pb3", [256, 1], F32)
    for l in (1, 2):
        din(f"wall{l}", [NF, NF], BF16)
        din(f"ahat{l}", [NF, 16], BF16)
        din(f"wo{l}", [NF, NF], BF16)
        din(f"aohat{l}", [NF, 2], BF16)
    din("tw", [NF, NF], BF16); din("tb", [NF, 1], F32)
    din("f1w", [NF, NF], BF16); din("f1b", [NF, 1], F32)
    din("f2w", [NF, NOUT], BF16); din("f2b", [NOUT, 1], F32)
    din("l1w", [NF, NF], BF16); din("l1b", [NF, 1], F32)
    din("l2w", [NF, NOUT], BF16); din("l2b", [NOUT, 1], F32)
    out_d = nc.dram_tensor("out", [8, 256], F32, kind="ExternalOutput")

    eng = _engine_seq()

    with tile.TileContext(nc) as tc:
        ctx_outer = [
            tc.tile_pool(name="pw", bufs=1),      # persistent inputs/weights
            tc.tile_pool(name="px", bufs=1),      # persistent activations
            tc.tile_pool(name="uvp", bufs=3),     # u/v/elu scratch
            tc.tile_pool(name="qp", bufs=3),      # q rotating (heads)
            tc.tile_pool(name="qmp", bufs=1),     # q persistent (out-map)
            tc.tile_pool(name="repp", bufs=2),    # broadcast rows
            tc.tile_pool(name="evp", bufs=1),     # whn/whno/xcat evacs
            tc.tile_pool(name="smallp", bufs=2),  # small scratch
            tc.tile_pool(name="rowp", bufs=1),    # f32 exp rows
            tc.tile_pool(name="dramp", bufs=1, space="DRAM"),
        ]
        import contextlib
        with contextlib.ExitStack() as ST:
            pw, px, uvp, qp, qmp, repp, evp, smallp, rowp, dramp = [
                ST.enter_context(c) for c in ctx_outer]

            def load(name, shape=None, dt=None, part=None, tag=None, pool=pw):
                h = D[name]
                shape = shape or list(h.shape)
                t = pool.tile(shape, dt or h.dtype, name=f"t_{name}_{part}",
                              tag=tag or f"t_{name}_{part}")
                src = h.ap()
                if part is not None:
                    src = src[part[0]:part[1], :]
                nc.sync.dma_start(t[:], src)
                return t

            adj = [load("adjb", [128, N], part=(128 * j, 128 * (j + 1)))
                   for j in range(NCH)]
            x_fm = [load("descsT", [128, N], part=(128 * j, 128 * (j + 1)))
                    for j in range(2)]
            tcnb = load("tcnb")
            onehot = load("onehot")
            maskrep = pw.tile([128, 4], F32, name="maskrep")
            nc.sync.dma_start(maskrep[:], D["edgemask"].ap().partition_broadcast(128))
            ones1 = pw.tile([128, 1], F32, name="ones1")
            nc.vector.memset(ones1[:], 1.0)
            srow_scr = px.tile([128, N], F32, name="srow_scr")

            # ============ points encoder (fp32, feature-major) ============
            with tc.tile_pool(name="pes", bufs=1) as pes, \
                 tc.tile_pool(name="peps", bufs=2, space="PSUM") as peps:
                ptsT = load("ptsT", pool=pes)
                pw1 = load("pw1", pool=pes); pb1 = load("pb1", pool=pes)
                pw2 = load("pw2", pool=pes); pb2 = load("pb2", pool=pes)
                pw3 = load("pw3", pool=pes)
                pb3 = [load("pb3", [128, 1], part=(128 * j, 128 * (j + 1)),
                            pool=pes) for j in range(2)]
                o1 = pes.tile([64, N], F32, name="pe_o1")
                for h in range(2):
                    ps = peps.tile([64, 512], F32, name=f"pe1_{h}", tag="pe_ps")
                    nc.tensor.matmul(ps[:], pw1[:], ptsT[:, 512 * h:512 * (h + 1)],
                                     start=True, stop=True)
                    nc.scalar.activation(o1[:, 512 * h:512 * (h + 1)], ps[:],
                                         AF.Relu, bias=pb1[0:64, :])
                o2 = pes.tile([128, N], F32, name="pe_o2")
                for h in range(2):
                    ps = peps.tile([128, 512], F32, name=f"pe2_{h}", tag="pe_ps")
                    nc.tensor.matmul(ps[:], pw2[:], o1[:, 512 * h:512 * (h + 1)],
                                     start=True, stop=True)
                    nc.scalar.activation(o2[:, 512 * h:512 * (h + 1)], ps[:],
                                         AF.Relu, bias=pb2[:])
                ep = [pes.tile([128, N], F32, name=f"pe_ep{c}") for c in range(2)]
                epsq = pes.tile([128, N], F32, name="pe_sq")
                sq_ps = [peps.tile([1, 512], F32, name=f"ssq{h}", tag="ssq_ps")
                         for h in range(2)]
                for c in range(2):
                    for h in range(2):
                        ps = peps.tile([128, 512], F32, name=f"pe3_{c}{h}",
                                       tag="pe_ps")
                        nc.tensor.matmul(ps[:], pw3[:, 128 * c:128 * (c + 1)],
                                         o2[:, 512 * h:512 * (h + 1)],
                                         start=True, stop=True)
                        sl = (slice(None), slice(512 * h, 512 * (h + 1)))
                        nc.scalar.activation(ep[c][sl], ps[:], AF.Identity,
                                             bias=pb3[c][:])
                        nc.scalar.activation(epsq[sl], ps[:], AF.Square,
                                             bias=pb3[c][:])
                        nc.tensor.matmul(sq_ps[h][:], ones1[:], epsq[sl],
                                         start=(c == 0), stop=(c == 1))
                for h in range(2):
                    nc.scalar.copy(srow_scr[0:1, 512 * h:512 * (h + 1)],
                                   sq_ps[h][:])
                ssq_dram = dramp.tile([1, N], F32, name="ssq_dram")
                nc.sync.dma_start(ssq_dram[:], srow_scr[0:1, :])
                ssq128 = pes.tile([128, 8], F32, name="ssq128")
                nc.sync.dma_start(ssq128[:],
                                  ssq_dram[:].rearrange("a (p c) -> (a p) c", c=8))
                nc.scalar.activation(ssq128[:], ssq128[:], AF.Sqrt)
                nc.vector.reciprocal(ssq128[:], ssq128[:])
                inv_dram = dramp.tile([1, N], F32, name="inv_dram")
                nc.sync.dma_start(inv_dram[:].rearrange("a (p c) -> (a p) c", c=8),
                                  ssq128[:])
                invrep = pes.tile([128, N], F32, name="invrep")
                nc.sync.dma_start(invrep[:], inv_dram[:].partition_broadcast(128))
                for c in range(2):
                    epn = px.tile([128, N], BF16, name=f"x_ep{c}")
                    nc.vector.tensor_mul(epn[:], ep[c][:], invrep[:])
                    x_fm.append(epn)

            # ===================== GAT layers =====================
            def gat_layer(l, x_in, gtag):
                with tc.tile_pool(name=f"gw{l}", bufs=1) as gw, \
                     tc.tile_pool(name=f"gps{l}", bufs=2, space="PSUM") as gpsm, \
                     tc.tile_pool(name=f"gpsa{l}", bufs=2, space="PSUM") as gpsa:
                    wall = [load(f"wall{l}", [128, NF],
                                 part=(128 * k, 128 * (k + 1)),
                                 tag=f"wall{k}", pool=gw) for k in range(4)]
                    ahat = [load(f"ahat{l}", [128, 16],
                                 part=(128 * k, 128 * (k + 1)),
                                 tag=f"ahat{k}", pool=gw) for k in range(4)]
                    wo = [load(f"wo{l}", [64, NF], part=(64 * k, 64 * (k + 1)),
                               tag=f"wo{k}", pool=gw) for k in range(8)]
                    aoh = [load(f"aohat{l}", [64, 2], part=(64 * k, 64 * (k + 1)),
                                tag=f"aoh{k}", pool=gw) for k in range(8)]

                    # Wh node-major, [Wh_h | ones] 65-stride interleave
                    whn = []
                    for j in range(NCH):
                        t = evp.tile([128, 65 * NH], BF16, name=f"whn_{j}",
                                     tag=f"whn{j}")
                        ps = gpsm.tile([128, 512], F32, name=f"whps_{j}",
                                       tag="mm_ps")
                        for k in range(4):
                            nc.tensor.matmul(ps[:],
                                             x_in[k][:, 128 * j:128 * (j + 1)],
                                             wall[k][:], start=(k == 0),
                                             stop=(k == 3))
                        ot = t[:].rearrange("p (h c) -> p h c", c=65)
                        nc.scalar.copy(ot[:, :, 0:64],
                                       ps[:].rearrange("p (h c) -> p h c", c=64))
                        nc.vector.memset(ot[:, :, 64:65], 1.0)
                        whn.append(t)

                    # f vectors feature-major; exp rows; transposed scalars
                    fps = [gpsa.tile([16, 512], F32, name=f"fps_{h}", tag="f_ps")
                           for h in range(2)]
                    for h in range(2):
                        for k in range(4):
                            nc.tensor.matmul(fps[h][:], ahat[k][:],
                                             x_in[k][:, 512 * h:512 * (h + 1)],
                                             start=(k == 0), stop=(k == 3))
                    arow = rowp.tile([16, N], BF16, name="arow", tag="arow")
                    crow = rowp.tile([16, N], BF16, name="crow", tag="crow")
                    arowF = rowp.tile([16, N], F32, name="arowF", tag="arowF")
                    crowF = rowp.tile([16, N], F32, name="crowF", tag="crowF")
                    for h in range(2):
                        sl = (slice(None), slice(512 * h, 512 * (h + 1)))
                        nc.scalar.activation(arow[sl], fps[h][:], AF.Exp,
                                             scale=0.2)
                        nc.scalar.activation(crow[sl], fps[h][:], AF.Exp,
                                             scale=1.0)
                        nc.scalar.activation(arowF[sl], fps[h][:], AF.Exp,
                                             scale=0.2)
                        nc.scalar.activation(crowF[sl], fps[h][:], AF.Exp,
                                             scale=1.0)
                    ac_dram = dramp.tile([32, N], BF16, name=f"acd{l}", tag="acd")
                    acF_dram = dramp.tile([32, N], F32, name=f"acdF{l}",
                                          tag="acdF")
                    nc.sync.dma_start(ac_dram[0:16, :], arow[:])
                    nc.sync.dma_start(ac_dram[16:32, :], crow[:])
                    nc.sync.dma_start(acF_dram[0:16, :], arowF[:])
                    nc.sync.dma_start(acF_dram[16:32, :], crowF[:])
                    acd3 = acF_dram[:].rearrange("(g two) n -> g two n", two=2)
                    aT, cT = [], []
                    for j in range(NCH):
                        at = rowp.tile([128, 8], F32, name=f"aT_{j}", tag=f"aT{j}")
                        ct = rowp.tile([128, 8], F32, name=f"cT_{j}", tag=f"cT{j}")
                        nc.sync.dma_start_transpose(
                            at[:], acd3[0:8, 1, 128 * j:128 * (j + 1)])
                        nc.sync.dma_start_transpose(
                            ct[:], acd3[8:16, 1, 128 * j:128 * (j + 1)])
                        aT.append(at)
                        cT.append(ct)

                    def make_q(j, brep, drep, a_col, c_col, qpool, qtag):
                        u = uvp.tile([128, N], BF16, name="u_t", tag="u_t")
                        v = uvp.tile([128, N], BF16, name="v_t", tag="v_t")
                        for (tt, rep, col) in ((u, brep, a_col), (v, drep, c_col)):
                            e = next(eng)
                            if e == "ACT":
                                nc.scalar.activation(tt[:], rep[:], AF.Copy,
                                                     scale=col)
                            elif e == "GPS":
                                nc.gpsimd.tensor_scalar_mul(tt[:], rep[:], col)
                            else:
                                nc.vector.tensor_scalar_mul(tt[:], rep[:], col)
                        q = qpool.tile([128, N], BF16, name="q_t", tag=qtag)
                        nc.vector.tensor_max(q[:], u[:], v[:])
                        nc.vector.tensor_mul(q[:], q[:], adj[j][:])
                        return q

                    # ---- heads ----
                    s_dram = dramp.tile([NH, N], F32, name=f"sdram{l}",
                                        tag="sdram")
                    xcat = []
                    for hh in range(NH):
                        brep = repp.tile([128, N], BF16, name="brep", tag="brep",
                                         bufs=3)
                        drep = repp.tile([128, N], BF16, name="drep", tag="drep",
                                         bufs=3)
                        nc.sync.dma_start(
                            brep[:],
                            ac_dram[2 * hh:2 * hh + 1, :].partition_broadcast(128))
                        nc.sync.dma_start(
                            drep[:],
                            ac_dram[16 + 2 * hh:16 + 2 * hh + 1, :]
                            .partition_broadcast(128))
                        hps = gpsa.tile([65, N], F32, name=f"hps_{hh}",
                                        tag="att_ps")
                        for j in range(NCH):
                            q = make_q(j, brep, drep, aT[j][:, hh:hh + 1],
                                       cT[j][:, hh:hh + 1], qp, "q_t")
                            for h in range(2):
                                nc.tensor.matmul(
                                    hps[:, 512 * h:512 * (h + 1)],
                                    whn[j][:, 65 * hh:65 * hh + 65],
                                    q[:, 512 * h:512 * (h + 1)],
                                    start=(j == 0), stop=(j == NCH - 1))
                        nc.scalar.copy(srow_scr[64:65, :], hps[64:65, :])
                        sdh = dramp.tile([1, N], F32, name=f"sdh{hh}",
                                         tag=f"sdh{hh % 4}", bufs=2)
                        nc.sync.dma_start(sdh[:], srow_scr[64:65, :])
                        ev = evp.tile([64, N], BF16, name=f"hev_{hh}",
                                      tag=f"hev{hh}")
                        nc.scalar.copy(ev[:], hps[0:64, :])
                        xcat.append(ev)
                        srow3 = sdh[:].rearrange("a (p c) -> (a p) c", c=8)
                        sh128 = smallp.tile([128, 8], F32, name="sh128",
                                            tag="sh128")
                        nc.sync.dma_start(sh128[:], srow3)
                        nc.vector.reciprocal(sh128[:], sh128[:])
                        nc.sync.dma_start(srow3, sh128[:])
                        sirep = repp.tile([128, N], F32, name="sirep",
                                          tag="sirep")
                        nc.sync.dma_start(sirep[:],
                                          sdh[:].partition_broadcast(128))
                        xh = ev
                        nc.gpsimd.tensor_mul(xh[:], xh[:], sirep[0:64, :])
                        ex = uvp.tile([64, N], BF16, name="elu_e", tag="u_t")
                        nc.scalar.activation(ex[:], xh[:], AF.Exp)
                        nc.vector.tensor_scalar_add(ex[:], ex[:], -1.0)
                        nc.vector.tensor_scalar_min(ex[:], ex[:], 0.0)
                        nc.vector.tensor_max(xh[:], xh[:], ex[:])

                    # ---- output attention layer ----
                    whno = []
                    for j in range(NCH):
                        t = evp.tile([128, 513], BF16, name=f"whno_{j}",
                                     tag=f"whno{j}")
                        ps = gpsm.tile([128, 512], F32, name=f"wops_{j}",
                                       tag="mm_ps")
                        for k in range(8):
                            nc.tensor.matmul(ps[:],
                                             xcat[k][:, 128 * j:128 * (j + 1)],
                                             wo[k][:], start=(k == 0),
                                             stop=(k == 7))
                        nc.scalar.copy(t[:, 0:512], ps[:])
                        nc.vector.memset(t[:, 512:513], 1.0)
                        whno.append(t)
                    fops = [gpsa.tile([2, 512], F32, name=f"fo_{h}", tag="f_ps")
                            for h in range(2)]
                    for h in range(2):
                        for k in range(8):
                            nc.tensor.matmul(fops[h][:], aoh[k][:],
                                             xcat[k][:, 512 * h:512 * (h + 1)],
                                             start=(k == 0), stop=(k == 7))
                    aco_dram = dramp.tile([4, N], BF16, name=f"acod{l}",
                                          tag="acod")
                    acoF_dram = dramp.tile([4, N], F32, name=f"acodF{l}",
                                           tag="acodF")
                    aorow = rowp.tile([2, N], BF16, name="aorow", tag="aorow")
                    corow = rowp.tile([2, N], BF16, name="corow", tag="corow")
                    aorowF = rowp.tile([2, N], F32, name="aorowF", tag="aorowF")
                    corowF = rowp.tile([2, N], F32, name="corowF", tag="corowF")
                    for h in range(2):
                        sl = (slice(None), slice(512 * h, 512 * (h + 1)))
                        nc.scalar.activation(aorow[sl], fops[h][:], AF.Exp,
                                             scale=0.2)
                        nc.scalar.activation(corow[sl], fops[h][:], AF.Exp,
                                             scale=1.0)
                        nc.scalar.activation(aorowF[sl], fops[h][:], AF.Exp,
                                             scale=0.2)
                        nc.scalar.activation(corowF[sl], fops[h][:], AF.Exp,
                                             scale=1.0)
                    nc.sync.dma_start(aco_dram[0:2, :], aorow[:])
                    nc.sync.dma_start(aco_dram[2:4, :], corow[:])
                    nc.sync.dma_start(acoF_dram[0:2, :], aorowF[:])
                    nc.sync.dma_start(acoF_dram[2:4, :], corowF[:])
                    aoT, coT = [], []
                    for j in range(NCH):
                        at = rowp.tile([128, 1], F32, name=f"aoT_{j}",
                                       tag=f"aoT{j}")
                        ct = rowp.tile([128, 1], F32, name=f"coT_{j}",
                                       tag=f"coT{j}")
                        nc.sync.dma_start_transpose(
                            at[:], acoF_dram[1:2, 128 * j:128 * (j + 1)])
                        nc.sync.dma_start_transpose(
                            ct[:], acoF_dram[3:4, 128 * j:128 * (j + 1)])
                        aoT.append(at)
                        coT.append(ct)

                    brep = repp.tile([128, N], BF16, name="brep", tag="brep",
                                     bufs=3)
                    drep = repp.tile([128, N], BF16, name="drep", tag="drep",
                                     bufs=3)
                    nc.sync.dma_start(brep[:],
                                      aco_dram[0:1, :].partition_broadcast(128))
                    nc.sync.dma_start(drep[:],
                                      aco_dram[2:3, :].partition_broadcast(128))
                    sps_o = [gpsa.tile([1, 512], F32, name=f"spso{h}", tag="f_ps")
                             for h in range(2)]
                    qmap = []
                    for j in range(NCH):
                        q = make_q(j, brep, drep, aoT[j][:, 0:1], coT[j][:, 0:1],
                                   qmp, f"qm{j}")
                        qmap.append(q)
                        for h in range(2):
                            nc.tensor.matmul(sps_o[h][:],
                                             whno[j][:, 512:513],
                                             q[:, 512 * h:512 * (h + 1)],
                                             start=(j == 0), stop=(j == NCH - 1))
                    for h in range(2):
                        nc.scalar.copy(srow_scr[0:1, 512 * h:512 * (h + 1)],
                                       sps_o[h][:])
                    so_dram = dramp.tile([1, N], F32, name=f"sod{l}", tag="sod")
                    nc.sync.dma_start(so_dram[:], srow_scr[0:1, :])
                    so128 = smallp.tile([128, 8], F32, name="so128", tag="so128")
                    nc.sync.dma_start(
                        so128[:], so_dram[:].rearrange("a (p c) -> (a p) c", c=8))
                    nc.vector.reciprocal(so128[:], so128[:])
                    nc.sync.dma_start(
                        so_dram[:].rearrange("a (p c) -> (a p) c", c=8), so128[:])
                    sorep = repp.tile([128, N], F32, name="sorep", tag="sirep")
                    nc.sync.dma_start(sorep[:],
                                      so_dram[:].partition_broadcast(128))
                    gout = []
                    for m in range(4):
                        ops = gpsa.tile([128, N], F32, name=f"ops_{m}",
                                        tag="att_ps")
                        for j in range(NCH):
                            for h in range(2):
                                nc.tensor.matmul(
                                    ops[:, 512 * h:512 * (h + 1)],
                                    whno[j][:, 128 * m:128 * (m + 1)],
                                    qmap[j][:, 512 * h:512 * (h + 1)],
                                    start=(j == 0), stop=(j == NCH - 1))
                        g = px.tile([128, N], BF16, name=f"gout{l}_{m}",
                                    tag=f"{gtag}{m}")
                        nc.scalar.copy(g[:], ops[:])
                        nc.gpsimd.tensor_mul(g[:], g[:], sorep[:])
                        ex = uvp.tile([128, N], BF16, name="elu_o", tag="u_t")
                        nc.scalar.activation(ex[:], g[:], AF.Exp)
                        nc.vector.tensor_scalar_add(ex[:], ex[:], -1.0)
                        nc.vector.tensor_scalar_min(ex[:], ex[:], 0.0)
                        nc.vector.tensor_max(g[:], g[:], ex[:])
                        gout.append(g)
                    return gout

            g1 = gat_layer(1, x_fm, "goutA")
            g2 = gat_layer(2, g1, "goutB")

            # ============ MLPs + payload (feature-major) ============
            with tc.tile_pool(name="mw", bufs=1) as mw, \
                 tc.tile_pool(name="mps", bufs=3, space="PSUM") as mps:

                def loadw(name, n_out, k):
                    return load(name, [128, n_out], part=(128 * k, 128 * (k + 1)),
                                tag=f"mlpw{k}", pool=mw)

                def loadb(name, m):
                    return load(name, [128, 1], part=(128 * m, 128 * (m + 1)),
                                tag=f"mlpb{m % 4}_{name}", pool=mw)

                def mlp(x_in, wname, bname, n_out, xtag, pool):
                    wv = [loadw(wname, n_out, k) for k in range(4)]
                    bv = [loadb(bname, m) for m in range(n_out // 128)]
                    out = []
                    for m in range(n_out // 128):
                        t = pool.tile([128, N], BF16, name=f"o_{wname}_{m}",
                                      tag=f"{xtag}{m}")
                        for h in range(2):
                            ps = mps.tile([128, 512], F32,
                                          name=f"mp{wname}{m}{h}", tag="mm_ps")
                            for k in range(4):
                                nc.tensor.matmul(
                                    ps[:], wv[k][:, 128 * m:128 * (m + 1)],
                                    x_in[k][:, 512 * h:512 * (h + 1)],
                                    start=(k == 0), stop=(k == 3))
                            nc.scalar.activation(t[:, 512 * h:512 * (h + 1)],
                                                 ps[:], AF.Relu, bias=bv[m][:])
                        out.append(t)
                    return out

                tr = mlp(g2, "tw", "tb", NF, "goutA", px)
                f1o = mlp(tr, "f1w", "f1b", NF, "hev", evp)
                l1o_tiles = []
                for m in range(4):
                    wv = [loadw("l1w", NF, k) for k in range(4)]
                    bv = loadb("l1b", m)
                    t = evp.tile([128, N], BF16, name=f"o_l1w_{m}",
                                 tag=f"hev{4 + m}")
                    for h in range(2):
                        ps = mps.tile([128, 512], F32, name=f"mpl1{m}{h}",
                                      tag="mm_ps")
                        for k in range(4):
                            nc.tensor.matmul(
                                ps[:], wv[k][:, 128 * m:128 * (m + 1)],
                                tr[k][:, 512 * h:512 * (h + 1)],
                                start=(k == 0), stop=(k == 3))
                        nc.scalar.activation(t[:, 512 * h:512 * (h + 1)],
                                             ps[:], AF.Relu, bias=bv[:])
                    l1o_tiles.append(t)
                l1o = l1o_tiles

                f2wv = [loadw("f2w", NOUT, k) for k in range(4)]
                f2bv = [loadb("f2b", m) for m in range(8)]
                l2wv = [load("l2w", [128, NOUT], part=(128 * k, 128 * (k + 1)),
                             tag=f"mlpw2{k}", pool=mw) for k in range(4)]
                l2bv = [load("l2b", [128, 1], part=(128 * m, 128 * (m + 1)),
                             tag=f"mlpb2{m}", pool=mw) for m in range(8)]

                pay = [smallp.tile([128, 5], F32, name=f"pay{j}", tag=f"pay{j}",
                       bufs=1) for j in range(NCH)]
                fej = px.tile([128, N], BF16, name="fej", tag="fej")
                lej = px.tile([128, N], BF16, name="lej", tag="lej")
                nfs = px.tile([128, N], BF16, name="nf_scr", tag="nf_scr")
                for j in range(NCH):
                    for (t, wv, bv, xi) in ((fej, f2wv, f2bv, f1o),
                                            (lej, l2wv, l2bv, l1o)):
                        for h in range(2):
                            ps = mps.tile([128, 512], F32, name=f"nfp{j}{h}",
                                          tag="mm_ps")
                            for k in range(4):
                                nc.tensor.matmul(
                                    ps[:], wv[k][:, 128 * j:128 * (j + 1)],
                                    xi[k][:, 512 * h:512 * (h + 1)],
                                    start=(k == 0), stop=(k == 3))
                            nc.scalar.activation(t[:, 512 * h:512 * (h + 1)],
                                                 ps[:], AF.Relu, bias=bv[j][:])
                    nc.vector.tensor_mul(nfs[:], fej[:], lej[:])
                    nc.scalar.activation(nfs[:], nfs[:], AF.Identity,
                                         accum_out=pay[j][:, 0:1])
                    e1 = smallp.tile([128, 2], F32, name="edge1", tag="edge1")
                    e2 = smallp.tile([128, 2], F32, name="edge2", tag="edge2")
                    nc.vector.tensor_mul(e1[:], fej[:, 0:2], lej[:, 0:2])
                    nc.vector.tensor_mul(e2[:], fej[:, 1022:1024],
                                         lej[:, 1022:1024])
                    nc.vector.tensor_mul(pay[j][:, 1:3], e1[:], maskrep[:, 0:2])
                    nc.vector.tensor_mul(pay[j][:, 3:5], e2[:], maskrep[:, 2:4])

                pay_in = dramp.tile([N, 5], F32, name="pay_in")
                pay_out = dramp.tile([N, 5], F32, name="pay_out")
                for j in range(NCH):
                    nc.sync.dma_start(pay_in[128 * j:128 * (j + 1), :], pay[j][:])
                nc.gpsimd.collective_compute(
                    "AllReduce", AL.add, replica_groups=[list(range(8))],
                    ins=[pay_in.opt()], outs=[pay_out.opt()])

                # s vectors + TCN matvec (k2t loaded into freed adj slots)
                k2 = [load("k2t", [128, 256], part=(128 * c, 128 * (c + 1)),
                           tag=f"t_adjb_({128 * (c % 8)}, {128 * (c % 8 + 1)})")
                      for c in range(24)]
                yps = mps.tile([1, 256], F32, name="yps", tag="yps")
                sfls = []
                for j in range(NCH):
                    red = smallp.tile([128, 5], F32, name=f"red{j}",
                                      tag=f"pay{j}", bufs=1)
                    nc.sync.dma_start(red[:], pay_out[128 * j:128 * (j + 1), :])
                    sfl = smallp.tile([128, 3], F32, name=f"sfl{j}",
                                      tag=f"sfl{j}", bufs=1)
                    t01 = smallp.tile([128, 1], F32, name=f"t01_{j}", tag="t01")
                    nc.vector.tensor_sub(sfl[:, 0:1], red[:, 0:1], red[:, 3:4])
                    nc.vector.tensor_sub(sfl[:, 0:1], sfl[:, 0:1], red[:, 4:5])
                    nc.vector.tensor_sub(t01[:], red[:, 0:1], red[:, 1:2])
                    nc.vector.tensor_sub(sfl[:, 1:2], t01[:], red[:, 4:5])
                    nc.vector.tensor_sub(sfl[:, 2:3], t01[:], red[:, 2:3])
                    sfls.append(sfl)
                for k in range(3):
                    for j in range(NCH):
                        ch = k * 8 + j
                        nc.tensor.matmul(yps[:], sfls[j][:, k:k + 1], k2[ch][:],
                                         start=(ch == 0), stop=(ch == 23))

                ysb = smallp.tile([128, 256], F32, name="ysb", tag="ysb")
                nc.vector.tensor_add(ysb[0:1, :], yps[:], tcnb[:])
                y_dram = dramp.tile([1, 256], F32, name="y_dram")
                nc.sync.dma_start(y_dram[:], ysb[0:1, :])
                yrep = smallp.tile([8, 256], F32, name="yrep", tag="yrep")
                nc.sync.dma_start(yrep[:], y_dram[:].partition_broadcast(8))
                ypad = smallp.tile([8, 256], F32, name="ypad", tag="ypad")
                nc.vector.tensor_scalar_mul(ypad[:], yrep[:], onehot[:])
                yar_in = dramp.tile([8, 256], F32, name="yar_in")
                yar_out = dramp.tile([8, 256], F32, name="yar_out")
                nc.sync.dma_start(yar_in[:], ypad[:])
                nc.gpsimd.collective_compute(
                    "AllReduce", AL.add, replica_groups=[list(range(8))],
                    ins=[yar_in.opt()], outs=[yar_out.opt()])
                yfull = smallp.tile([8, 256], F32, name="yfull", tag="yfull")
                nc.sync.dma_start(yfull[:], yar_out[:])
                ysq = smallp.tile([8, 256], F32, name="ysq", tag="ysq")
                ss8 = smallp.tile([8, 1], F32, name="ss8", tag="ss8")
                nc.scalar.activation(ysq[:], yfull[:], AF.Square,
                                     accum_out=ss8[:])
                sstot = smallp.tile([1, 1], F32, name="sstot", tag="sstot")
                nc.gpsimd.tensor_reduce(sstot[:], ss8[:],
                                        axis=mybir.AxisListType.C, op=AL.add)
                nc.scalar.activation(sstot[:], sstot[:], AF.Sqrt)
                nc.vector.reciprocal(sstot[:], sstot[:])
                invn8 = smallp.tile([8, 1], F32, name="invn8", tag="invn8")
                nc.gpsimd.partition_broadcast(invn8[:], sstot[:])
                yn = smallp.tile([8, 256], F32, name="yn", tag="yn")
                nc.scalar.activation(yn[:], yfull[:], AF.Copy, scale=invn8[:])
                nc.sync.dma_start(out_d.ap(), yn[:])

    nc.compile()
    _CACHE["nc"] = nc
    return nc


def _prep_inputs(batch_points, batch_descs, batch_adj, params):
    f32 = np.float32

    def A(x):
        return np.asarray(x, dtype=f32)

    pts = A(batch_points); descs = A(batch_descs); adjf = A(batch_adj)
    pe = params["pe"]; gcn = params["gcn"]
    eps = 1e-5
    pe_w = [A(w) for w in pe["W"]]
    pe_b = [A(b) for b in pe["b"]]
    folded = []
    for i in range(3):
        W, b = pe_w[i], pe_b[i]
        if i < 2:
            g, bt, m, v = [A(t) for t in pe["bn"][i]]
            gp = g / np.sqrt(v + eps)
            W = W * gp[None, :]
            b = b * gp + (bt - m * gp)
        folded.append((W, b))

    shared = {
        "pw1": folded[0][0], "pb1": folded[0][1][:, None],
        "pw2": folded[1][0], "pb2": folded[1][1][:, None],
        "pw3": folded[2][0], "pb3": folded[2][1][:, None],
    }
    for l, gk in ((1, "gat1"), (2, "gat2")):
        g = gcn[gk]
        W = A(g["W"]); a = A(g["a"]); Wo = A(g["Wo"]); ao = A(g["ao"])
        wall = np.transpose(W, (1, 0, 2)).reshape(NF, NF)
        ahat = np.zeros((NF, 16), f32)
        for h in range(NH):
            ahat[:, 2 * h] = W[h] @ a[h][:NHID]
            ahat[:, 2 * h + 1] = W[h] @ a[h][NHID:]
        aohat = np.stack([Wo @ ao[:NF], Wo @ ao[NF:]], axis=1)
        shared[f"wall{l}"] = wall.astype(BF)
        shared[f"ahat{l}"] = ahat.astype(BF)
        shared[f"wo{l}"] = Wo.astype(BF)
        shared[f"aohat{l}"] = aohat.astype(BF)
    for nm, wk, bk in (("tw", "tran1_W", "tran1_b"), ("f1", "fe1_W", "fe1_b"),
                      ("f2", "fe2_W", "fe2_b"), ("l1", "le1_W", "le1_b"),
                      ("l2", "le2_W", "le2_b")):
        wn = nm if nm == "tw" else nm + "w"
        bn = "tb" if nm == "tw" else nm + "b"
        shared[wn] = A(gcn[wk]).astype(BF)
        shared[bn] = A(gcn[bk])[:, None]

    K2 = np.transpose(A(params["tcn_K"]), (0, 2, 1)).reshape(TCN_OUT, 3 * NOUT)
    K2T = np.ascontiguousarray((K2 / float(L_OUT)).T)
    tcn_b = A(params["tcn_b"])

    in_maps = []
    for c in range(8):
        m = dict(shared)
        m["ptsT"] = np.ascontiguousarray(pts[c].T)
        m["descsT"] = np.ascontiguousarray(descs[c].T).astype(BF)
        m["adjb"] = adjf[c].astype(BF)
        em = np.zeros((1, 4), f32)
        if c == 0:
            em[0, 0] = em[0, 1] = 1.0
        if c == 7:
            em[0, 2] = em[0, 3] = 1.0
        m["edgemask"] = em
        oh = np.zeros((8, 1), f32); oh[c, 0] = 1.0
        m["onehot"] = oh
        m["k2t"] = np.ascontiguousarray(K2T[:, 256 * c:256 * (c + 1)])
        m["tcnb"] = np.ascontiguousarray(tcn_b[256 * c:256 * (c + 1)])[None, :]
        for k in list(m):
            if m[k].dtype == np.float64:
                m[k] = m[k].astype(np.float32)
        in_maps.append(m)
    return in_maps


def kernel(batch_points, batch_descs, batch_adj, params, _trace=False):
    nc = _build()
    in_maps = _prep_inputs(batch_points, batch_descs, batch_adj, params)
    res = run_bass_kernel_spmd(nc, in_maps, core_ids=list(range(8)),
                               trace=_trace)
    kernel.last_result = res
    return res.results[0]["out"].reshape(1, TCN_OUT).astype(np.float32)


# revision 13
# speedup vs baseline: 1.2779x; 1.2779x over previous
"""AirObject GNN kernel for 8 Trainium2 NeuronCores (Bass/Tile).

Data-parallel over the T=8 graphs (one graph per core). Feature-major
activations. Attention uses the exact identity
  exp(leaky_0.2(e)) = max(exp(0.2 e), exp(e)),  e[j,i] = f1[i]+f2[j]
which is rank-1 in exp space: Q = adj * max(a_j*b_i, c_j*d_i); per map
only 2 tensor_scalar + 2 tensor_tensor passes, spread over DVE/ACT/GPS.
Softmax denominator rides the PE matmul as a ones-column. The TCN+mean
collapses exactly to a matvec against sliding column-sums, so only
[1024,5] column-sum/edge data crosses cores (AllReduce), each core does
a 256-row slice of the final matvec, AllReduce-gathers y, l2norms.
"""
import numpy as np
import ml_dtypes

import concourse.bacc as bacc
import concourse.tile as tile
import concourse.mybir as mybir
from concourse.bass_utils import run_bass_kernel_spmd

F32 = mybir.dt.float32
BF16 = mybir.dt.bfloat16
AL = mybir.AluOpType
AF = mybir.ActivationFunctionType

T, N = 8, 1024
NF = 512
NHID, NH = 64, 8
NOUT = 1024
TCN_OUT, TCN_K = 2048, 3
L_OUT = T * N - TCN_K + 1
NCH = N // 128
BF = ml_dtypes.bfloat16

_CACHE = {}


def _engine_seq():
    pat = ["DVE", "DVE", "ACT", "DVE"]
    i = 0
    while True:
        yield pat[i % len(pat)]
        i += 1


def _build():
    if "nc" in _CACHE:
        return _CACHE["nc"]
    nc = bacc.Bacc("TRN2", debug=False, num_devices=8)
    D = {}

    def din(name, shape, dt):
        D[name] = nc.dram_tensor(name, shape, dt, kind="ExternalInput")

    din("ptsT", [2, N], F32)
    din("descsT", [256, N], BF16)
    din("adjb", [N, N], BF16)
    din("edgemask", [1, 4], F32)
    din("onehot", [8, 1], F32)
    din("k2t", [3 * NOUT, 256], F32)
    din("tcnb", [1, 256], F32)
    din("pw1", [2, 64], F32); din("pb1", [64, 1], F32)
    din("pw2", [64, 128], F32); din("pb2", [128, 1], F32)
    din("pw3", [128, 256], F32); din("pb3", [256, 1], F32)
    for l in (1, 2):
        din(f"wall{l}", [NF, NF], BF16)
        din(f"ahat{l}", [NF, 16], BF16)
        din(f"wo{l}", [NF, NF], BF16)
        din(f"aohat{l}", [NF, 2], BF16)
    din("tw", [NF, NF], BF16); din("tb", [NF, 1], F32)
    din("f1w", [NF, NF], BF16); din("f1b", [NF, 1], F32)
    din("f2w", [NF, NOUT], BF16); din("f2b", [NOUT, 1], F32)
    din("l1w", [NF, NF], BF16); din("l1b", [NF, 1], F32)
    din("l2w", [NF, NOUT], BF16); din("l2b", [NOUT, 1], F32)
    out_d = nc.dram_tensor("out", [8, 256], F32, kind="ExternalOutput")

    eng = _engine_seq()

    with tile.TileContext(nc) as tc:
        ctx_outer = [
            tc.tile_pool(name="pw", bufs=1),      # persistent inputs/weights
            tc.tile_pool(name="px", bufs=1),      # persistent activations
            tc.tile_pool(name="uvp", bufs=3),     # u/v/elu scratch
            tc.tile_pool(name="qp", bufs=3),      # q rotating (heads)
            tc.tile_pool(name="qmp", bufs=1),     # q persistent (out-map)
            tc.tile_pool(name="repp", bufs=2),    # broadcast rows
            tc.tile_pool(name="evp", bufs=1),     # whn/whno/xcat evacs
            tc.tile_pool(name="smallp", bufs=2),  # small scratch
            tc.tile_pool(name="rowp", bufs=1),    # f32 exp rows
            tc.tile_pool(name="dramp", bufs=1, space="DRAM"),
        ]
        import contextlib
        with contextlib.ExitStack() as ST:
            pw, px, uvp, qp, qmp, repp, evp, smallp, rowp, dramp = [
                ST.enter_context(c) for c in ctx_outer]

            def load(name, shape=None, dt=None, part=None, tag=None, pool=pw):
                h = D[name]
                shape = shape or list(h.shape)
                t = pool.tile(shape, dt or h.dtype, name=f"t_{name}_{part}",
                              tag=tag or f"t_{name}_{part}")
                src = h.ap()
                if part is not None:
                    src = src[part[0]:part[1], :]
                nc.sync.dma_start(t[:], src)
                return t

            adj = [load("adjb", [128, N], part=(128 * j, 128 * (j + 1)))
                   for j in range(NCH)]
            x_fm = [load("descsT", [128, N], part=(128 * j, 128 * (j + 1)))
                    for j in range(2)]
            tcnb = load("tcnb")
            onehot = load("onehot")
            maskrep = pw.tile([128, 4], F32, name="maskrep")
            nc.sync.dma_start(maskrep[:], D["edgemask"].ap().partition_broadcast(128))
            ones1 = pw.tile([128, 1], F32, name="ones1")
            nc.vector.memset(ones1[:], 1.0)
            srow_scr = px.tile([128, N], F32, name="srow_scr")

            # ============ points encoder (fp32, feature-major) ============
            with tc.tile_pool(name="pes", bufs=1) as pes, \
                 tc.tile_pool(name="peps", bufs=2, space="PSUM") as peps:
                ptsT = load("ptsT", pool=pes)
                pw1 = load("pw1", pool=pes); pb1 = load("pb1", pool=pes)
                pw2 = load("pw2", pool=pes); pb2 = load("pb2", pool=pes)
                pw3 = load("pw3", pool=pes)
                pb3 = [load("pb3", [128, 1], part=(128 * j, 128 * (j + 1)),
                            pool=pes) for j in range(2)]
                o1 = pes.tile([64, N], F32, name="pe_o1")
                for h in range(2):
                    ps = peps.tile([64, 512], F32, name=f"pe1_{h}", tag="pe_ps")
                    nc.tensor.matmul(ps[:], pw1[:], ptsT[:, 512 * h:512 * (h + 1)],
                                     start=True, stop=True)
                    nc.scalar.activation(o1[:, 512 * h:512 * (h + 1)], ps[:],
                                         AF.Relu, bias=pb1[0:64, :])
                o2 = pes.tile([128, N], F32, name="pe_o2")
                for h in range(2):
                    ps = peps.tile([128, 512], F32, name=f"pe2_{h}", tag="pe_ps")
                    nc.tensor.matmul(ps[:], pw2[:], o1[:, 512 * h:512 * (h + 1)],
                                     start=True, stop=True)
                    nc.scalar.activation(o2[:, 512 * h:512 * (h + 1)], ps[:],
                                         AF.Relu, bias=pb2[:])
                ep = [pes.tile([128, N], F32, name=f"pe_ep{c}") for c in range(2)]
                epsq = pes.tile([128, N], F32, name="pe_sq")
                sq_ps = [peps.tile([1, 512], F32, name=f"ssq{h}", tag="ssq_ps")
                         for h in range(2)]
                for c in range(2):
                    for h in range(2):
                        ps = peps.tile([128, 512], F32, name=f"pe3_{c}{h}",
                                       tag="pe_ps")
                        nc.tensor.matmul(ps[:], pw3[:, 128 * c:128 * (c + 1)],
                                         o2[:, 512 * h:512 * (h + 1)],
                                         start=True, stop=True)
                        sl = (slice(None), slice(512 * h, 512 * (h + 1)))
                        nc.scalar.activation(ep[c][sl], ps[:], AF.Identity,
                                             bias=pb3[c][:])
                        nc.scalar.activation(epsq[sl], ps[:], AF.Square,
                                             bias=pb3[c][:])
                        nc.tensor.matmul(sq_ps[h][:], ones1[:], epsq[sl],
                                         start=(c == 0), stop=(c == 1))
                for h in range(2):
                    nc.scalar.copy(srow_scr[0:1, 512 * h:512 * (h + 1)],
                                   sq_ps[h][:])
                ssq_dram = dramp.tile([1, N], F32, name="ssq_dram")
                nc.sync.dma_start(ssq_dram[:], srow_scr[0:1, :])
                ssq128 = pes.tile([128, 8], F32, name="ssq128")
                nc.sync.dma_start(ssq128[:],
                                  ssq_dram[:].rearrange("a (p c) -> (a p) c", c=8))
                nc.scalar.activation(ssq128[:], ssq128[:], AF.Sqrt)
                nc.vector.reciprocal(ssq128[:], ssq128[:])
                inv_dram = dramp.tile([1, N], F32, name="inv_dram")
                nc.sync.dma_start(inv_dram[:].rearrange("a (p c) -> (a p) c", c=8),
                                  ssq128[:])
                invrep = pes.tile([128, N], F32, name="invrep")
                nc.sync.dma_start(invrep[:], inv_dram[:].partition_broadcast(128))
                for c in range(2):
                    epn = px.tile([128, N], BF16, name=f"x_ep{c}")
                    nc.vector.tensor_mul(epn[:], ep[c][:], invrep[:])
                    x_fm.append(epn)

            # ===================== GAT layers =====================
            def gat_layer(l, x_in, gtag):
                with tc.tile_pool(name=f"gw{l}", bufs=1) as gw, \
                     tc.tile_pool(name=f"gps{l}", bufs=2, space="PSUM") as gpsm, \
                     tc.tile_pool(name=f"gpsa{l}", bufs=2, space="PSUM") as gpsa:
                    wall = [load(f"wall{l}", [128, NF],
                                 part=(128 * k, 128 * (k + 1)),
                                 tag=f"wall{k}", pool=gw) for k in range(4)]
                    ahat = [load(f"ahat{l}", [128, 16],
                                 part=(128 * k, 128 * (k + 1)),
                                 tag=f"ahat{k}", pool=gw) for k in range(4)]
                    wo = [load(f"wo{l}", [64, NF], part=(64 * k, 64 * (k + 1)),
                               tag=f"wo{k}", pool=gw) for k in range(8)]
                    aoh = [load(f"aohat{l}", [64, 2], part=(64 * k, 64 * (k + 1)),
                                tag=f"aoh{k}", pool=gw) for k in range(8)]

                    # Wh node-major, [Wh_h | ones] 65-stride interleave
                    whn = []
                    for j in range(NCH):
                        t = evp.tile([128, 65 * NH], BF16, name=f"whn_{j}",
                                     tag=f"whn{j}")
                        ps = gpsm.tile([128, 512], F32, name=f"whps_{j}",
                                       tag="mm_ps")
                        for k in range(4):
                            nc.tensor.matmul(ps[:],
                                             x_in[k][:, 128 * j:128 * (j + 1)],
                                             wall[k][:], start=(k == 0),
                                             stop=(k == 3))
                        ot = t[:].rearrange("p (h c) -> p h c", c=65)
                        nc.scalar.copy(ot[:, :, 0:64],
                                       ps[:].rearrange("p (h c) -> p h c", c=64))
                        nc.vector.memset(ot[:, :, 64:65], 1.0)
                        whn.append(t)

                    # f vectors feature-major; exp rows; transposed scalars
                    fps = [gpsa.tile([16, 512], F32, name=f"fps_{h}", tag="f_ps")
                           for h in range(2)]
                    for h in range(2):
                        for k in range(4):
                            nc.tensor.matmul(fps[h][:], ahat[k][:],
                                             x_in[k][:, 512 * h:512 * (h + 1)],
                                             start=(k == 0), stop=(k == 3))
                    arow = rowp.tile([16, N], BF16, name="arow", tag="arow")
                    crow = rowp.tile([16, N], BF16, name="crow", tag="crow")
                    arowF = rowp.tile([16, N], F32, name="arowF", tag="arowF")
                    crowF = rowp.tile([16, N], F32, name="crowF", tag="crowF")
                    for h in range(2):
                        sl = (slice(None), slice(512 * h, 512 * (h + 1)))
                        nc.scalar.activation(arow[sl], fps[h][:], AF.Exp,
                                             scale=0.2)
                        nc.scalar.activation(crow[sl], fps[h][:], AF.Exp,
                                             scale=1.0)
                        nc.scalar.activation(arowF[sl], fps[h][:], AF.Exp,
                                             scale=0.2)
                        nc.scalar.activation(crowF[sl], fps[h][:], AF.Exp,
                                             scale=1.0)
                    ac_dram = dramp.tile([32, N], BF16, name=f"acd{l}", tag="acd")
                    acF_dram = dramp.tile([32, N], F32, name=f"acdF{l}",
                                          tag="acdF")
                    nc.sync.dma_start(ac_dram[0:16, :], arow[:])
                    nc.sync.dma_start(ac_dram[16:32, :], crow[:])
                    nc.sync.dma_start(acF_dram[0:16, :], arowF[:])
                    nc.sync.dma_start(acF_dram[16:32, :], crowF[:])
                    acd3 = acF_dram[:].rearrange("(g two) n -> g two n", two=2)
                    aT, cT = [], []
                    for j in range(NCH):
                        at = rowp.tile([128, 8], F32, name=f"aT_{j}", tag=f"aT{j}")
                        ct = rowp.tile([128, 8], F32, name=f"cT_{j}", tag=f"cT{j}")
                        nc.sync.dma_start_transpose(
                            at[:], acd3[0:8, 1, 128 * j:128 * (j + 1)])
                        nc.sync.dma_start_transpose(
                            ct[:], acd3[8:16, 1, 128 * j:128 * (j + 1)])
                        aT.append(at)
                        cT.append(ct)

                    def make_q(j, brep, drep, a_col, c_col, qpool, qtag):
                        u = uvp.tile([128, N], BF16, name="u_t", tag="u_t")
                        v = uvp.tile([128, N], BF16, name="v_t", tag="v_t")
                        for (tt, rep, col) in ((u, brep, a_col), (v, drep, c_col)):
                            e = next(eng)
                            if e == "ACT":
                                nc.scalar.activation(tt[:], rep[:], AF.Copy,
                                                     scale=col)
                            elif e == "GPS":
                                nc.gpsimd.tensor_scalar_mul(tt[:], rep[:], col)
                            else:
                                nc.vector.tensor_scalar_mul(tt[:], rep[:], col)
                        q = qpool.tile([128, N], BF16, name="q_t", tag=qtag)
                        nc.vector.tensor_max(q[:], u[:], v[:])
                        nc.vector.tensor_mul(q[:], q[:], adj[j][:])
                        return q

                    # ---- heads ----
                    s_dram = dramp.tile([NH, N], F32, name=f"sdram{l}",
                                        tag="sdram")
                    xcat = []
                    for hh in range(NH):
                        brep = repp.tile([128, N], BF16, name="brep", tag="brep",
                                         bufs=3)
                        drep = repp.tile([128, N], BF16, name="drep", tag="drep",
                                         bufs=3)
                        nc.sync.dma_start(
                            brep[:],
                            ac_dram[2 * hh:2 * hh + 1, :].partition_broadcast(128))
                        nc.sync.dma_start(
                            drep[:],
                            ac_dram[16 + 2 * hh:16 + 2 * hh + 1, :]
                            .partition_broadcast(128))
                        hps = gpsa.tile([65, N], F32, name=f"hps_{hh}",
                                        tag="att_ps")
                        for j in range(NCH):
                            q = make_q(j, brep, drep, aT[j][:, hh:hh + 1],
                                       cT[j][:, hh:hh + 1], qp, "q_t")
                            for h in range(2):
                                nc.tensor.matmul(
                                    hps[:, 512 * h:512 * (h + 1)],
                                    whn[j][:, 65 * hh:65 * hh + 65],
                                    q[:, 512 * h:512 * (h + 1)],
                                    start=(j == 0), stop=(j == NCH - 1))
                        nc.scalar.copy(srow_scr[64:65, :], hps[64:65, :])
                        nc.sync.dma_start(s_dram[hh:hh + 1, :],
                                          srow_scr[64:65, :])
                        ev = evp.tile([64, N], BF16, name=f"hev_{hh}",
                                      tag=f"hev{hh}")
                        nc.scalar.copy(ev[:], hps[0:64, :])
                        xcat.append(ev)
                    s128 = smallp.tile([128, 64], F32, name="s128", tag="s128")
                    nc.sync.dma_start(
                        s128[:], s_dram[:].rearrange("h (g c) -> (h g) c", c=64))
                    nc.vector.reciprocal(s128[:], s128[:])
                    nc.sync.dma_start(
                        s_dram[:].rearrange("h (g c) -> (h g) c", c=64), s128[:])
                    for hh in range(NH):
                        sirep = repp.tile([128, N], F32, name="sirep",
                                          tag="sirep")
                        nc.sync.dma_start(
                            sirep[:],
                            s_dram[hh:hh + 1, :].partition_broadcast(128))
                        xh = xcat[hh]
                        nc.gpsimd.tensor_mul(xh[:], xh[:], sirep[0:64, :])
                        ex = uvp.tile([64, N], BF16, name="elu_e", tag="u_t")
                        nc.scalar.activation(ex[:], xh[:], AF.Exp)
                        nc.vector.tensor_scalar_add(ex[:], ex[:], -1.0)
                        nc.vector.tensor_scalar_min(ex[:], ex[:], 0.0)
                        nc.vector.tensor_max(xh[:], xh[:], ex[:])

                    # ---- output attention layer ----
                    whno = []
                    for j in range(NCH):
                        t = evp.tile([128, 513], BF16, name=f"whno_{j}",
                                     tag=f"whno{j}")
                        ps = gpsm.tile([128, 512], F32, name=f"wops_{j}",
                                       tag="mm_ps")
                        for k in range(8):
                            nc.tensor.matmul(ps[:],
                                             xcat[k][:, 128 * j:128 * (j + 1)],
                                             wo[k][:], start=(k == 0),
                                             stop=(k == 7))
                        nc.scalar.copy(t[:, 0:512], ps[:])
                        nc.vector.memset(t[:, 512:513], 1.0)
                        whno.append(t)
                    fops = [gpsa.tile([2, 512], F32, name=f"fo_{h}", tag="f_ps")
                            for h in range(2)]
                    for h in range(2):
                        for k in range(8):
                            nc.tensor.matmul(fops[h][:], aoh[k][:],
                                             xcat[k][:, 512 * h:512 * (h + 1)],
                                             start=(k == 0), stop=(k == 7))
                    aco_dram = dramp.tile([4, N], BF16, name=f"acod{l}",
                                          tag="acod")
                    acoF_dram = dramp.tile([4, N], F32, name=f"acodF{l}",
                                           tag="acodF")
                    aorow = rowp.tile([2, N], BF16, name="aorow", tag="aorow")
                    corow = rowp.tile([2, N], BF16, name="corow", tag="corow")
                    aorowF = rowp.tile([2, N], F32, name="aorowF", tag="aorowF")
                    corowF = rowp.tile([2, N], F32, name="corowF", tag="corowF")
                    for h in range(2):
                        sl = (slice(None), slice(512 * h, 512 * (h + 1)))
                        nc.scalar.activation(aorow[sl], fops[h][:], AF.Exp,
                                             scale=0.2)
                        nc.scalar.activation(corow[sl], fops[h][:], AF.Exp,
                                             scale=1.0)
                        nc.scalar.activation(aorowF[sl], fops[h][:], AF.Exp,
                                             scale=0.2)
                        nc.scalar.activation(corowF[sl], fops[h][:], AF.Exp,
                                             scale=1.0)
                    nc.sync.dma_start(aco_dram[0:2, :], aorow[:])
                    nc.sync.dma_start(aco_dram[2:4, :], corow[:])
                    nc.sync.dma_start(acoF_dram[0:2, :], aorowF[:])
                    nc.sync.dma_start(acoF_dram[2:4, :], corowF[:])
                    aoT, coT = [], []
                    for j in range(NCH):
                        at = rowp.tile([128, 1], F32, name=f"aoT_{j}",
                                       tag=f"aoT{j}")
                        ct = rowp.tile([128, 1], F32, name=f"coT_{j}",
                                       tag=f"coT{j}")
                        nc.sync.dma_start_transpose(
                            at[:], acoF_dram[1:2, 128 * j:128 * (j + 1)])
                        nc.sync.dma_start_transpose(
                            ct[:], acoF_dram[3:4, 128 * j:128 * (j + 1)])
                        aoT.append(at)
                        coT.append(ct)

                    brep = repp.tile([128, N], BF16, name="brep", tag="brep",
                                     bufs=3)
                    drep = repp.tile([128, N], BF16, name="drep", tag="drep",
                                     bufs=3)
                    nc.sync.dma_start(brep[:],
                                      aco_dram[0:1, :].partition_broadcast(128))
                    nc.sync.dma_start(drep[:],
                                      aco_dram[2:3, :].partition_broadcast(128))
                    sps_o = [gpsa.tile([1, 512], F32, name=f"spso{h}", tag="f_ps")
                             for h in range(2)]
                    qmap = []
                    for j in range(NCH):
                        q = make_q(j, brep, drep, aoT[j][:, 0:1], coT[j][:, 0:1],
                                   qmp, f"qm{j}")
                        qmap.append(q)
                        for h in range(2):
                            nc.tensor.matmul(sps_o[h][:],
                                             whno[j][:, 512:513],
                                             q[:, 512 * h:512 * (h + 1)],
                                             start=(j == 0), stop=(j == NCH - 1))
                    for h in range(2):
                        nc.scalar.copy(srow_scr[0:1, 512 * h:512 * (h + 1)],
                                       sps_o[h][:])
                    so_dram = dramp.tile([1, N], F32, name=f"sod{l}", tag="sod")
                    nc.sync.dma_start(so_dram[:], srow_scr[0:1, :])
                    so128 = smallp.tile([128, 8], F32, name="so128", tag="so128")
                    nc.sync.dma_start(
                        so128[:], so_dram[:].rearrange("a (p c) -> (a p) c", c=8))
                    nc.vector.reciprocal(so128[:], so128[:])
                    nc.sync.dma_start(
                        so_dram[:].rearrange("a (p c) -> (a p) c", c=8), so128[:])
                    sorep = repp.tile([128, N], F32, name="sorep", tag="sirep")
                    nc.sync.dma_start(sorep[:],
                                      so_dram[:].partition_broadcast(128))
                    gout = []
                    for m in range(4):
                        ops = gpsa.tile([128, N], F32, name=f"ops_{m}",
                                        tag="att_ps")
                        for j in range(NCH):
                            for h in range(2):
                                nc.tensor.matmul(
                                    ops[:, 512 * h:512 * (h + 1)],
                                    whno[j][:, 128 * m:128 * (m + 1)],
                                    qmap[j][:, 512 * h:512 * (h + 1)],
                                    start=(j == 0), stop=(j == NCH - 1))
                        g = px.tile([128, N], BF16, name=f"gout{l}_{m}",
                                    tag=f"{gtag}{m}")
                        nc.scalar.copy(g[:], ops[:])
                        nc.gpsimd.tensor_mul(g[:], g[:], sorep[:])
                        ex = uvp.tile([128, N], BF16, name="elu_o", tag="u_t")
                        nc.scalar.activation(ex[:], g[:], AF.Exp)
                        nc.vector.tensor_scalar_add(ex[:], ex[:], -1.0)
                        nc.vector.tensor_scalar_min(ex[:], ex[:], 0.0)
                        nc.vector.tensor_max(g[:], g[:], ex[:])
                        gout.append(g)
                    return gout

            g1 = gat_layer(1, x_fm, "goutA")
            g2 = gat_layer(2, g1, "goutB")

            # ============ MLPs + payload (feature-major) ============
            with tc.tile_pool(name="mw", bufs=1) as mw, \
                 tc.tile_pool(name="mps", bufs=3, space="PSUM") as mps:

                def loadw(name, n_out, k):
                    return load(name, [128, n_out], part=(128 * k, 128 * (k + 1)),
                                tag=f"mlpw{k}", pool=mw)

                def loadb(name, m):
                    return load(name, [128, 1], part=(128 * m, 128 * (m + 1)),
                                tag=f"mlpb{m % 4}_{name}", pool=mw)

                def mlp(x_in, wname, bname, n_out, xtag, pool):
                    wv = [loadw(wname, n_out, k) for k in range(4)]
                    bv = [loadb(bname, m) for m in range(n_out // 128)]
                    out = []
                    for m in range(n_out // 128):
                        t = pool.tile([128, N], BF16, name=f"o_{wname}_{m}",
                                      tag=f"{xtag}{m}")
                        for h in range(2):
                            ps = mps.tile([128, 512], F32,
                                          name=f"mp{wname}{m}{h}", tag="mm_ps")
                            for k in range(4):
                                nc.tensor.matmul(
                                    ps[:], wv[k][:, 128 * m:128 * (m + 1)],
                                    x_in[k][:, 512 * h:512 * (h + 1)],
                                    start=(k == 0), stop=(k == 3))
                            nc.scalar.activation(t[:, 512 * h:512 * (h + 1)],
                                                 ps[:], AF.Relu, bias=bv[m][:])
                        out.append(t)
                    return out

                tr = mlp(g2, "tw", "tb", NF, "goutA", px)
                f1o = mlp(tr, "f1w", "f1b", NF, "hev", evp)
                l1o_tiles = []
                for m in range(4):
                    wv = [loadw("l1w", NF, k) for k in range(4)]
                    bv = loadb("l1b", m)
                    t = evp.tile([128, N], BF16, name=f"o_l1w_{m}",
                                 tag=f"hev{4 + m}")
                    for h in range(2):
                        ps = mps.tile([128, 512], F32, name=f"mpl1{m}{h}",
                                      tag="mm_ps")
                        for k in range(4):
                            nc.tensor.matmul(
                                ps[:], wv[k][:, 128 * m:128 * (m + 1)],
                                tr[k][:, 512 * h:512 * (h + 1)],
                                start=(k == 0), stop=(k == 3))
                        nc.scalar.activation(t[:, 512 * h:512 * (h + 1)],
                                             ps[:], AF.Relu, bias=bv[:])
                    l1o_tiles.append(t)
                l1o = l1o_tiles

                f2wv = [loadw("f2w", NOUT, k) for k in range(4)]
                f2bv = [loadb("f2b", m) for m in range(8)]
                l2wv = [load("l2w", [128, NOUT], part=(128 * k, 128 * (k + 1)),
                             tag=f"mlpw2{k}", pool=mw) for k in range(4)]
                l2bv = [load("l2b", [128, 1], part=(128 * m, 128 * (m + 1)),
                             tag=f"mlpb2{m}", pool=mw) for m in range(8)]

                pay = [smallp.tile([128, 5], F32, name=f"pay{j}", tag=f"pay{j}",
                       bufs=1) for j in range(NCH)]
                fej = px.tile([128, N], BF16, name="fej", tag="fej")
                lej = px.tile([128, N], BF16, name="lej", tag="lej")
                nfs = px.tile([128, N], BF16, name="nf_scr", tag="nf_scr")
                for j in range(NCH):
                    for (t, wv, bv, xi) in ((fej, f2wv, f2bv, f1o),
                                            (lej, l2wv, l2bv, l1o)):
                        for h in range(2):
                            ps = mps.tile([128, 512], F32, name=f"nfp{j}{h}",
                                          tag="mm_ps")
                            for k in range(4):
                                nc.tensor.matmul(
                                    ps[:], wv[k][:, 128 * j:128 * (j + 1)],
                                    xi[k][:, 512 * h:512 * (h + 1)],
                                    start=(k == 0), stop=(k == 3))
                            nc.scalar.activation(t[:, 512 * h:512 * (h + 1)],
                                                 ps[:], AF.Relu, bias=bv[j][:])
                    nc.vector.tensor_mul(nfs[:], fej[:], lej[:])
                    nc.scalar.activation(nfs[:], nfs[:], AF.Identity,
                                         accum_out=pay[j][:, 0:1])
                    e1 = smallp.tile([128, 2], F32, name="edge1", tag="edge1")
                    e2 = smallp.tile([128, 2], F32, name="edge2", tag="edge2")
                    nc.vector.tensor_mul(e1[:], fej[:, 0:2], lej[:, 0:2])
                    nc.vector.tensor_mul(e2[:], fej[:, 1022:1024],
                                         lej[:, 1022:1024])
                    nc.vector.tensor_mul(pay[j][:, 1:3], e1[:], maskrep[:, 0:2])
                    nc.vector.tensor_mul(pay[j][:, 3:5], e2[:], maskrep[:, 2:4])

                pay_in = dramp.tile([N, 5], F32, name="pay_in")
                pay_out = dramp.tile([N, 5], F32, name="pay_out")
                for j in range(NCH):
                    nc.sync.dma_start(pay_in[128 * j:128 * (j + 1), :], pay[j][:])
                nc.gpsimd.collective_compute(
                    "AllReduce", AL.add, replica_groups=[list(range(8))],
                    ins=[pay_in.opt()], outs=[pay_out.opt()])

                # s vectors + TCN matvec (k2t loaded into freed adj slots)
                k2 = [load("k2t", [128, 256], part=(128 * c, 128 * (c + 1)),
                           tag=f"t_adjb_({128 * (c % 8)}, {128 * (c % 8 + 1)})")
                      for c in range(24)]
                yps = mps.tile([1, 256], F32, name="yps", tag="yps")
                sfls = []
                for j in range(NCH):
                    red = smallp.tile([128, 5], F32, name=f"red{j}",
                                      tag=f"pay{j}", bufs=1)
                    nc.sync.dma_start(red[:], pay_out[128 * j:128 * (j + 1), :])
                    sfl = smallp.tile([128, 3], F32, name=f"sfl{j}",
                                      tag=f"sfl{j}", bufs=1)
                    t01 = smallp.tile([128, 1], F32, name=f"t01_{j}", tag="t01")
                    nc.vector.tensor_sub(sfl[:, 0:1], red[:, 0:1], red[:, 3:4])
                    nc.vector.tensor_sub(sfl[:, 0:1], sfl[:, 0:1], red[:, 4:5])
                    nc.vector.tensor_sub(t01[:], red[:, 0:1], red[:, 1:2])
                    nc.vector.tensor_sub(sfl[:, 1:2], t01[:], red[:, 4:5])
                    nc.vector.tensor_sub(sfl[:, 2:3], t01[:], red[:, 2:3])
                    sfls.append(sfl)
                for k in range(3):
                    for j in range(NCH):
                        ch = k * 8 + j
                        nc.tensor.matmul(yps[:], sfls[j][:, k:k + 1], k2[ch][:],
                                         start=(ch == 0), stop=(ch == 23))

                ysb = smallp.tile([128, 256], F32, name="ysb", tag="ysb")
                nc.vector.tensor_add(ysb[0:1, :], yps[:], tcnb[:])
                y_dram = dramp.tile([1, 256], F32, name="y_dram")
                nc.sync.dma_start(y_dram[:], ysb[0:1, :])
                yrep = smallp.tile([8, 256], F32, name="yrep", tag="yrep")
                nc.sync.dma_start(yrep[:], y_dram[:].partition_broadcast(8))
                ypad = smallp.tile([8, 256], F32, name="ypad", tag="ypad")
                nc.vector.tensor_scalar_mul(ypad[:], yrep[:], onehot[:])
                yar_in = dramp.tile([8, 256], F32, name="yar_in")
                yar_out = dramp.tile([8, 256], F32, name="yar_out")
                nc.sync.dma_start(yar_in[:], ypad[:])
                nc.gpsimd.collective_compute(
                    "AllReduce", AL.add, replica_groups=[list(range(8))],
                    ins=[yar_in.opt()], outs=[yar_out.opt()])
                yfull = smallp.tile([8, 256], F32, name="yfull", tag="yfull")
                nc.sync.dma_start(yfull[:], yar_out[:])
                ysq = smallp.tile([8, 256], F32, name="ysq", tag="ysq")
                ss8 = smallp.tile([8, 1], F32, name="ss8", tag="ss8")
                nc.scalar.activation(ysq[:], yfull[:], AF.Square,
                                     accum_out=ss8[:])
                sstot = smallp.tile([1, 1], F32, name="sstot", tag="sstot")
                nc.gpsimd.tensor_reduce(sstot[:], ss8[:],
                                        axis=mybir.AxisListType.C, op=AL.add)
                nc.scalar.activation(sstot[:], sstot[:], AF.Sqrt)
                nc.vector.reciprocal(sstot[:], sstot[:])
                invn8 = smallp.tile([8, 1], F32, name="invn8", tag="invn8")
                nc.gpsimd.partition_broadcast(invn8[:], sstot[:])
                yn = smallp.tile([8, 256], F32, name="yn", tag="yn")
                nc.scalar.activation(yn[:], yfull[:], AF.Copy, scale=invn8[:])
                nc.sync.dma_start(out_d.ap(), yn[:])

    nc.compile()
    _CACHE["nc"] = nc
    return nc


def _prep_inputs(batch_points, batch_descs, batch_adj, params):
    f32 = np.float32

    def A(x):
        return np.asarray(x, dtype=f32)

    pts = A(batch_points); descs = A(batch_descs); adjf = A(batch_adj)
    pe = params["pe"]; gcn = params["gcn"]
    eps = 1e-5
    pe_w = [A(w) for w in pe["W"]]
    pe_b = [A(b) for b in pe["b"]]
    folded = []
    for i in range(3):
        W, b = pe_w[i], pe_b[i]
        if i < 2:
            g, bt, m, v = [A(t) for t in pe["bn"][i]]
            gp = g / np.sqrt(v + eps)
            W = W * gp[None, :]
            b = b * gp + (bt - m * gp)
        folded.append((W, b))

    shared = {
        "pw1": folded[0][0], "pb1": folded[0][1][:, None],
        "pw2": folded[1][0], "pb2": folded[1][1][:, None],
        "pw3": folded[2][0], "pb3": folded[2][1][:, None],
    }
    for l, gk in ((1, "gat1"), (2, "gat2")):
        g = gcn[gk]
        W = A(g["W"]); a = A(g["a"]); Wo = A(g["Wo"]); ao = A(g["ao"])
        wall = np.transpose(W, (1, 0, 2)).reshape(NF, NF)
        ahat = np.zeros((NF, 16), f32)
        for h in range(NH):
            ahat[:, 2 * h] = W[h] @ a[h][:NHID]
            ahat[:, 2 * h + 1] = W[h] @ a[h][NHID:]
        aohat = np.stack([Wo @ ao[:NF], Wo @ ao[NF:]], axis=1)
        shared[f"wall{l}"] = wall.astype(BF)
        shared[f"ahat{l}"] = ahat.astype(BF)
        shared[f"wo{l}"] = Wo.astype(BF)
        shared[f"aohat{l}"] = aohat.astype(BF)
    for nm, wk, bk in (("tw", "tran1_W", "tran1_b"), ("f1", "fe1_W", "fe1_b"),
                      ("f2", "fe2_W", "fe2_b"), ("l1", "le1_W", "le1_b"),
                      ("l2", "le2_W", "le2_b")):
        wn = nm if nm == "tw" else nm + "w"
        bn = "tb" if nm == "tw" else nm + "b"
        shared[wn] = A(gcn[wk]).astype(BF)
        shared[bn] = A(gcn[bk])[:, None]

    K2 = np.transpose(A(params["tcn_K"]), (0, 2, 1)).reshape(TCN_OUT, 3 * NOUT)
    K2T = np.ascontiguousarray((K2 / float(L_OUT)).T)
    tcn_b = A(params["tcn_b"])

    in_maps = []
    for c in range(8):
        m = dict(shared)
        m["ptsT"] = np.ascontiguousarray(pts[c].T)
        m["descsT"] = np.ascontiguousarray(descs[c].T).astype(BF)
        m["adjb"] = adjf[c].astype(BF)
        em = np.zeros((1, 4), f32)
        if c == 0:
            em[0, 0] = em[0, 1] = 1.0
        if c == 7:
            em[0, 2] = em[0, 3] = 1.0
        m["edgemask"] = em
        oh = np.zeros((8, 1), f32); oh[c, 0] = 1.0
        m["onehot"] = oh
        m["k2t"] = np.ascontiguousarray(K2T[:, 256 * c:256 * (c + 1)])
        m["tcnb"] = np.ascontiguousarray(tcn_b[256 * c:256 * (c + 1)])[None, :]
        for k in list(m):
            if m[k].dtype == np.float64:
                m[k] = m[k].astype(np.float32)
        in_maps.append(m)
    return in_maps


def kernel(batch_points, batch_descs, batch_adj, params, _trace=False):
    nc = _build()
    in_maps = _prep_inputs(batch_points, batch_descs, batch_adj, params)
    res = run_bass_kernel_spmd(nc, in_maps, core_ids=list(range(8)),
                               trace=_trace)
    kernel.last_result = res
    return res.results[0]["out"].reshape(1, TCN_OUT).astype(np.float32)


# revision 14
# speedup vs baseline: 1.3024x; 1.0192x over previous
"""AirObject GNN kernel for 8 Trainium2 NeuronCores (Bass/Tile).

Data-parallel over the T=8 graphs (one graph per core). Feature-major
activations. Attention uses the exact identity
  exp(leaky_0.2(e)) = max(exp(0.2 e), exp(e)),  e[j,i] = f1[i]+f2[j]
which is rank-1 in exp space: Q = adj * max(a_j*b_i, c_j*d_i); per map
only 2 tensor_scalar + 2 tensor_tensor passes, spread over DVE/ACT/GPS.
Softmax denominator rides the PE matmul as a ones-column. The TCN+mean
collapses exactly to a matvec against sliding column-sums, so only
[1024,5] column-sum/edge data crosses cores (AllReduce), each core does
a 256-row slice of the final matvec, AllReduce-gathers y, l2norms.
"""
import numpy as np
import ml_dtypes

import concourse.bacc as bacc
import concourse.tile as tile
import concourse.mybir as mybir
from concourse.bass_utils import run_bass_kernel_spmd

F32 = mybir.dt.float32
BF16 = mybir.dt.bfloat16
AL = mybir.AluOpType
AF = mybir.ActivationFunctionType

T, N = 8, 1024
NF = 512
NHID, NH = 64, 8
NOUT = 1024
TCN_OUT, TCN_K = 2048, 3
L_OUT = T * N - TCN_K + 1
NCH = N // 128
BF = ml_dtypes.bfloat16

_CACHE = {}


def _engine_seq():
    pat = ["DVE", "ACT"]
    i = 0
    while True:
        yield pat[i % len(pat)]
        i += 1


def _build():
    if "nc" in _CACHE:
        return _CACHE["nc"]
    nc = bacc.Bacc("TRN2", debug=False, num_devices=8)
    D = {}

    def din(name, shape, dt):
        D[name] = nc.dram_tensor(name, shape, dt, kind="ExternalInput")

    din("ptsT", [2, N], F32)
    din("descsT", [256, N], BF16)
    din("adjb", [N, N], BF16)
    din("edgemask", [1, 4], F32)
    din("onehot", [8, 1], F32)
    din("k2t", [3 * NOUT, 256], F32)
    din("tcnb", [1, 256], F32)
    din("pw1", [2, 64], F32); din("pb1", [64, 1], F32)
    din("pw2", [64, 128], F32); din("pb2", [128, 1], F32)
    din("pw3", [128, 256], F32); din("pb3", [256, 1], F32)
    for l in (1, 2):
        din(f"wall{l}", [NF, NF], BF16)
        din(f"ahat{l}", [NF, 16], BF16)
        din(f"wo{l}", [NF, NF], BF16)
        din(f"aohat{l}", [NF, 2], BF16)
    din("tw", [NF, NF], BF16); din("tb", [NF, 1], F32)
    din("f1w", [NF, NF], BF16); din("f1b", [NF, 1], F32)
    din("f2w", [NF, NOUT], BF16); din("f2b", [NOUT, 1], F32)
    din("l1w", [NF, NF], BF16); din("l1b", [NF, 1], F32)
    din("l2w", [NF, NOUT], BF16); din("l2b", [NOUT, 1], F32)
    out_d = nc.dram_tensor("out", [8, 256], F32, kind="ExternalOutput")

    eng = _engine_seq()

    with tile.TileContext(nc) as tc:
        ctx_outer = [
            tc.tile_pool(name="pw", bufs=1),      # persistent inputs/weights
            tc.tile_pool(name="px", bufs=1),      # persistent activations
            tc.tile_pool(name="uvp", bufs=3),     # u/v/elu scratch
            tc.tile_pool(name="qp", bufs=3),      # q rotating (heads)
            tc.tile_pool(name="qmp", bufs=1),     # q persistent (out-map)
            tc.tile_pool(name="repp", bufs=2),    # broadcast rows
            tc.tile_pool(name="evp", bufs=1),     # whn/whno/xcat evacs
            tc.tile_pool(name="smallp", bufs=2),  # small scratch
            tc.tile_pool(name="rowp", bufs=1),    # f32 exp rows
            tc.tile_pool(name="dramp", bufs=1, space="DRAM"),
        ]
        import contextlib
        with contextlib.ExitStack() as ST:
            pw, px, uvp, qp, qmp, repp, evp, smallp, rowp, dramp = [
                ST.enter_context(c) for c in ctx_outer]

            def load(name, shape=None, dt=None, part=None, tag=None, pool=pw):
                h = D[name]
                shape = shape or list(h.shape)
                t = pool.tile(shape, dt or h.dtype, name=f"t_{name}_{part}",
                              tag=tag or f"t_{name}_{part}")
                src = h.ap()
                if part is not None:
                    src = src[part[0]:part[1], :]
                nc.sync.dma_start(t[:], src)
                return t

            adj = [load("adjb", [128, N], part=(128 * j, 128 * (j + 1)))
                   for j in range(NCH)]
            x_fm = [load("descsT", [128, N], part=(128 * j, 128 * (j + 1)))
                    for j in range(2)]
            tcnb = load("tcnb")
            onehot = load("onehot")
            maskrep = pw.tile([128, 4], F32, name="maskrep")
            nc.sync.dma_start(maskrep[:], D["edgemask"].ap().partition_broadcast(128))
            ones1 = pw.tile([128, 1], F32, name="ones1")
            nc.vector.memset(ones1[:], 1.0)
            srow_scr = px.tile([128, N], F32, name="srow_scr")

            # ============ points encoder (fp32, feature-major) ============
            with tc.tile_pool(name="pes", bufs=1) as pes, \
                 tc.tile_pool(name="peps", bufs=2, space="PSUM") as peps:
                ptsT = load("ptsT", pool=pes)
                pw1 = load("pw1", pool=pes); pb1 = load("pb1", pool=pes)
                pw2 = load("pw2", pool=pes); pb2 = load("pb2", pool=pes)
                pw3 = load("pw3", pool=pes)
                pb3 = [load("pb3", [128, 1], part=(128 * j, 128 * (j + 1)),
                            pool=pes) for j in range(2)]
                o1 = pes.tile([64, N], F32, name="pe_o1")
                for h in range(2):
                    ps = peps.tile([64, 512], F32, name=f"pe1_{h}", tag="pe_ps")
                    nc.tensor.matmul(ps[:], pw1[:], ptsT[:, 512 * h:512 * (h + 1)],
                                     start=True, stop=True)
                    nc.scalar.activation(o1[:, 512 * h:512 * (h + 1)], ps[:],
                                         AF.Relu, bias=pb1[0:64, :])
                o2 = pes.tile([128, N], F32, name="pe_o2")
                for h in range(2):
                    ps = peps.tile([128, 512], F32, name=f"pe2_{h}", tag="pe_ps")
                    nc.tensor.matmul(ps[:], pw2[:], o1[:, 512 * h:512 * (h + 1)],
                                     start=True, stop=True)
                    nc.scalar.activation(o2[:, 512 * h:512 * (h + 1)], ps[:],
                                         AF.Relu, bias=pb2[:])
                ep = [pes.tile([128, N], F32, name=f"pe_ep{c}") for c in range(2)]
                epsq = pes.tile([128, N], F32, name="pe_sq")
                sq_ps = [peps.tile([1, 512], F32, name=f"ssq{h}", tag="ssq_ps")
                         for h in range(2)]
                for c in range(2):
                    for h in range(2):
                        ps = peps.tile([128, 512], F32, name=f"pe3_{c}{h}",
                                       tag="pe_ps")
                        nc.tensor.matmul(ps[:], pw3[:, 128 * c:128 * (c + 1)],
                                         o2[:, 512 * h:512 * (h + 1)],
                                         start=True, stop=True)
                        sl = (slice(None), slice(512 * h, 512 * (h + 1)))
                        nc.scalar.activation(ep[c][sl], ps[:], AF.Identity,
                                             bias=pb3[c][:])
                        nc.scalar.activation(epsq[sl], ps[:], AF.Square,
                                             bias=pb3[c][:])
                        nc.tensor.matmul(sq_ps[h][:], ones1[:], epsq[sl],
                                         start=(c == 0), stop=(c == 1))
                for h in range(2):
                    nc.scalar.copy(srow_scr[0:1, 512 * h:512 * (h + 1)],
                                   sq_ps[h][:])
                ssq_dram = dramp.tile([1, N], F32, name="ssq_dram")
                nc.sync.dma_start(ssq_dram[:], srow_scr[0:1, :])
                ssq128 = pes.tile([128, 8], F32, name="ssq128")
                nc.sync.dma_start(ssq128[:],
                                  ssq_dram[:].rearrange("a (p c) -> (a p) c", c=8))
                nc.scalar.activation(ssq128[:], ssq128[:], AF.Sqrt)
                nc.vector.reciprocal(ssq128[:], ssq128[:])
                inv_dram = dramp.tile([1, N], F32, name="inv_dram")
                nc.sync.dma_start(inv_dram[:].rearrange("a (p c) -> (a p) c", c=8),
                                  ssq128[:])
                invrep = pes.tile([128, N], F32, name="invrep")
                nc.sync.dma_start(invrep[:], inv_dram[:].partition_broadcast(128))
                for c in range(2):
                    epn = px.tile([128, N], BF16, name=f"x_ep{c}")
                    nc.vector.tensor_mul(epn[:], ep[c][:], invrep[:])
                    x_fm.append(epn)

            # ===================== GAT layers =====================
            def gat_layer(l, x_in, gtag):
                with tc.tile_pool(name=f"gw{l}", bufs=1) as gw, \
                     tc.tile_pool(name=f"gps{l}", bufs=2, space="PSUM") as gpsm, \
                     tc.tile_pool(name=f"gpsa{l}", bufs=2, space="PSUM") as gpsa:
                    wall = [load(f"wall{l}", [128, NF],
                                 part=(128 * k, 128 * (k + 1)),
                                 tag=f"wall{k}", pool=gw) for k in range(4)]
                    ahat = [load(f"ahat{l}", [128, 16],
                                 part=(128 * k, 128 * (k + 1)),
                                 tag=f"ahat{k}", pool=gw) for k in range(4)]
                    wo = [load(f"wo{l}", [64, NF], part=(64 * k, 64 * (k + 1)),
                               tag=f"wo{k}", pool=gw) for k in range(8)]
                    aoh = [load(f"aohat{l}", [64, 2], part=(64 * k, 64 * (k + 1)),
                                tag=f"aoh{k}", pool=gw) for k in range(8)]

                    # Wh node-major, [Wh_h | ones] 65-stride interleave
                    whn = []
                    for j in range(NCH):
                        t = evp.tile([128, 65 * NH], BF16, name=f"whn_{j}",
                                     tag=f"whn{j}")
                        ps = gpsm.tile([128, 512], F32, name=f"whps_{j}",
                                       tag="mm_ps")
                        for k in range(4):
                            nc.tensor.matmul(ps[:],
                                             x_in[k][:, 128 * j:128 * (j + 1)],
                                             wall[k][:], start=(k == 0),
                                             stop=(k == 3))
                        ot = t[:].rearrange("p (h c) -> p h c", c=65)
                        nc.scalar.copy(ot[:, :, 0:64],
                                       ps[:].rearrange("p (h c) -> p h c", c=64))
                        nc.vector.memset(ot[:, :, 64:65], 1.0)
                        whn.append(t)

                    # f vectors feature-major; exp rows; transposed scalars
                    fps = [gpsa.tile([16, 512], F32, name=f"fps_{h}", tag="f_ps")
                           for h in range(2)]
                    for h in range(2):
                        for k in range(4):
                            nc.tensor.matmul(fps[h][:], ahat[k][:],
                                             x_in[k][:, 512 * h:512 * (h + 1)],
                                             start=(k == 0), stop=(k == 3))
                    arow = rowp.tile([16, N], BF16, name="arow", tag="arow")
                    crow = rowp.tile([16, N], BF16, name="crow", tag="crow")
                    arowF = rowp.tile([16, N], F32, name="arowF", tag="arowF")
                    crowF = rowp.tile([16, N], F32, name="crowF", tag="crowF")
                    for h in range(2):
                        sl = (slice(None), slice(512 * h, 512 * (h + 1)))
                        nc.scalar.activation(arow[sl], fps[h][:], AF.Exp,
                                             scale=0.2)
                        nc.scalar.activation(crow[sl], fps[h][:], AF.Exp,
                                             scale=1.0)
                        nc.scalar.activation(arowF[sl], fps[h][:], AF.Exp,
                                             scale=0.2)
                        nc.scalar.activation(crowF[sl], fps[h][:], AF.Exp,
                                             scale=1.0)
                    ac_dram = dramp.tile([32, N], BF16, name=f"acd{l}", tag="acd")
                    acF_dram = dramp.tile([32, N], F32, name=f"acdF{l}",
                                          tag="acdF")
                    nc.sync.dma_start(ac_dram[0:16, :], arow[:])
                    nc.sync.dma_start(ac_dram[16:32, :], crow[:])
                    nc.sync.dma_start(acF_dram[0:16, :], arowF[:])
                    nc.sync.dma_start(acF_dram[16:32, :], crowF[:])
                    acd3 = acF_dram[:].rearrange("(g two) n -> g two n", two=2)
                    aT, cT = [], []
                    for j in range(NCH):
                        at = rowp.tile([128, 8], F32, name=f"aT_{j}", tag=f"aT{j}")
                        ct = rowp.tile([128, 8], F32, name=f"cT_{j}", tag=f"cT{j}")
                        nc.sync.dma_start_transpose(
                            at[:], acd3[0:8, 1, 128 * j:128 * (j + 1)])
                        nc.sync.dma_start_transpose(
                            ct[:], acd3[8:16, 1, 128 * j:128 * (j + 1)])
                        aT.append(at)
                        cT.append(ct)

                    def make_q(j, brep, drep, a_col, c_col, qpool, qtag):
                        u = uvp.tile([128, N], BF16, name="u_t", tag="u_t")
                        v = uvp.tile([128, N], BF16, name="v_t", tag="v_t")
                        for (tt, rep, col) in ((u, brep, a_col), (v, drep, c_col)):
                            e = next(eng)
                            if e == "ACT":
                                nc.scalar.activation(tt[:], rep[:], AF.Copy,
                                                     scale=col)
                            elif e == "GPS":
                                nc.gpsimd.tensor_scalar_mul(tt[:], rep[:], col)
                            else:
                                nc.vector.tensor_scalar_mul(tt[:], rep[:], col)
                        q = qpool.tile([128, N], BF16, name="q_t", tag=qtag)
                        nc.vector.tensor_max(q[:], u[:], v[:])
                        nc.vector.tensor_mul(q[:], q[:], adj[j][:])
                        return q

                    # ---- heads ----
                    s_dram = dramp.tile([NH, N], F32, name=f"sdram{l}",
                                        tag="sdram")
                    xcat = []
                    for hh in range(NH):
                        brep = repp.tile([128, N], BF16, name="brep", tag="brep",
                                         bufs=3)
                        drep = repp.tile([128, N], BF16, name="drep", tag="drep",
                                         bufs=3)
                        nc.sync.dma_start(
                            brep[:],
                            ac_dram[2 * hh:2 * hh + 1, :].partition_broadcast(128))
                        nc.sync.dma_start(
                            drep[:],
                            ac_dram[16 + 2 * hh:16 + 2 * hh + 1, :]
                            .partition_broadcast(128))
                        hps = gpsa.tile([65, N], F32, name=f"hps_{hh}",
                                        tag="att_ps")
                        for j in range(NCH):
                            q = make_q(j, brep, drep, aT[j][:, hh:hh + 1],
                                       cT[j][:, hh:hh + 1], qp, "q_t")
                            for h in range(2):
                                nc.tensor.matmul(
                                    hps[:, 512 * h:512 * (h + 1)],
                                    whn[j][:, 65 * hh:65 * hh + 65],
                                    q[:, 512 * h:512 * (h + 1)],
                                    start=(j == 0), stop=(j == NCH - 1))
                        nc.scalar.copy(srow_scr[64:65, :], hps[64:65, :])
                        nc.sync.dma_start(s_dram[hh:hh + 1, :],
                                          srow_scr[64:65, :])
                        ev = evp.tile([64, N], BF16, name=f"hev_{hh}",
                                      tag=f"hev{hh}")
                        nc.scalar.copy(ev[:], hps[0:64, :])
                        xcat.append(ev)
                    s128 = smallp.tile([128, 64], F32, name="s128", tag="s128")
                    nc.sync.dma_start(
                        s128[:], s_dram[:].rearrange("h (g c) -> (h g) c", c=64))
                    nc.vector.reciprocal(s128[:], s128[:])
                    nc.sync.dma_start(
                        s_dram[:].rearrange("h (g c) -> (h g) c", c=64), s128[:])
                    for hh in range(NH):
                        sirep = repp.tile([128, N], F32, name="sirep",
                                          tag="sirep")
                        nc.sync.dma_start(
                            sirep[:],
                            s_dram[hh:hh + 1, :].partition_broadcast(128))
                        xh = xcat[hh]
                        nc.gpsimd.tensor_mul(xh[:], xh[:], sirep[0:64, :])
                        ex = uvp.tile([64, N], BF16, name="elu_e", tag="u_t")
                        nc.scalar.activation(ex[:], xh[:], AF.Exp)
                        nc.vector.tensor_scalar_add(ex[:], ex[:], -1.0)
                        nc.vector.tensor_scalar_min(ex[:], ex[:], 0.0)
                        nc.vector.tensor_max(xh[:], xh[:], ex[:])

                    # ---- output attention layer ----
                    whno = []
                    for j in range(NCH):
                        t = evp.tile([128, 513], BF16, name=f"whno_{j}",
                                     tag=f"whno{j}")
                        ps = gpsm.tile([128, 512], F32, name=f"wops_{j}",
                                       tag="mm_ps")
                        for k in range(8):
                            nc.tensor.matmul(ps[:],
                                             xcat[k][:, 128 * j:128 * (j + 1)],
                                             wo[k][:], start=(k == 0),
                                             stop=(k == 7))
                        nc.scalar.copy(t[:, 0:512], ps[:])
                        nc.vector.memset(t[:, 512:513], 1.0)
                        whno.append(t)
                    fops = [gpsa.tile([2, 512], F32, name=f"fo_{h}", tag="f_ps")
                            for h in range(2)]
                    for h in range(2):
                        for k in range(8):
                            nc.tensor.matmul(fops[h][:], aoh[k][:],
                                             xcat[k][:, 512 * h:512 * (h + 1)],
                                             start=(k == 0), stop=(k == 7))
                    aco_dram = dramp.tile([4, N], BF16, name=f"acod{l}",
                                          tag="acod")
                    acoF_dram = dramp.tile([4, N], F32, name=f"acodF{l}",
                                           tag="acodF")
                    aorow = rowp.tile([2, N], BF16, name="aorow", tag="aorow")
                    corow = rowp.tile([2, N], BF16, name="corow", tag="corow")
                    aorowF = rowp.tile([2, N], F32, name="aorowF", tag="aorowF")
                    corowF = rowp.tile([2, N], F32, name="corowF", tag="corowF")
                    for h in range(2):
                        sl = (slice(None), slice(512 * h, 512 * (h + 1)))
                        nc.scalar.activation(aorow[sl], fops[h][:], AF.Exp,
                                             scale=0.2)
                        nc.scalar.activation(corow[sl], fops[h][:], AF.Exp,
                                             scale=1.0)
                        nc.scalar.activation(aorowF[sl], fops[h][:], AF.Exp,
                                             scale=0.2)
                        nc.scalar.activation(corowF[sl], fops[h][:], AF.Exp,
                                             scale=1.0)
                    nc.sync.dma_start(aco_dram[0:2, :], aorow[:])
                    nc.sync.dma_start(aco_dram[2:4, :], corow[:])
                    nc.sync.dma_start(acoF_dram[0:2, :], aorowF[:])
                    nc.sync.dma_start(acoF_dram[2:4, :], corowF[:])
                    aoT, coT = [], []
                    for j in range(NCH):
                        at = rowp.tile([128, 1], F32, name=f"aoT_{j}",
                                       tag=f"aoT{j}")
                        ct = rowp.tile([128, 1], F32, name=f"coT_{j}",
                                       tag=f"coT{j}")
                        nc.sync.dma_start_transpose(
                            at[:], acoF_dram[1:2, 128 * j:128 * (j + 1)])
                        nc.sync.dma_start_transpose(
                            ct[:], acoF_dram[3:4, 128 * j:128 * (j + 1)])
                        aoT.append(at)
                        coT.append(ct)

                    brep = repp.tile([128, N], BF16, name="brep", tag="brep",
                                     bufs=3)
                    drep = repp.tile([128, N], BF16, name="drep", tag="drep",
                                     bufs=3)
                    nc.sync.dma_start(brep[:],
                                      aco_dram[0:1, :].partition_broadcast(128))
                    nc.sync.dma_start(drep[:],
                                      aco_dram[2:3, :].partition_broadcast(128))
                    sps_o = [gpsa.tile([1, 512], F32, name=f"spso{h}", tag="f_ps")
                             for h in range(2)]
                    qmap = []
                    for j in range(NCH):
                        q = make_q(j, brep, drep, aoT[j][:, 0:1], coT[j][:, 0:1],
                                   qmp, f"qm{j}")
                        qmap.append(q)
                        for h in range(2):
                            nc.tensor.matmul(sps_o[h][:],
                                             whno[j][:, 512:513],
                                             q[:, 512 * h:512 * (h + 1)],
                                             start=(j == 0), stop=(j == NCH - 1))
                    for h in range(2):
                        nc.scalar.copy(srow_scr[0:1, 512 * h:512 * (h + 1)],
                                       sps_o[h][:])
                    so_dram = dramp.tile([1, N], F32, name=f"sod{l}", tag="sod")
                    nc.sync.dma_start(so_dram[:], srow_scr[0:1, :])
                    so128 = smallp.tile([128, 8], F32, name="so128", tag="so128")
                    nc.sync.dma_start(
                        so128[:], so_dram[:].rearrange("a (p c) -> (a p) c", c=8))
                    nc.vector.reciprocal(so128[:], so128[:])
                    nc.sync.dma_start(
                        so_dram[:].rearrange("a (p c) -> (a p) c", c=8), so128[:])
                    sorep = repp.tile([128, N], F32, name="sorep", tag="sirep")
                    nc.sync.dma_start(sorep[:],
                                      so_dram[:].partition_broadcast(128))
                    gout = []
                    for m in range(4):
                        ops = gpsa.tile([128, N], F32, name=f"ops_{m}",
                                        tag="att_ps")
                        for j in range(NCH):
                            for h in range(2):
                                nc.tensor.matmul(
                                    ops[:, 512 * h:512 * (h + 1)],
                                    whno[j][:, 128 * m:128 * (m + 1)],
                                    qmap[j][:, 512 * h:512 * (h + 1)],
                                    start=(j == 0), stop=(j == NCH - 1))
                        g = px.tile([128, N], BF16, name=f"gout{l}_{m}",
                                    tag=f"{gtag}{m}")
                        nc.scalar.copy(g[:], ops[:])
                        nc.gpsimd.tensor_mul(g[:], g[:], sorep[:])
                        ex = uvp.tile([128, N], BF16, name="elu_o", tag="u_t")
                        nc.scalar.activation(ex[:], g[:], AF.Exp)
                        nc.vector.tensor_scalar_add(ex[:], ex[:], -1.0)
                        nc.vector.tensor_scalar_min(ex[:], ex[:], 0.0)
                        nc.vector.tensor_max(g[:], g[:], ex[:])
                        gout.append(g)
                    return gout

            g1 = gat_layer(1, x_fm, "goutA")
            g2 = gat_layer(2, g1, "goutB")

            # ============ MLPs + payload (feature-major) ============
            with tc.tile_pool(name="mw", bufs=1) as mw, \
                 tc.tile_pool(name="mps", bufs=4, space="PSUM") as mps:

                def loadw(name, n_out, k):
                    return load(name, [128, n_out], part=(128 * k, 128 * (k + 1)),
                                tag=f"mlpw{k}", pool=mw)

                def loadb(name, m):
                    return load(name, [128, 1], part=(128 * m, 128 * (m + 1)),
                                tag=f"mlpb{m % 4}_{name}", pool=mw)

                def mlp(x_in, wname, bname, n_out, xtag, pool):
                    wv = [loadw(wname, n_out, k) for k in range(4)]
                    bv = [loadb(bname, m) for m in range(n_out // 128)]
                    out = []
                    for m in range(n_out // 128):
                        t = pool.tile([128, N], BF16, name=f"o_{wname}_{m}",
                                      tag=f"{xtag}{m}")
                        for h in range(2):
                            ps = mps.tile([128, 512], F32,
                                          name=f"mp{wname}{m}{h}", tag="mm_ps")
                            for k in range(4):
                                nc.tensor.matmul(
                                    ps[:], wv[k][:, 128 * m:128 * (m + 1)],
                                    x_in[k][:, 512 * h:512 * (h + 1)],
                                    start=(k == 0), stop=(k == 3))
                            nc.scalar.activation(t[:, 512 * h:512 * (h + 1)],
                                                 ps[:], AF.Relu, bias=bv[m][:])
                        out.append(t)
                    return out

                tr = mlp(g2, "tw", "tb", NF, "goutA", px)
                f1o = mlp(tr, "f1w", "f1b", NF, "hev", evp)
                l1o_tiles = []
                for m in range(4):
                    wv = [loadw("l1w", NF, k) for k in range(4)]
                    bv = loadb("l1b", m)
                    t = evp.tile([128, N], BF16, name=f"o_l1w_{m}",
                                 tag=f"hev{4 + m}")
                    for h in range(2):
                        ps = mps.tile([128, 512], F32, name=f"mpl1{m}{h}",
                                      tag="mm_ps")
                        for k in range(4):
                            nc.tensor.matmul(
                                ps[:], wv[k][:, 128 * m:128 * (m + 1)],
                                tr[k][:, 512 * h:512 * (h + 1)],
                                start=(k == 0), stop=(k == 3))
                        nc.scalar.activation(t[:, 512 * h:512 * (h + 1)],
                                             ps[:], AF.Relu, bias=bv[:])
                    l1o_tiles.append(t)
                l1o = l1o_tiles

                f2wv = [loadw("f2w", NOUT, k) for k in range(4)]
                f2bv = [loadb("f2b", m) for m in range(8)]
                l2wv = [load("l2w", [128, NOUT], part=(128 * k, 128 * (k + 1)),
                             tag=f"mlpw2{k}", pool=mw) for k in range(4)]
                l2bv = [load("l2b", [128, 1], part=(128 * m, 128 * (m + 1)),
                             tag=f"mlpb2{m}", pool=mw) for m in range(8)]

                pay = [smallp.tile([128, 5], F32, name=f"pay{j}", tag=f"pay{j}",
                       bufs=1) for j in range(NCH)]
                fej = px.tile([128, N], BF16, name="fej", tag="fej")
                lej = px.tile([128, N], BF16, name="lej", tag="lej")
                nfs = px.tile([128, N], BF16, name="nf_scr", tag="nf_scr")
                for j in range(NCH):
                    for (t, wv, bv, xi) in ((fej, f2wv, f2bv, f1o),
                                            (lej, l2wv, l2bv, l1o)):
                        for h in range(2):
                            ps = mps.tile([128, 512], F32, name=f"nfp{j}{h}",
                                          tag="mm_ps")
                            for k in range(4):
                                nc.tensor.matmul(
                                    ps[:], wv[k][:, 128 * j:128 * (j + 1)],
                                    xi[k][:, 512 * h:512 * (h + 1)],
                                    start=(k == 0), stop=(k == 3))
                            nc.scalar.activation(t[:, 512 * h:512 * (h + 1)],
                                                 ps[:], AF.Relu, bias=bv[j][:])
                    nc.vector.tensor_mul(nfs[:], fej[:], lej[:])
                    nc.scalar.activation(nfs[:], nfs[:], AF.Identity,
                                         accum_out=pay[j][:, 0:1])
                    e1 = smallp.tile([128, 2], F32, name="edge1", tag="edge1")
                    e2 = smallp.tile([128, 2], F32, name="edge2", tag="edge2")
                    nc.vector.tensor_mul(e1[:], fej[:, 0:2], lej[:, 0:2])
                    nc.vector.tensor_mul(e2[:], fej[:, 1022:1024],
                                         lej[:, 1022:1024])
                    nc.vector.tensor_mul(pay[j][:, 1:3], e1[:], maskrep[:, 0:2])
                    nc.vector.tensor_mul(pay[j][:, 3:5], e2[:], maskrep[:, 2:4])

                pay_in = dramp.tile([N, 5], F32, name="pay_in")
                pay_out = dramp.tile([N, 5], F32, name="pay_out")
                for j in range(NCH):
                    nc.sync.dma_start(pay_in[128 * j:128 * (j + 1), :], pay[j][:])
                nc.gpsimd.collective_compute(
                    "AllReduce", AL.add, replica_groups=[list(range(8))],
                    ins=[pay_in.opt()], outs=[pay_out.opt()])

                # s vectors + TCN matvec (k2t loaded into freed adj slots)
                k2 = [load("k2t", [128, 256], part=(128 * c, 128 * (c + 1)),
                           tag=f"t_adjb_({128 * (c % 8)}, {128 * (c % 8 + 1)})")
                      for c in range(24)]
                yps = mps.tile([1, 256], F32, name="yps", tag="yps")
                sfls = []
                for j in range(NCH):
                    red = smallp.tile([128, 5], F32, name=f"red{j}",
                                      tag=f"pay{j}", bufs=1)
                    nc.sync.dma_start(red[:], pay_out[128 * j:128 * (j + 1), :])
                    sfl = smallp.tile([128, 3], F32, name=f"sfl{j}",
                                      tag=f"sfl{j}", bufs=1)
                    t01 = smallp.tile([128, 1], F32, name=f"t01_{j}", tag="t01")
                    nc.vector.tensor_sub(sfl[:, 0:1], red[:, 0:1], red[:, 3:4])
                    nc.vector.tensor_sub(sfl[:, 0:1], sfl[:, 0:1], red[:, 4:5])
                    nc.vector.tensor_sub(t01[:], red[:, 0:1], red[:, 1:2])
                    nc.vector.tensor_sub(sfl[:, 1:2], t01[:], red[:, 4:5])
                    nc.vector.tensor_sub(sfl[:, 2:3], t01[:], red[:, 2:3])
                    sfls.append(sfl)
                for k in range(3):
                    for j in range(NCH):
                        ch = k * 8 + j
                        nc.tensor.matmul(yps[:], sfls[j][:, k:k + 1], k2[ch][:],
                                         start=(ch == 0), stop=(ch == 23))

                ysb = smallp.tile([128, 256], F32, name="ysb", tag="ysb")
                nc.vector.tensor_add(ysb[0:1, :], yps[:], tcnb[:])
                y_dram = dramp.tile([1, 256], F32, name="y_dram")
                nc.sync.dma_start(y_dram[:], ysb[0:1, :])
                yrep = smallp.tile([8, 256], F32, name="yrep", tag="yrep")
                nc.sync.dma_start(yrep[:], y_dram[:].partition_broadcast(8))
                ypad = smallp.tile([8, 256], F32, name="ypad", tag="ypad")
                nc.vector.tensor_scalar_mul(ypad[:], yrep[:], onehot[:])
                yar_in = dramp.tile([8, 256], F32, name="yar_in")
                yar_out = dramp.tile([8, 256], F32, name="yar_out")
                nc.sync.dma_start(yar_in[:], ypad[:])
                nc.gpsimd.collective_compute(
                    "AllReduce", AL.add, replica_groups=[list(range(8))],
                    ins=[yar_in.opt()], outs=[yar_out.opt()])
                yfull = smallp.tile([8, 256], F32, name="yfull", tag="yfull")
                nc.sync.dma_start(yfull[:], yar_out[:])
                ysq = smallp.tile([8, 256], F32, name="ysq", tag="ysq")
                ss8 = smallp.tile([8, 1], F32, name="ss8", tag="ss8")
                nc.scalar.activation(ysq[:], yfull[:], AF.Square,
                                     accum_out=ss8[:])
                sstot = smallp.tile([1, 1], F32, name="sstot", tag="sstot")
                nc.gpsimd.tensor_reduce(sstot[:], ss8[:],
                                        axis=mybir.AxisListType.C, op=AL.add)
                nc.scalar.activation(sstot[:], sstot[:], AF.Sqrt)
                nc.vector.reciprocal(sstot[:], sstot[:])
                invn8 = smallp.tile([8, 1], F32, name="invn8", tag="invn8")
                nc.gpsimd.partition_broadcast(invn8[:], sstot[:])
                yn = smallp.tile([8, 256], F32, name="yn", tag="yn")
                nc.scalar.activation(yn[:], yfull[:], AF.Copy, scale=invn8[:])
                nc.sync.dma_start(out_d.ap(), yn[:])

    nc.compile()
    _CACHE["nc"] = nc
    return nc


def _prep_inputs(batch_points, batch_descs, batch_adj, params):
    f32 = np.float32

    def A(x):
        return np.asarray(x, dtype=f32)

    pts = A(batch_points); descs = A(batch_descs); adjf = A(batch_adj)
    pe = params["pe"]; gcn = params["gcn"]
    eps = 1e-5
    pe_w = [A(w) for w in pe["W"]]
    pe_b = [A(b) for b in pe["b"]]
    folded = []
    for i in range(3):
        W, b = pe_w[i], pe_b[i]
        if i < 2:
            g, bt, m, v = [A(t) for t in pe["bn"][i]]
            gp = g / np.sqrt(v + eps)
            W = W * gp[None, :]
            b = b * gp + (bt - m * gp)
        folded.append((W, b))

    shared = {
        "pw1": folded[0][0], "pb1": folded[0][1][:, None],
        "pw2": folded[1][0], "pb2": folded[1][1][:, None],
        "pw3": folded[2][0], "pb3": folded[2][1][:, None],
    }
    for l, gk in ((1, "gat1"), (2, "gat2")):
        g = gcn[gk]
        W = A(g["W"]); a = A(g["a"]); Wo = A(g["Wo"]); ao = A(g["ao"])
        wall = np.transpose(W, (1, 0, 2)).reshape(NF, NF)
        ahat = np.zeros((NF, 16), f32)
        for h in range(NH):
            ahat[:, 2 * h] = W[h] @ a[h][:NHID]
            ahat[:, 2 * h + 1] = W[h] @ a[h][NHID:]
        aohat = np.stack([Wo @ ao[:NF], Wo @ ao[NF:]], axis=1)
        shared[f"wall{l}"] = wall.astype(BF)
        shared[f"ahat{l}"] = ahat.astype(BF)
        shared[f"wo{l}"] = Wo.astype(BF)
        shared[f"aohat{l}"] = aohat.astype(BF)
    for nm, wk, bk in (("tw", "tran1_W", "tran1_b"), ("f1", "fe1_W", "fe1_b"),
                      ("f2", "fe2_W", "fe2_b"), ("l1", "le1_W", "le1_b"),
                      ("l2", "le2_W", "le2_b")):
        wn = nm if nm == "tw" else nm + "w"
        bn = "tb" if nm == "tw" else nm + "b"
        shared[wn] = A(gcn[wk]).astype(BF)
        shared[bn] = A(gcn[bk])[:, None]

    K2 = np.transpose(A(params["tcn_K"]), (0, 2, 1)).reshape(TCN_OUT, 3 * NOUT)
    K2T = np.ascontiguousarray((K2 / float(L_OUT)).T)
    tcn_b = A(params["tcn_b"])

    in_maps = []
    for c in range(8):
        m = dict(shared)
        m["ptsT"] = np.ascontiguousarray(pts[c].T)
        m["descsT"] = np.ascontiguousarray(descs[c].T).astype(BF)
        m["adjb"] = adjf[c].astype(BF)
        em = np.zeros((1, 4), f32)
        if c == 0:
            em[0, 0] = em[0, 1] = 1.0
        if c == 7:
            em[0, 2] = em[0, 3] = 1.0
        m["edgemask"] = em
        oh = np.zeros((8, 1), f32); oh[c, 0] = 1.0
        m["onehot"] = oh
        m["k2t"] = np.ascontiguousarray(K2T[:, 256 * c:256 * (c + 1)])
        m["tcnb"] = np.ascontiguousarray(tcn_b[256 * c:256 * (c + 1)])[None, :]
        for k in list(m):
            if m[k].dtype == np.float64:
                m[k] = m[k].astype(np.float32)
        in_maps.append(m)
    return in_maps


def kernel(batch_points, batch_descs, batch_adj, params, _trace=False):
    nc = _build()
    in_maps = _prep_inputs(batch_points, batch_descs, batch_adj, params)
    res = run_bass_kernel_spmd(nc, in_maps, core_ids=list(range(8)),
                               trace=_trace)
    kernel.last_result = res
    return res.results[0]["out"].reshape(1, TCN_OUT).astype(np.float32)


# revision 17
# speedup vs baseline: 1.3175x; 1.0116x over previous
"""AirObject GNN kernel for 8 Trainium2 NeuronCores (Bass/Tile).

Data-parallel over the T=8 graphs (one graph per core). Feature-major
activations. Attention uses the exact identity
  exp(leaky_0.2(e)) = max(exp(0.2 e), exp(e)),  e[j,i] = f1[i]+f2[j]
which is rank-1 in exp space: Q = adj * max(a_j*b_i, c_j*d_i); per map
only 2 tensor_scalar + 2 tensor_tensor passes, spread over DVE/ACT/GPS.
Softmax denominator rides the PE matmul as a ones-column. The TCN+mean
collapses exactly to a matvec against sliding column-sums, so only
[1024,5] column-sum/edge data crosses cores (AllReduce), each core does
a 256-row slice of the final matvec, AllReduce-gathers y, l2norms.
"""
import numpy as np
import ml_dtypes

import concourse.bacc as bacc
import concourse.tile as tile
import concourse.mybir as mybir
from concourse.bass_utils import run_bass_kernel_spmd

F32 = mybir.dt.float32
BF16 = mybir.dt.bfloat16
AL = mybir.AluOpType
AF = mybir.ActivationFunctionType

T, N = 8, 1024
NF = 512
NHID, NH = 64, 8
NOUT = 1024
TCN_OUT, TCN_K = 2048, 3
L_OUT = T * N - TCN_K + 1
NCH = N // 128
BF = ml_dtypes.bfloat16

_CACHE = {}


def _engine_seq():
    pat = ["DVE", "ACT"]
    i = 0
    while True:
        yield pat[i % len(pat)]
        i += 1


def _build():
    if "nc" in _CACHE:
        return _CACHE["nc"]
    nc = bacc.Bacc("TRN2", debug=False, num_devices=8)
    D = {}

    def din(name, shape, dt):
        D[name] = nc.dram_tensor(name, shape, dt, kind="ExternalInput")

    din("ptsT", [2, N], F32)
    din("descsT", [256, N], BF16)
    din("adjb", [N, N], BF16)
    din("edgemask", [1, 4], F32)
    din("onehot", [8, 1], F32)
    din("k2t", [3 * NOUT, 256], F32)
    din("tcnb", [1, 256], F32)
    din("pw1", [2, 64], F32); din("pb1", [64, 1], F32)
    din("pw2", [64, 128], F32); din("pb2", [128, 1], F32)
    din("pw3", [128, 256], F32); din("pb3", [256, 1], F32)
    for l in (1, 2):
        din(f"wall{l}", [NF, NF], BF16)
        din(f"ahat{l}", [NF, 16], BF16)
        din(f"wo{l}", [NF, NF], BF16)
        din(f"aohat{l}", [NF, 2], BF16)
    din("tw", [NF, NF], BF16); din("tb", [NF, 1], F32)
    din("f1w", [NF, NF], BF16); din("f1b", [NF, 1], F32)
    din("f2w", [NF, NOUT], BF16); din("f2b", [NOUT, 1], F32)
    din("l1w", [NF, NF], BF16); din("l1b", [NF, 1], F32)
    din("l2w", [NF, NOUT], BF16); din("l2b", [NOUT, 1], F32)
    out_d = nc.dram_tensor("out", [8, 256], F32, kind="ExternalOutput")

    eng = _engine_seq()

    with tile.TileContext(nc) as tc:
        ctx_outer = [
            tc.tile_pool(name="pw", bufs=1),      # persistent inputs/weights
            tc.tile_pool(name="px", bufs=1),      # persistent activations
            tc.tile_pool(name="uvp", bufs=3),     # u/v/elu scratch
            tc.tile_pool(name="qp", bufs=3),      # q rotating (heads)
            tc.tile_pool(name="qmp", bufs=1),     # q persistent (out-map)
            tc.tile_pool(name="repp", bufs=2),    # broadcast rows
            tc.tile_pool(name="evp", bufs=1),     # whn/whno/xcat evacs
            tc.tile_pool(name="smallp", bufs=2),  # small scratch
            tc.tile_pool(name="rowp", bufs=1),    # f32 exp rows
            tc.tile_pool(name="dramp", bufs=1, space="DRAM"),
        ]
        import contextlib
        with contextlib.ExitStack() as ST:
            pw, px, uvp, qp, qmp, repp, evp, smallp, rowp, dramp = [
                ST.enter_context(c) for c in ctx_outer]

            _ldq = [nc.sync, nc.scalar]
            _ldi = [0]

            def load(name, shape=None, dt=None, part=None, tag=None, pool=pw):
                h = D[name]
                shape = shape or list(h.shape)
                t = pool.tile(shape, dt or h.dtype, name=f"t_{name}_{part}",
                              tag=tag or f"t_{name}_{part}")
                src = h.ap()
                if part is not None:
                    src = src[part[0]:part[1], :]
                _ldq[_ldi[0] % 2].dma_start(t[:], src)
                _ldi[0] += 1
                return t

            adj = [load("adjb", [128, N], part=(128 * j, 128 * (j + 1)))
                   for j in range(NCH)]
            x_fm = [load("descsT", [128, N], part=(128 * j, 128 * (j + 1)))
                    for j in range(2)]
            tcnb = load("tcnb")
            onehot = load("onehot")
            maskrep = pw.tile([128, 4], F32, name="maskrep")
            nc.sync.dma_start(maskrep[:], D["edgemask"].ap().partition_broadcast(128))
            ones1 = pw.tile([128, 1], F32, name="ones1")
            nc.vector.memset(ones1[:], 1.0)
            srow_scr = px.tile([128, N], F32, name="srow_scr")

            # ============ points encoder (fp32, feature-major) ============
            with tc.tile_pool(name="pes", bufs=1) as pes, \
                 tc.tile_pool(name="peps", bufs=2, space="PSUM") as peps:
                ptsT = load("ptsT", pool=pes)
                pw1 = load("pw1", pool=pes); pb1 = load("pb1", pool=pes)
                pw2 = load("pw2", pool=pes); pb2 = load("pb2", pool=pes)
                pw3 = load("pw3", pool=pes)
                pb3 = [load("pb3", [128, 1], part=(128 * j, 128 * (j + 1)),
                            pool=pes) for j in range(2)]
                o1 = pes.tile([64, N], F32, name="pe_o1")
                for h in range(2):
                    ps = peps.tile([64, 512], F32, name=f"pe1_{h}", tag="pe_ps")
                    nc.tensor.matmul(ps[:], pw1[:], ptsT[:, 512 * h:512 * (h + 1)],
                                     start=True, stop=True)
                    nc.scalar.activation(o1[:, 512 * h:512 * (h + 1)], ps[:],
                                         AF.Relu, bias=pb1[0:64, :])
                o2 = pes.tile([128, N], F32, name="pe_o2")
                for h in range(2):
                    ps = peps.tile([128, 512], F32, name=f"pe2_{h}", tag="pe_ps")
                    nc.tensor.matmul(ps[:], pw2[:], o1[:, 512 * h:512 * (h + 1)],
                                     start=True, stop=True)
                    nc.scalar.activation(o2[:, 512 * h:512 * (h + 1)], ps[:],
                                         AF.Relu, bias=pb2[:])
                ep = [pes.tile([128, N], F32, name=f"pe_ep{c}") for c in range(2)]
                epsq = pes.tile([128, N], F32, name="pe_sq", tag="pe_o1")
                sq_ps = [peps.tile([1, 512], F32, name=f"ssq{h}", tag="ssq_ps")
                         for h in range(2)]
                for c in range(2):
                    for h in range(2):
                        ps = peps.tile([128, 512], F32, name=f"pe3_{c}{h}",
                                       tag="pe_ps")
                        nc.tensor.matmul(ps[:], pw3[:, 128 * c:128 * (c + 1)],
                                         o2[:, 512 * h:512 * (h + 1)],
                                         start=True, stop=True)
                        sl = (slice(None), slice(512 * h, 512 * (h + 1)))
                        nc.scalar.activation(ep[c][sl], ps[:], AF.Identity,
                                             bias=pb3[c][:])
                        nc.scalar.activation(epsq[sl], ps[:], AF.Square,
                                             bias=pb3[c][:])
                        nc.tensor.matmul(sq_ps[h][:], ones1[:], epsq[sl],
                                         start=(c == 0), stop=(c == 1))
                for h in range(2):
                    nc.scalar.copy(srow_scr[0:1, 512 * h:512 * (h + 1)],
                                   sq_ps[h][:])
                ssq_dram = dramp.tile([1, N], F32, name="ssq_dram")
                nc.sync.dma_start(ssq_dram[:], srow_scr[0:1, :])
                ssq128 = pes.tile([128, 8], F32, name="ssq128")
                nc.sync.dma_start(ssq128[:],
                                  ssq_dram[:].rearrange("a (p c) -> (a p) c", c=8))
                nc.scalar.activation(ssq128[:], ssq128[:], AF.Sqrt)
                nc.vector.reciprocal(ssq128[:], ssq128[:])
                inv_dram = dramp.tile([1, N], F32, name="inv_dram")
                nc.sync.dma_start(inv_dram[:].rearrange("a (p c) -> (a p) c", c=8),
                                  ssq128[:])
                invrep = pes.tile([128, N], F32, name="invrep", tag="pe_o2")
                nc.sync.dma_start(invrep[:], inv_dram[:].partition_broadcast(128))
                for c in range(2):
                    epn = px.tile([128, N], BF16, name=f"x_ep{c}")
                    nc.vector.tensor_mul(epn[:], ep[c][:], invrep[:])
                    x_fm.append(epn)

            # ===================== GAT layers =====================
            def gat_layer(l, x_in, gtag):
                with tc.tile_pool(name=f"gw{l}", bufs=1) as gw, \
                     tc.tile_pool(name=f"gps{l}", bufs=2, space="PSUM") as gpsm, \
                     tc.tile_pool(name=f"gpsa{l}", bufs=2, space="PSUM") as gpsa:
                    wall = [load(f"wall{l}", [128, NF],
                                 part=(128 * k, 128 * (k + 1)),
                                 tag=f"wall{k}", pool=gw) for k in range(4)]
                    ahat = [load(f"ahat{l}", [128, 16],
                                 part=(128 * k, 128 * (k + 1)),
                                 tag=f"ahat{k}", pool=gw) for k in range(4)]
                    wo = [load(f"wo{l}", [64, NF], part=(64 * k, 64 * (k + 1)),
                               tag=f"wo{k}", pool=gw) for k in range(8)]
                    aoh = [load(f"aohat{l}", [64, 2], part=(64 * k, 64 * (k + 1)),
                                tag=f"aoh{k}", pool=gw) for k in range(8)]

                    # Wh node-major, [Wh_h | ones] 65-stride interleave
                    whn = []
                    for j in range(NCH):
                        t = evp.tile([128, 65 * NH], BF16, name=f"whn_{j}",
                                     tag=f"whn{j}")
                        ps = gpsm.tile([128, 512], F32, name=f"whps_{j}",
                                       tag="mm_ps")
                        for k in range(4):
                            nc.tensor.matmul(ps[:],
                                             x_in[k][:, 128 * j:128 * (j + 1)],
                                             wall[k][:], start=(k == 0),
                                             stop=(k == 3))
                        ot = t[:].rearrange("p (h c) -> p h c", c=65)
                        nc.scalar.copy(ot[:, :, 0:64],
                                       ps[:].rearrange("p (h c) -> p h c", c=64))
                        nc.vector.memset(ot[:, :, 64:65], 1.0)
                        whn.append(t)

                    # f vectors feature-major; exp rows; transposed scalars
                    fps = [gpsa.tile([16, 512], F32, name=f"fps_{h}", tag="f_ps")
                           for h in range(2)]
                    for h in range(2):
                        for k in range(4):
                            nc.tensor.matmul(fps[h][:], ahat[k][:],
                                             x_in[k][:, 512 * h:512 * (h + 1)],
                                             start=(k == 0), stop=(k == 3))
                    arow = rowp.tile([16, N], BF16, name="arow", tag="arow")
                    crow = rowp.tile([16, N], BF16, name="crow", tag="crow")
                    arowF = rowp.tile([16, N], F32, name="arowF", tag="arowF")
                    crowF = rowp.tile([16, N], F32, name="crowF", tag="crowF")
                    for h in range(2):
                        sl = (slice(None), slice(512 * h, 512 * (h + 1)))
                        nc.scalar.activation(arow[sl], fps[h][:], AF.Exp,
                                             scale=0.2)
                        nc.scalar.activation(crow[sl], fps[h][:], AF.Exp,
                                             scale=1.0)
                        nc.scalar.activation(arowF[sl], fps[h][:], AF.Exp,
                                             scale=0.2)
                        nc.scalar.activation(crowF[sl], fps[h][:], AF.Exp,
                                             scale=1.0)
                    ac_dram = dramp.tile([32, N], BF16, name=f"acd{l}", tag="acd")
                    acF_dram = dramp.tile([32, N], F32, name=f"acdF{l}",
                                          tag="acdF")
                    nc.sync.dma_start(ac_dram[0:16, :], arow[:])
                    nc.sync.dma_start(ac_dram[16:32, :], crow[:])
                    nc.sync.dma_start(acF_dram[0:16, :], arowF[:])
                    nc.sync.dma_start(acF_dram[16:32, :], crowF[:])
                    acd3 = acF_dram[:].rearrange("(g two) n -> g two n", two=2)
                    aT, cT = [], []
                    for j in range(NCH):
                        at = rowp.tile([128, 8], F32, name=f"aT_{j}", tag=f"aT{j}")
                        ct = rowp.tile([128, 8], F32, name=f"cT_{j}", tag=f"cT{j}")
                        nc.sync.dma_start_transpose(
                            at[:], acd3[0:8, 1, 128 * j:128 * (j + 1)])
                        nc.sync.dma_start_transpose(
                            ct[:], acd3[8:16, 1, 128 * j:128 * (j + 1)])
                        aT.append(at)
                        cT.append(ct)

                    def make_q(j, brep, drep, a_col, c_col, qpool, qtag):
                        u = uvp.tile([128, N], BF16, name="u_t", tag="u_t")
                        v = uvp.tile([128, N], BF16, name="v_t", tag="v_t")
                        for (tt, rep, col) in ((u, brep, a_col), (v, drep, c_col)):
                            e = next(eng)
                            if e == "ACT":
                                nc.scalar.activation(tt[:], rep[:], AF.Copy,
                                                     scale=col)
                            elif e == "GPS":
                                nc.gpsimd.tensor_scalar_mul(tt[:], rep[:], col)
                            else:
                                nc.vector.tensor_scalar_mul(tt[:], rep[:], col)
                        q = qpool.tile([128, N], BF16, name="q_t", tag=qtag)
                        nc.vector.tensor_max(q[:], u[:], v[:])
                        nc.vector.tensor_mul(q[:], q[:], adj[j][:])
                        return q

                    # ---- heads ----
                    s_dram = dramp.tile([NH, N], F32, name=f"sdram{l}",
                                        tag="sdram")
                    xcat = []
                    for hh in range(NH):
                        brep = repp.tile([128, N], BF16, name="brep", tag="brep",
                                         bufs=3)
                        drep = repp.tile([128, N], BF16, name="drep", tag="drep",
                                         bufs=3)
                        nc.sync.dma_start(
                            brep[:],
                            ac_dram[2 * hh:2 * hh + 1, :].partition_broadcast(128))
                        nc.sync.dma_start(
                            drep[:],
                            ac_dram[16 + 2 * hh:16 + 2 * hh + 1, :]
                            .partition_broadcast(128))
                        hps = gpsa.tile([65, N], F32, name=f"hps_{hh}",
                                        tag="att_ps")
                        for j in range(NCH):
                            q = make_q(j, brep, drep, aT[j][:, hh:hh + 1],
                                       cT[j][:, hh:hh + 1], qp, "q_t")
                            for h in range(2):
                                nc.tensor.matmul(
                                    hps[:, 512 * h:512 * (h + 1)],
                                    whn[j][:, 65 * hh:65 * hh + 65],
                                    q[:, 512 * h:512 * (h + 1)],
                                    start=(j == 0), stop=(j == NCH - 1))
                        nc.scalar.copy(srow_scr[64:65, :], hps[64:65, :])
                        nc.sync.dma_start(s_dram[hh:hh + 1, :],
                                          srow_scr[64:65, :])
                        ev = evp.tile([64, N], BF16, name=f"hev_{hh}",
                                      tag=f"hev{hh}")
                        nc.scalar.copy(ev[:], hps[0:64, :])
                        xcat.append(ev)
                    s128 = smallp.tile([128, 64], F32, name="s128", tag="s128")
                    nc.sync.dma_start(
                        s128[:], s_dram[:].rearrange("h (g c) -> (h g) c", c=64))
                    nc.vector.reciprocal(s128[:], s128[:])
                    nc.sync.dma_start(
                        s_dram[:].rearrange("h (g c) -> (h g) c", c=64), s128[:])
                    for hh in range(NH):
                        sirep = repp.tile([128, N], F32, name="sirep",
                                          tag="sirep")
                        nc.sync.dma_start(
                            sirep[:],
                            s_dram[hh:hh + 1, :].partition_broadcast(128))
                        xh = xcat[hh]
                        nc.gpsimd.tensor_mul(xh[:], xh[:], sirep[0:64, :])
                        ex = uvp.tile([64, N], BF16, name="elu_e", tag="u_t")
                        nc.scalar.activation(ex[:], xh[:], AF.Exp)
                        nc.vector.tensor_scalar_add(ex[:], ex[:], -1.0)
                        nc.vector.tensor_scalar_min(ex[:], ex[:], 0.0)
                        nc.vector.tensor_max(xh[:], xh[:], ex[:])

                    # ---- output attention layer ----
                    whno = []
                    for j in range(NCH):
                        t = evp.tile([128, 513], BF16, name=f"whno_{j}",
                                     tag=f"whno{j}")
                        ps = gpsm.tile([128, 512], F32, name=f"wops_{j}",
                                       tag="mm_ps")
                        for k in range(8):
                            nc.tensor.matmul(ps[:],
                                             xcat[k][:, 128 * j:128 * (j + 1)],
                                             wo[k][:], start=(k == 0),
                                             stop=(k == 7))
                        nc.scalar.copy(t[:, 0:512], ps[:])
                        nc.vector.memset(t[:, 512:513], 1.0)
                        whno.append(t)
                    fops = [gpsa.tile([2, 512], F32, name=f"fo_{h}", tag="f_ps")
                            for h in range(2)]
                    for h in range(2):
                        for k in range(8):
                            nc.tensor.matmul(fops[h][:], aoh[k][:],
                                             xcat[k][:, 512 * h:512 * (h + 1)],
                                             start=(k == 0), stop=(k == 7))
                    aco_dram = dramp.tile([4, N], BF16, name=f"acod{l}",
                                          tag="acod")
                    acoF_dram = dramp.tile([4, N], F32, name=f"acodF{l}",
                                           tag="acodF")
                    aorow = rowp.tile([2, N], BF16, name="aorow", tag="aorow")
                    corow = rowp.tile([2, N], BF16, name="corow", tag="corow")
                    aorowF = rowp.tile([2, N], F32, name="aorowF", tag="aorowF")
                    corowF = rowp.tile([2, N], F32, name="corowF", tag="corowF")
                    for h in range(2):
                        sl = (slice(None), slice(512 * h, 512 * (h + 1)))
                        nc.scalar.activation(aorow[sl], fops[h][:], AF.Exp,
                                             scale=0.2)
                        nc.scalar.activation(corow[sl], fops[h][:], AF.Exp,
                                             scale=1.0)
                        nc.scalar.activation(aorowF[sl], fops[h][:], AF.Exp,
                                             scale=0.2)
                        nc.scalar.activation(corowF[sl], fops[h][:], AF.Exp,
                                             scale=1.0)
                    nc.sync.dma_start(aco_dram[0:2, :], aorow[:])
                    nc.sync.dma_start(aco_dram[2:4, :], corow[:])
                    nc.sync.dma_start(acoF_dram[0:2, :], aorowF[:])
                    nc.sync.dma_start(acoF_dram[2:4, :], corowF[:])
                    aoT, coT = [], []
                    for j in range(NCH):
                        at = rowp.tile([128, 1], F32, name=f"aoT_{j}",
                                       tag=f"aoT{j}")
                        ct = rowp.tile([128, 1], F32, name=f"coT_{j}",
                                       tag=f"coT{j}")
                        nc.sync.dma_start_transpose(
                            at[:], acoF_dram[1:2, 128 * j:128 * (j + 1)])
                        nc.sync.dma_start_transpose(
                            ct[:], acoF_dram[3:4, 128 * j:128 * (j + 1)])
                        aoT.append(at)
                        coT.append(ct)

                    brep = repp.tile([128, N], BF16, name="brep", tag="brep",
                                     bufs=3)
                    drep = repp.tile([128, N], BF16, name="drep", tag="drep",
                                     bufs=3)
                    nc.sync.dma_start(brep[:],
                                      aco_dram[0:1, :].partition_broadcast(128))
                    nc.sync.dma_start(drep[:],
                                      aco_dram[2:3, :].partition_broadcast(128))
                    sps_o = [gpsa.tile([1, 512], F32, name=f"spso{h}", tag="f_ps")
                             for h in range(2)]
                    qmap = []
                    for j in range(NCH):
                        q = make_q(j, brep, drep, aoT[j][:, 0:1], coT[j][:, 0:1],
                                   qmp, f"qm{j}")
                        qmap.append(q)
                        for h in range(2):
                            nc.tensor.matmul(sps_o[h][:],
                                             whno[j][:, 512:513],
                                             q[:, 512 * h:512 * (h + 1)],
                                             start=(j == 0), stop=(j == NCH - 1))
                    for h in range(2):
                        nc.scalar.copy(srow_scr[0:1, 512 * h:512 * (h + 1)],
                                       sps_o[h][:])
                    so_dram = dramp.tile([1, N], F32, name=f"sod{l}", tag="sod")
                    nc.sync.dma_start(so_dram[:], srow_scr[0:1, :])
                    so128 = smallp.tile([128, 8], F32, name="so128", tag="so128")
                    nc.sync.dma_start(
                        so128[:], so_dram[:].rearrange("a (p c) -> (a p) c", c=8))
                    nc.vector.reciprocal(so128[:], so128[:])
                    nc.sync.dma_start(
                        so_dram[:].rearrange("a (p c) -> (a p) c", c=8), so128[:])
                    sorep = repp.tile([128, N], F32, name="sorep", tag="sirep")
                    nc.sync.dma_start(sorep[:],
                                      so_dram[:].partition_broadcast(128))
                    gout = []
                    ops_t = {}
                    for grp in range(2):
                        for m in (2 * grp, 2 * grp + 1):
                            ops_t[m] = gpsa.tile([128, N], F32, name=f"ops_{m}",
                                                 tag="att_ps")
                        for j in range(NCH):
                            for m in (2 * grp, 2 * grp + 1):
                                for h in range(2):
                                    nc.tensor.matmul(
                                        ops_t[m][:, 512 * h:512 * (h + 1)],
                                        whno[j][:, 128 * m:128 * (m + 1)],
                                        qmap[j][:, 512 * h:512 * (h + 1)],
                                        start=(j == 0), stop=(j == NCH - 1))
                    for m in range(4):
                        ops = ops_t[m]
                        g = px.tile([128, N], BF16, name=f"gout{l}_{m}",
                                    tag=f"{gtag}{m}")
                        nc.scalar.copy(g[:], ops[:])
                        nc.gpsimd.tensor_mul(g[:], g[:], sorep[:])
                        ex = uvp.tile([128, N], BF16, name="elu_o", tag="u_t")
                        nc.scalar.activation(ex[:], g[:], AF.Exp)
                        nc.vector.tensor_scalar_add(ex[:], ex[:], -1.0)
                        nc.vector.tensor_scalar_min(ex[:], ex[:], 0.0)
                        nc.vector.tensor_max(g[:], g[:], ex[:])
                        gout.append(g)
                    return gout

            g1 = gat_layer(1, x_fm, "goutA")
            g2 = gat_layer(2, g1, "goutB")

            # ============ MLPs + payload (feature-major) ============
            with tc.tile_pool(name="mw", bufs=1) as mw, \
                 tc.tile_pool(name="mps", bufs=4, space="PSUM") as mps:

                def loadw(name, n_out, k):
                    return load(name, [128, n_out], part=(128 * k, 128 * (k + 1)),
                                tag=f"mlpw{k}", pool=mw)

                def loadb(name, m):
                    return load(name, [128, 1], part=(128 * m, 128 * (m + 1)),
                                tag=f"mlpb{m % 4}_{name}", pool=mw)

                def mlp(x_in, wname, bname, n_out, xtag, pool):
                    wv = [loadw(wname, n_out, k) for k in range(4)]
                    bv = [loadb(bname, m) for m in range(n_out // 128)]
                    out = []
                    for m in range(n_out // 128):
                        t = pool.tile([128, N], BF16, name=f"o_{wname}_{m}",
                                      tag=f"{xtag}{m}")
                        for h in range(2):
                            ps = mps.tile([128, 512], F32,
                                          name=f"mp{wname}{m}{h}", tag="mm_ps")
                            for k in range(4):
                                nc.tensor.matmul(
                                    ps[:], wv[k][:, 128 * m:128 * (m + 1)],
                                    x_in[k][:, 512 * h:512 * (h + 1)],
                                    start=(k == 0), stop=(k == 3))
                            nc.scalar.activation(t[:, 512 * h:512 * (h + 1)],
                                                 ps[:], AF.Relu, bias=bv[m][:])
                        out.append(t)
                    return out

                tr = mlp(g2, "tw", "tb", NF, "goutA", px)
                f1o = mlp(tr, "f1w", "f1b", NF, "hev", evp)
                l1o_tiles = []
                for m in range(4):
                    wv = [loadw("l1w", NF, k) for k in range(4)]
                    bv = loadb("l1b", m)
                    t = evp.tile([128, N], BF16, name=f"o_l1w_{m}",
                                 tag=f"hev{4 + m}")
                    for h in range(2):
                        ps = mps.tile([128, 512], F32, name=f"mpl1{m}{h}",
                                      tag="mm_ps")
                        for k in range(4):
                            nc.tensor.matmul(
                                ps[:], wv[k][:, 128 * m:128 * (m + 1)],
                                tr[k][:, 512 * h:512 * (h + 1)],
                                start=(k == 0), stop=(k == 3))
                        nc.scalar.activation(t[:, 512 * h:512 * (h + 1)],
                                             ps[:], AF.Relu, bias=bv[:])
                    l1o_tiles.append(t)
                l1o = l1o_tiles

                f2wv = [loadw("f2w", NOUT, k) for k in range(4)]
                f2bv = [loadb("f2b", m) for m in range(8)]
                l2wv = [load("l2w", [128, NOUT], part=(128 * k, 128 * (k + 1)),
                             tag=f"mlpw2{k}", pool=mw) for k in range(4)]
                l2bv = [load("l2b", [128, 1], part=(128 * m, 128 * (m + 1)),
                             tag=f"mlpb2{m}", pool=mw) for m in range(8)]

                pay = [smallp.tile([128, 5], F32, name=f"pay{j}", tag=f"pay{j}",
                       bufs=1) for j in range(NCH)]
                for j in range(NCH):
                    fej = px.tile([128, N], BF16, name="fej", tag="fej", bufs=2)
                    lej = px.tile([128, N], BF16, name="lej", tag="lej", bufs=2)
                    nfs = px.tile([128, N], BF16, name="nf_scr", tag="nf_scr",
                                  bufs=2)
                    for (t, wv, bv, xi) in ((fej, f2wv, f2bv, f1o),
                                            (lej, l2wv, l2bv, l1o)):
                        for h in range(2):
                            ps = mps.tile([128, 512], F32, name=f"nfp{j}{h}",
                                          tag="mm_ps")
                            for k in range(4):
                                nc.tensor.matmul(
                                    ps[:], wv[k][:, 128 * j:128 * (j + 1)],
                                    xi[k][:, 512 * h:512 * (h + 1)],
                                    start=(k == 0), stop=(k == 3))
                            nc.scalar.activation(t[:, 512 * h:512 * (h + 1)],
                                                 ps[:], AF.Relu, bias=bv[j][:])
                    nc.vector.tensor_mul(nfs[:], fej[:], lej[:])
                    nc.scalar.activation(nfs[:], nfs[:], AF.Identity,
                                         accum_out=pay[j][:, 0:1])
                    e1 = smallp.tile([128, 2], F32, name="edge1", tag="edge1")
                    e2 = smallp.tile([128, 2], F32, name="edge2", tag="edge2")
                    nc.vector.tensor_mul(e1[:], fej[:, 0:2], lej[:, 0:2])
                    nc.vector.tensor_mul(e2[:], fej[:, 1022:1024],
                                         lej[:, 1022:1024])
                    nc.vector.tensor_mul(pay[j][:, 1:3], e1[:], maskrep[:, 0:2])
                    nc.vector.tensor_mul(pay[j][:, 3:5], e2[:], maskrep[:, 2:4])

                pay_in = dramp.tile([N, 5], F32, name="pay_in")
                pay_out = dramp.tile([N, 5], F32, name="pay_out")
                for j in range(NCH):
                    nc.sync.dma_start(pay_in[128 * j:128 * (j + 1), :], pay[j][:])
                nc.gpsimd.collective_compute(
                    "AllReduce", AL.add, replica_groups=[list(range(8))],
                    ins=[pay_in.opt()], outs=[pay_out.opt()])

                # s vectors + TCN matvec (k2t loaded into freed adj slots)
                k2 = [load("k2t", [128, 256], part=(128 * c, 128 * (c + 1)),
                           tag=f"t_adjb_({128 * (c % 8)}, {128 * (c % 8 + 1)})")
                      for c in range(24)]
                yps = mps.tile([1, 256], F32, name="yps", tag="yps")
                sfls = []
                for j in range(NCH):
                    red = smallp.tile([128, 5], F32, name=f"red{j}",
                                      tag=f"pay{j}", bufs=1)
                    nc.sync.dma_start(red[:], pay_out[128 * j:128 * (j + 1), :])
                    sfl = smallp.tile([128, 3], F32, name=f"sfl{j}",
                                      tag=f"sfl{j}", bufs=1)
                    t01 = smallp.tile([128, 1], F32, name=f"t01_{j}", tag="t01")
                    nc.vector.tensor_sub(sfl[:, 0:1], red[:, 0:1], red[:, 3:4])
                    nc.vector.tensor_sub(sfl[:, 0:1], sfl[:, 0:1], red[:, 4:5])
                    nc.vector.tensor_sub(t01[:], red[:, 0:1], red[:, 1:2])
                    nc.vector.tensor_sub(sfl[:, 1:2], t01[:], red[:, 4:5])
                    nc.vector.tensor_sub(sfl[:, 2:3], t01[:], red[:, 2:3])
                    sfls.append(sfl)
                for k in range(3):
                    for j in range(NCH):
                        ch = k * 8 + j
                        nc.tensor.matmul(yps[:], sfls[j][:, k:k + 1], k2[ch][:],
                                         start=(ch == 0), stop=(ch == 23))

                ysb = smallp.tile([128, 256], F32, name="ysb", tag="ysb")
                nc.vector.tensor_add(ysb[0:1, :], yps[:], tcnb[:])
                y_dram = dramp.tile([1, 256], F32, name="y_dram")
                nc.sync.dma_start(y_dram[:], ysb[0:1, :])
                yrep = smallp.tile([8, 256], F32, name="yrep", tag="yrep")
                nc.sync.dma_start(yrep[:], y_dram[:].partition_broadcast(8))
                ypad = smallp.tile([8, 256], F32, name="ypad", tag="ypad")
                nc.vector.tensor_scalar_mul(ypad[:], yrep[:], onehot[:])
                yar_in = dramp.tile([8, 256], F32, name="yar_in")
                yar_out = dramp.tile([8, 256], F32, name="yar_out")
                nc.sync.dma_start(yar_in[:], ypad[:])
                nc.gpsimd.collective_compute(
                    "AllReduce", AL.add, replica_groups=[list(range(8))],
                    ins=[yar_in.opt()], outs=[yar_out.opt()])
                yfull = smallp.tile([8, 256], F32, name="yfull", tag="yfull")
                nc.sync.dma_start(yfull[:], yar_out[:])
                ysq = smallp.tile([8, 256], F32, name="ysq", tag="ysq")
                ss8 = smallp.tile([8, 1], F32, name="ss8", tag="ss8")
                nc.scalar.activation(ysq[:], yfull[:], AF.Square,
                                     accum_out=ss8[:])
                sstot = smallp.tile([1, 1], F32, name="sstot", tag="sstot")
                nc.gpsimd.tensor_reduce(sstot[:], ss8[:],
                                        axis=mybir.AxisListType.C, op=AL.add)
                nc.scalar.activation(sstot[:], sstot[:], AF.Sqrt)
                nc.vector.reciprocal(sstot[:], sstot[:])
                invn8 = smallp.tile([8, 1], F32, name="invn8", tag="invn8")
                nc.gpsimd.partition_broadcast(invn8[:], sstot[:])
                yn = smallp.tile([8, 256], F32, name="yn", tag="yn")
                nc.scalar.activation(yn[:], yfull[:], AF.Copy, scale=invn8[:])
                nc.sync.dma_start(out_d.ap(), yn[:])

    nc.compile()
    _CACHE["nc"] = nc
    return nc


def _prep_inputs(batch_points, batch_descs, batch_adj, params):
    f32 = np.float32

    def A(x):
        return np.asarray(x, dtype=f32)

    pts = A(batch_points); descs = A(batch_descs); adjf = A(batch_adj)
    pe = params["pe"]; gcn = params["gcn"]
    eps = 1e-5
    pe_w = [A(w) for w in pe["W"]]
    pe_b = [A(b) for b in pe["b"]]
    folded = []
    for i in range(3):
        W, b = pe_w[i], pe_b[i]
        if i < 2:
            g, bt, m, v = [A(t) for t in pe["bn"][i]]
            gp = g / np.sqrt(v + eps)
            W = W * gp[None, :]
            b = b * gp + (bt - m * gp)
        folded.append((W, b))

    shared = {
        "pw1": folded[0][0], "pb1": folded[0][1][:, None],
        "pw2": folded[1][0], "pb2": folded[1][1][:, None],
        "pw3": folded[2][0], "pb3": folded[2][1][:, None],
    }
    for l, gk in ((1, "gat1"), (2, "gat2")):
        g = gcn[gk]
        W = A(g["W"]); a = A(g["a"]); Wo = A(g["Wo"]); ao = A(g["ao"])
        wall = np.transpose(W, (1, 0, 2)).reshape(NF, NF)
        ahat = np.zeros((NF, 16), f32)
        for h in range(NH):
            ahat[:, 2 * h] = W[h] @ a[h][:NHID]
            ahat[:, 2 * h + 1] = W[h] @ a[h][NHID:]
        aohat = np.stack([Wo @ ao[:NF], Wo @ ao[NF:]], axis=1)
        shared[f"wall{l}"] = wall.astype(BF)
        shared[f"ahat{l}"] = ahat.astype(BF)
        shared[f"wo{l}"] = Wo.astype(BF)
        shared[f"aohat{l}"] = aohat.astype(BF)
    for nm, wk, bk in (("tw", "tran1_W", "tran1_b"), ("f1", "fe1_W", "fe1_b"),
                      ("f2", "fe2_W", "fe2_b"), ("l1", "le1_W", "le1_b"),
                      ("l2", "le2_W", "le2_b")):
        wn = nm if nm == "tw" else nm + "w"
        bn = "tb" if nm == "tw" else nm + "b"
        shared[wn] = A(gcn[wk]).astype(BF)
        shared[bn] = A(gcn[bk])[:, None]

    K2 = np.transpose(A(params["tcn_K"]), (0, 2, 1)).reshape(TCN_OUT, 3 * NOUT)
    K2T = np.ascontiguousarray((K2 / float(L_OUT)).T)
    tcn_b = A(params["tcn_b"])

    in_maps = []
    for c in range(8):
        m = dict(shared)
        m["ptsT"] = np.ascontiguousarray(pts[c].T)
        m["descsT"] = np.ascontiguousarray(descs[c].T).astype(BF)
        m["adjb"] = adjf[c].astype(BF)
        em = np.zeros((1, 4), f32)
        if c == 0:
            em[0, 0] = em[0, 1] = 1.0
        if c == 7:
            em[0, 2] = em[0, 3] = 1.0
        m["edgemask"] = em
        oh = np.zeros((8, 1), f32); oh[c, 0] = 1.0
        m["onehot"] = oh
        m["k2t"] = np.ascontiguousarray(K2T[:, 256 * c:256 * (c + 1)])
        m["tcnb"] = np.ascontiguousarray(tcn_b[256 * c:256 * (c + 1)])[None, :]
        for k in list(m):
            if m[k].dtype == np.float64:
                m[k] = m[k].astype(np.float32)
        in_maps.append(m)
    return in_maps


def kernel(batch_points, batch_descs, batch_adj, params, _trace=False):
    nc = _build()
    in_maps = _prep_inputs(batch_points, batch_descs, batch_adj, params)
    res = run_bass_kernel_spmd(nc, in_maps, core_ids=list(range(8)),
                               trace=_trace)
    kernel.last_result = res
    return res.results[0]["out"].reshape(1, TCN_OUT).astype(np.float32)


# revision 18
# speedup vs baseline: 1.5135x; 1.1488x over previous
"""AirObject GNN kernel for 8 Trainium2 NeuronCores (Bass/Tile).

Data-parallel over the T=8 graphs (one graph per core). Feature-major
activations. Attention uses the exact identity
  exp(leaky_0.2(e)) = max(exp(0.2 e), exp(e)),  e[j,i] = f1[i]+f2[j]
which is rank-1 in exp space: Q = adj * max(a_j*b_i, c_j*d_i); per map
only 2 tensor_scalar + 2 tensor_tensor passes, spread over DVE/ACT/GPS.
Softmax denominator rides the PE matmul as a ones-column. The TCN+mean
collapses exactly to a matvec against sliding column-sums, so only
[1024,5] column-sum/edge data crosses cores (AllReduce), each core does
a 256-row slice of the final matvec, AllReduce-gathers y, l2norms.
"""
import numpy as np
import ml_dtypes

import concourse.bacc as bacc
import concourse.tile as tile
import concourse.mybir as mybir
from concourse.bass_utils import run_bass_kernel_spmd

F32 = mybir.dt.float32
BF16 = mybir.dt.bfloat16
AL = mybir.AluOpType
AF = mybir.ActivationFunctionType

T, N = 8, 1024
NF = 512
NHID, NH = 64, 8
NOUT = 1024
TCN_OUT, TCN_K = 2048, 3
L_OUT = T * N - TCN_K + 1
NCH = N // 128
BF = ml_dtypes.bfloat16

_CACHE = {}


def _engine_seq():
    pat = ["DVE", "ACT"]
    i = 0
    while True:
        yield pat[i % len(pat)]
        i += 1


def _build():
    if "nc" in _CACHE:
        return _CACHE["nc"]
    nc = bacc.Bacc("TRN2", debug=False, num_devices=8)
    D = {}

    def din(name, shape, dt):
        D[name] = nc.dram_tensor(name, shape, dt, kind="ExternalInput")

    din("ptsT", [2, N], F32)
    din("descsT", [256, N], BF16)
    din("adjb", [N, N], BF16)
    din("edgemask", [1, 4], F32)
    din("onehot", [8, 1], F32)
    din("k2t", [3 * NOUT, 256], F32)
    din("tcnb", [1, 256], F32)
    din("pw1", [2, 64], F32); din("pb1", [64, 1], F32)
    din("pw2", [64, 128], F32); din("pb2", [128, 1], F32)
    din("pw3", [128, 256], F32); din("pb3", [256, 1], F32)
    for l in (1, 2):
        din(f"wall{l}", [NF, NF], BF16)
        din(f"ahat{l}", [NF, 16], BF16)
        din(f"wo{l}", [NF, NF], BF16)
        din(f"aohat{l}", [NF, 2], BF16)
    din("tw", [NF, NF], BF16); din("tb", [NF, 1], F32)
    din("f1w", [NF, NF], BF16); din("f1b", [NF, 1], F32)
    din("f2w", [NF, NOUT], BF16); din("f2b", [NOUT, 1], F32)
    din("l1w", [NF, NF], BF16); din("l1b", [NF, 1], F32)
    din("l2w", [NF, NOUT], BF16); din("l2b", [NOUT, 1], F32)
    out_d = nc.dram_tensor("out", [8, 256], F32, kind="ExternalOutput")

    eng = _engine_seq()

    with tile.TileContext(nc) as tc:
        ctx_outer = [
            tc.tile_pool(name="pw", bufs=1),      # persistent inputs/weights
            tc.tile_pool(name="px", bufs=1),      # persistent activations
            tc.tile_pool(name="uvp", bufs=3),     # u/v/elu scratch
            tc.tile_pool(name="qp", bufs=3),      # q rotating (heads)
            tc.tile_pool(name="qmp", bufs=1),     # q persistent (out-map)
            tc.tile_pool(name="repp", bufs=2),    # broadcast rows
            tc.tile_pool(name="evp", bufs=1),     # whn/whno/xcat evacs
            tc.tile_pool(name="smallp", bufs=2),  # small scratch
            tc.tile_pool(name="rowp", bufs=1),    # f32 exp rows
            tc.tile_pool(name="dramp", bufs=1, space="DRAM"),
        ]
        import contextlib
        with contextlib.ExitStack() as ST:
            pw, px, uvp, qp, qmp, repp, evp, smallp, rowp, dramp = [
                ST.enter_context(c) for c in ctx_outer]

            _ldq = [nc.sync, nc.scalar]
            _ldi = [0]

            def load(name, shape=None, dt=None, part=None, tag=None, pool=pw):
                h = D[name]
                shape = shape or list(h.shape)
                t = pool.tile(shape, dt or h.dtype, name=f"t_{name}_{part}",
                              tag=tag or f"t_{name}_{part}")
                src = h.ap()
                if part is not None:
                    src = src[part[0]:part[1], :]
                _ldq[_ldi[0] % 2].dma_start(t[:], src)
                _ldi[0] += 1
                return t

            adj = [load("adjb", [128, N], part=(128 * j, 128 * (j + 1)))
                   for j in range(NCH)]
            x_fm = [load("descsT", [128, N], part=(128 * j, 128 * (j + 1)))
                    for j in range(2)]
            tcnb = load("tcnb")
            onehot = load("onehot")
            maskrep = pw.tile([128, 4], F32, name="maskrep")
            nc.sync.dma_start(maskrep[:], D["edgemask"].ap().partition_broadcast(128))
            ones1 = pw.tile([128, 1], F32, name="ones1")
            nc.vector.memset(ones1[:], 1.0)
            srow_scr = px.tile([128, N], F32, name="srow_scr")

            # ============ points encoder (fp32, feature-major) ============
            with tc.tile_pool(name="pes", bufs=1) as pes, \
                 tc.tile_pool(name="peps", bufs=2, space="PSUM") as peps:
                ptsT = load("ptsT", pool=pes)
                pw1 = load("pw1", pool=pes); pb1 = load("pb1", pool=pes)
                pw2 = load("pw2", pool=pes); pb2 = load("pb2", pool=pes)
                pw3 = load("pw3", pool=pes)
                pb3 = [load("pb3", [128, 1], part=(128 * j, 128 * (j + 1)),
                            pool=pes) for j in range(2)]
                o1 = pes.tile([64, N], F32, name="pe_o1")
                for h in range(2):
                    ps = peps.tile([64, 512], F32, name=f"pe1_{h}", tag="pe_ps")
                    nc.tensor.matmul(ps[:], pw1[:], ptsT[:, 512 * h:512 * (h + 1)],
                                     start=True, stop=True)
                    nc.scalar.activation(o1[:, 512 * h:512 * (h + 1)], ps[:],
                                         AF.Relu, bias=pb1[0:64, :])
                o2 = pes.tile([128, N], F32, name="pe_o2")
                for h in range(2):
                    ps = peps.tile([128, 512], F32, name=f"pe2_{h}", tag="pe_ps")
                    nc.tensor.matmul(ps[:], pw2[:], o1[:, 512 * h:512 * (h + 1)],
                                     start=True, stop=True)
                    nc.scalar.activation(o2[:, 512 * h:512 * (h + 1)], ps[:],
                                         AF.Relu, bias=pb2[:])
                ep = [pes.tile([128, N], F32, name=f"pe_ep{c}") for c in range(2)]
                epsq = pes.tile([128, N], F32, name="pe_sq", tag="pe_o1")
                sq_ps = [peps.tile([1, 512], F32, name=f"ssq{h}", tag="ssq_ps")
                         for h in range(2)]
                for c in range(2):
                    for h in range(2):
                        ps = peps.tile([128, 512], F32, name=f"pe3_{c}{h}",
                                       tag="pe_ps")
                        nc.tensor.matmul(ps[:], pw3[:, 128 * c:128 * (c + 1)],
                                         o2[:, 512 * h:512 * (h + 1)],
                                         start=True, stop=True)
                        sl = (slice(None), slice(512 * h, 512 * (h + 1)))
                        nc.scalar.activation(ep[c][sl], ps[:], AF.Identity,
                                             bias=pb3[c][:])
                        nc.scalar.activation(epsq[sl], ps[:], AF.Square,
                                             bias=pb3[c][:])
                        nc.tensor.matmul(sq_ps[h][:], ones1[:], epsq[sl],
                                         start=(c == 0), stop=(c == 1))
                for h in range(2):
                    nc.scalar.copy(srow_scr[0:1, 512 * h:512 * (h + 1)],
                                   sq_ps[h][:])
                ssq_dram = dramp.tile([1, N], F32, name="ssq_dram")
                nc.sync.dma_start(ssq_dram[:], srow_scr[0:1, :])
                ssq128 = pes.tile([128, 8], F32, name="ssq128")
                nc.sync.dma_start(ssq128[:],
                                  ssq_dram[:].rearrange("a (p c) -> (a p) c", c=8))
                nc.scalar.activation(ssq128[:], ssq128[:], AF.Sqrt)
                nc.vector.reciprocal(ssq128[:], ssq128[:])
                inv_dram = dramp.tile([1, N], F32, name="inv_dram")
                nc.sync.dma_start(inv_dram[:].rearrange("a (p c) -> (a p) c", c=8),
                                  ssq128[:])
                invrep = pes.tile([128, N], F32, name="invrep", tag="pe_o2")
                nc.sync.dma_start(invrep[:], inv_dram[:].partition_broadcast(128))
                for c in range(2):
                    epn = px.tile([128, N], BF16, name=f"x_ep{c}")
                    nc.vector.tensor_mul(epn[:], ep[c][:], invrep[:])
                    x_fm.append(epn)

            # ===================== GAT layers =====================
            def gat_layer(l, x_in, gtag):
                with tc.tile_pool(name=f"gw{l}", bufs=1) as gw, \
                     tc.tile_pool(name=f"gps{l}", bufs=2, space="PSUM") as gpsm, \
                     tc.tile_pool(name=f"gpsa{l}", bufs=2, space="PSUM") as gpsa:
                    wall = [load(f"wall{l}", [128, NF],
                                 part=(128 * k, 128 * (k + 1)),
                                 tag=f"wall{k}", pool=gw) for k in range(4)]
                    ahat = [load(f"ahat{l}", [128, 16],
                                 part=(128 * k, 128 * (k + 1)),
                                 tag=f"ahat{k}", pool=gw) for k in range(4)]
                    wo = [load(f"wo{l}", [64, NF], part=(64 * k, 64 * (k + 1)),
                               tag=f"wo{k}", pool=gw) for k in range(8)]
                    aoh = [load(f"aohat{l}", [64, 2], part=(64 * k, 64 * (k + 1)),
                                tag=f"aoh{k}", pool=gw) for k in range(8)]

                    # Wh node-major, [Wh_h | ones] 65-stride interleave
                    whn = []
                    for j in range(NCH):
                        t = evp.tile([128, 65 * NH], BF16, name=f"whn_{j}",
                                     tag=f"whn{j}")
                        ps = gpsm.tile([128, 512], F32, name=f"whps_{j}",
                                       tag="mm_ps")
                        for k in range(4):
                            nc.tensor.matmul(ps[:],
                                             x_in[k][:, 128 * j:128 * (j + 1)],
                                             wall[k][:], start=(k == 0),
                                             stop=(k == 3))
                        ot = t[:].rearrange("p (h c) -> p h c", c=65)
                        nc.scalar.copy(ot[:, :, 0:64],
                                       ps[:].rearrange("p (h c) -> p h c", c=64))
                        nc.vector.memset(ot[:, :, 64:65], 1.0)
                        whn.append(t)

                    # f vectors feature-major; exp rows; transposed scalars
                    fps = [gpsa.tile([16, 512], F32, name=f"fps_{h}", tag="f_ps")
                           for h in range(2)]
                    for h in range(2):
                        for k in range(4):
                            nc.tensor.matmul(fps[h][:], ahat[k][:],
                                             x_in[k][:, 512 * h:512 * (h + 1)],
                                             start=(k == 0), stop=(k == 3))
                    arow = rowp.tile([16, N], BF16, name="arow", tag="arow")
                    crow = rowp.tile([16, N], BF16, name="crow", tag="crow")
                    for h in range(2):
                        sl = (slice(None), slice(512 * h, 512 * (h + 1)))
                        nc.scalar.activation(arow[sl], fps[h][:], AF.Exp,
                                             scale=0.2)
                        nc.scalar.activation(crow[sl], fps[h][:], AF.Exp,
                                             scale=1.0)
                    ac_dram = dramp.tile([32, N], BF16, name=f"acd{l}", tag="acd")
                    nc.sync.dma_start(ac_dram[0:16, :], arow[:])
                    nc.sync.dma_start(ac_dram[16:32, :], crow[:])
                    aT, cT = [], []
                    for j in range(NCH):
                        fnm = gpsa.tile([128, 16], F32, name=f"fnm_{j}",
                                        tag="f_ps")
                        for k in range(4):
                            nc.tensor.matmul(fnm[:],
                                             x_in[k][:, 128 * j:128 * (j + 1)],
                                             ahat[k][:], start=(k == 0),
                                             stop=(k == 3))
                        at = rowp.tile([128, 16], F32, name=f"aT_{j}",
                                       tag=f"aT{j}")
                        ct = rowp.tile([128, 16], F32, name=f"cT_{j}",
                                       tag=f"cT{j}")
                        nc.scalar.activation(at[:], fnm[:], AF.Exp, scale=0.2)
                        nc.scalar.activation(ct[:], fnm[:], AF.Exp, scale=1.0)
                        aT.append(at)
                        cT.append(ct)

                    def make_q(j, brep, drep, a_col, c_col, qpool, qtag):
                        u = uvp.tile([128, N], BF16, name="u_t", tag="u_t")
                        v = uvp.tile([128, N], BF16, name="v_t", tag="v_t")
                        for (tt, rep, col) in ((u, brep, a_col), (v, drep, c_col)):
                            e = next(eng)
                            if e == "ACT":
                                nc.scalar.activation(tt[:], rep[:], AF.Copy,
                                                     scale=col)
                            elif e == "GPS":
                                nc.gpsimd.tensor_scalar_mul(tt[:], rep[:], col)
                            else:
                                nc.vector.tensor_scalar_mul(tt[:], rep[:], col)
                        q = qpool.tile([128, N], BF16, name="q_t", tag=qtag)
                        nc.vector.tensor_max(q[:], u[:], v[:])
                        nc.vector.tensor_mul(q[:], q[:], adj[j][:])
                        return q

                    # ---- heads ----
                    s_dram = dramp.tile([NH, N], F32, name=f"sdram{l}",
                                        tag="sdram")
                    xcat = []
                    for hh in range(NH):
                        brep = repp.tile([128, N], BF16, name="brep", tag="brep",
                                         bufs=3)
                        drep = repp.tile([128, N], BF16, name="drep", tag="drep",
                                         bufs=3)
                        nc.sync.dma_start(
                            brep[:],
                            ac_dram[2 * hh:2 * hh + 1, :].partition_broadcast(128))
                        nc.sync.dma_start(
                            drep[:],
                            ac_dram[16 + 2 * hh:16 + 2 * hh + 1, :]
                            .partition_broadcast(128))
                        hps = gpsa.tile([65, N], F32, name=f"hps_{hh}",
                                        tag="att_ps")
                        for j in range(NCH):
                            q = make_q(j, brep, drep,
                                       aT[j][:, 2 * hh + 1:2 * hh + 2],
                                       cT[j][:, 2 * hh + 1:2 * hh + 2],
                                       qp, "q_t")
                            for h in range(2):
                                nc.tensor.matmul(
                                    hps[:, 512 * h:512 * (h + 1)],
                                    whn[j][:, 65 * hh:65 * hh + 65],
                                    q[:, 512 * h:512 * (h + 1)],
                                    start=(j == 0), stop=(j == NCH - 1))
                        nc.scalar.copy(srow_scr[64:65, :], hps[64:65, :])
                        nc.sync.dma_start(s_dram[hh:hh + 1, :],
                                          srow_scr[64:65, :])
                        ev = evp.tile([64, N], BF16, name=f"hev_{hh}",
                                      tag=f"hev{hh}")
                        nc.scalar.copy(ev[:], hps[0:64, :])
                        xcat.append(ev)
                    s128 = smallp.tile([128, 64], F32, name="s128", tag="s128")
                    nc.sync.dma_start(
                        s128[:], s_dram[:].rearrange("h (g c) -> (h g) c", c=64))
                    nc.vector.reciprocal(s128[:], s128[:])
                    nc.sync.dma_start(
                        s_dram[:].rearrange("h (g c) -> (h g) c", c=64), s128[:])
                    for hh in range(NH):
                        sirep = repp.tile([128, N], F32, name="sirep",
                                          tag="sirep")
                        nc.sync.dma_start(
                            sirep[:],
                            s_dram[hh:hh + 1, :].partition_broadcast(128))
                        xh = xcat[hh]
                        nc.gpsimd.tensor_mul(xh[:], xh[:], sirep[0:64, :])
                        ex = uvp.tile([64, N], BF16, name="elu_e", tag="u_t")
                        nc.scalar.activation(ex[:], xh[:], AF.Exp)
                        nc.vector.tensor_scalar_add(ex[:], ex[:], -1.0)
                        nc.vector.tensor_scalar_min(ex[:], ex[:], 0.0)
                        nc.vector.tensor_max(xh[:], xh[:], ex[:])

                    # ---- output attention layer ----
                    whno = []
                    for j in range(NCH):
                        t = evp.tile([128, 513], BF16, name=f"whno_{j}",
                                     tag=f"whno{j}")
                        ps = gpsm.tile([128, 512], F32, name=f"wops_{j}",
                                       tag="mm_ps")
                        for k in range(8):
                            nc.tensor.matmul(ps[:],
                                             xcat[k][:, 128 * j:128 * (j + 1)],
                                             wo[k][:], start=(k == 0),
                                             stop=(k == 7))
                        nc.scalar.copy(t[:, 0:512], ps[:])
                        nc.vector.memset(t[:, 512:513], 1.0)
                        whno.append(t)
                    fops = [gpsa.tile([2, 512], F32, name=f"fo_{h}", tag="f_ps")
                            for h in range(2)]
                    for h in range(2):
                        for k in range(8):
                            nc.tensor.matmul(fops[h][:], aoh[k][:],
                                             xcat[k][:, 512 * h:512 * (h + 1)],
                                             start=(k == 0), stop=(k == 7))
                    aco_dram = dramp.tile([4, N], BF16, name=f"acod{l}",
                                          tag="acod")
                    aorow = rowp.tile([2, N], BF16, name="aorow", tag="aorow")
                    corow = rowp.tile([2, N], BF16, name="corow", tag="corow")
                    for h in range(2):
                        sl = (slice(None), slice(512 * h, 512 * (h + 1)))
                        nc.scalar.activation(aorow[sl], fops[h][:], AF.Exp,
                                             scale=0.2)
                        nc.scalar.activation(corow[sl], fops[h][:], AF.Exp,
                                             scale=1.0)
                    nc.sync.dma_start(aco_dram[0:2, :], aorow[:])
                    nc.sync.dma_start(aco_dram[2:4, :], corow[:])
                    aoT, coT = [], []
                    for j in range(NCH):
                        fonm = gpsa.tile([128, 2], F32, name=f"fonm_{j}",
                                         tag="f_ps")
                        for k in range(8):
                            nc.tensor.matmul(fonm[:],
                                             xcat[k][:, 128 * j:128 * (j + 1)],
                                             aoh[k][:], start=(k == 0),
                                             stop=(k == 7))
                        at = rowp.tile([128, 2], F32, name=f"aoT_{j}",
                                       tag=f"aoT{j}")
                        ct = rowp.tile([128, 2], F32, name=f"coT_{j}",
                                       tag=f"coT{j}")
                        nc.scalar.activation(at[:], fonm[:], AF.Exp, scale=0.2)
                        nc.scalar.activation(ct[:], fonm[:], AF.Exp, scale=1.0)
                        aoT.append(at)
                        coT.append(ct)

                    brep = repp.tile([128, N], BF16, name="brep", tag="brep",
                                     bufs=3)
                    drep = repp.tile([128, N], BF16, name="drep", tag="drep",
                                     bufs=3)
                    nc.sync.dma_start(brep[:],
                                      aco_dram[0:1, :].partition_broadcast(128))
                    nc.sync.dma_start(drep[:],
                                      aco_dram[2:3, :].partition_broadcast(128))
                    sps_o = [gpsa.tile([1, 512], F32, name=f"spso{h}", tag="f_ps")
                             for h in range(2)]
                    qmap = []
                    for j in range(NCH):
                        q = make_q(j, brep, drep, aoT[j][:, 1:2],
                                   coT[j][:, 1:2], qmp, f"qm{j}")
                        qmap.append(q)
                        for h in range(2):
                            nc.tensor.matmul(sps_o[h][:],
                                             whno[j][:, 512:513],
                                             q[:, 512 * h:512 * (h + 1)],
                                             start=(j == 0), stop=(j == NCH - 1))
                    for h in range(2):
                        nc.scalar.copy(srow_scr[0:1, 512 * h:512 * (h + 1)],
                                       sps_o[h][:])
                    so_dram = dramp.tile([1, N], F32, name=f"sod{l}", tag="sod")
                    nc.sync.dma_start(so_dram[:], srow_scr[0:1, :])
                    so128 = smallp.tile([128, 8], F32, name="so128", tag="so128")
                    nc.sync.dma_start(
                        so128[:], so_dram[:].rearrange("a (p c) -> (a p) c", c=8))
                    nc.vector.reciprocal(so128[:], so128[:])
                    nc.sync.dma_start(
                        so_dram[:].rearrange("a (p c) -> (a p) c", c=8), so128[:])
                    sorep = repp.tile([128, N], F32, name="sorep", tag="sirep")
                    nc.sync.dma_start(sorep[:],
                                      so_dram[:].partition_broadcast(128))
                    gout = []
                    ops_t = {}
                    for grp in range(2):
                        for m in (2 * grp, 2 * grp + 1):
                            ops_t[m] = gpsa.tile([128, N], F32, name=f"ops_{m}",
                                                 tag="att_ps")
                        for j in range(NCH):
                            for m in (2 * grp, 2 * grp + 1):
                                for h in range(2):
                                    nc.tensor.matmul(
                                        ops_t[m][:, 512 * h:512 * (h + 1)],
                                        whno[j][:, 128 * m:128 * (m + 1)],
                                        qmap[j][:, 512 * h:512 * (h + 1)],
                                        start=(j == 0), stop=(j == NCH - 1))
                    for m in range(4):
                        ops = ops_t[m]
                        g = px.tile([128, N], BF16, name=f"gout{l}_{m}",
                                    tag=f"{gtag}{m}")
                        nc.scalar.copy(g[:], ops[:])
                        nc.gpsimd.tensor_mul(g[:], g[:], sorep[:])
                        ex = uvp.tile([128, N], BF16, name="elu_o", tag="u_t")
                        nc.scalar.activation(ex[:], g[:], AF.Exp)
                        nc.vector.tensor_scalar_add(ex[:], ex[:], -1.0)
                        nc.vector.tensor_scalar_min(ex[:], ex[:], 0.0)
                        nc.vector.tensor_max(g[:], g[:], ex[:])
                        gout.append(g)
                    return gout

            g1 = gat_layer(1, x_fm, "goutA")
            g2 = gat_layer(2, g1, "goutB")

            # ============ MLPs + payload (feature-major) ============
            with tc.tile_pool(name="mw", bufs=1) as mw, \
                 tc.tile_pool(name="mps", bufs=4, space="PSUM") as mps:

                def loadw(name, n_out, k):
                    return load(name, [128, n_out], part=(128 * k, 128 * (k + 1)),
                                tag=f"mlpw{k}", pool=mw)

                def loadb(name, m):
                    return load(name, [128, 1], part=(128 * m, 128 * (m + 1)),
                                tag=f"mlpb{m % 4}_{name}", pool=mw)

                def mlp(x_in, wname, bname, n_out, xtag, pool):
                    wv = [loadw(wname, n_out, k) for k in range(4)]
                    bv = [loadb(bname, m) for m in range(n_out // 128)]
                    out = []
                    for m in range(n_out // 128):
                        t = pool.tile([128, N], BF16, name=f"o_{wname}_{m}",
                                      tag=f"{xtag}{m}")
                        for h in range(2):
                            ps = mps.tile([128, 512], F32,
                                          name=f"mp{wname}{m}{h}", tag="mm_ps")
                            for k in range(4):
                                nc.tensor.matmul(
                                    ps[:], wv[k][:, 128 * m:128 * (m + 1)],
                                    x_in[k][:, 512 * h:512 * (h + 1)],
                                    start=(k == 0), stop=(k == 3))
                            nc.scalar.activation(t[:, 512 * h:512 * (h + 1)],
                                                 ps[:], AF.Relu, bias=bv[m][:])
                        out.append(t)
                    return out

                tr = mlp(g2, "tw", "tb", NF, "goutA", px)
                f1o = mlp(tr, "f1w", "f1b", NF, "hev", evp)
                l1o_tiles = []
                for m in range(4):
                    wv = [loadw("l1w", NF, k) for k in range(4)]
                    bv = loadb("l1b", m)
                    t = evp.tile([128, N], BF16, name=f"o_l1w_{m}",
                                 tag=f"hev{4 + m}")
                    for h in range(2):
                        ps = mps.tile([128, 512], F32, name=f"mpl1{m}{h}",
                                      tag="mm_ps")
                        for k in range(4):
                            nc.tensor.matmul(
                                ps[:], wv[k][:, 128 * m:128 * (m + 1)],
                                tr[k][:, 512 * h:512 * (h + 1)],
                                start=(k == 0), stop=(k == 3))
                        nc.scalar.activation(t[:, 512 * h:512 * (h + 1)],
                                             ps[:], AF.Relu, bias=bv[:])
                    l1o_tiles.append(t)
                l1o = l1o_tiles

                f2wv = [loadw("f2w", NOUT, k) for k in range(4)]
                f2bv = [loadb("f2b", m) for m in range(8)]
                l2wv = [load("l2w", [128, NOUT], part=(128 * k, 128 * (k + 1)),
                             tag=f"mlpw2{k}", pool=mw) for k in range(4)]
                l2bv = [load("l2b", [128, 1], part=(128 * m, 128 * (m + 1)),
                             tag=f"mlpb2{m}", pool=mw) for m in range(8)]

                pay = [smallp.tile([128, 5], F32, name=f"pay{j}", tag=f"pay{j}",
                       bufs=1) for j in range(NCH)]
                for j in range(NCH):
                    fej = px.tile([128, N], BF16, name="fej", tag="fej", bufs=2)
                    lej = px.tile([128, N], BF16, name="lej", tag="lej", bufs=2)
                    nfs = px.tile([128, N], BF16, name="nf_scr", tag="nf_scr",
                                  bufs=2)
                    for (t, wv, bv, xi) in ((fej, f2wv, f2bv, f1o),
                                            (lej, l2wv, l2bv, l1o)):
                        for h in range(2):
                            ps = mps.tile([128, 512], F32, name=f"nfp{j}{h}",
                                          tag="mm_ps")
                            for k in range(4):
                                nc.tensor.matmul(
                                    ps[:], wv[k][:, 128 * j:128 * (j + 1)],
                                    xi[k][:, 512 * h:512 * (h + 1)],
                                    start=(k == 0), stop=(k == 3))
                            nc.scalar.activation(t[:, 512 * h:512 * (h + 1)],
                                                 ps[:], AF.Relu, bias=bv[j][:])
                    nc.vector.tensor_mul(nfs[:], fej[:], lej[:])
                    nc.scalar.activation(nfs[:], nfs[:], AF.Identity,
                                         accum_out=pay[j][:, 0:1])
                    e1 = smallp.tile([128, 2], F32, name="edge1", tag="edge1")
                    e2 = smallp.tile([128, 2], F32, name="edge2", tag="edge2")
                    nc.vector.tensor_mul(e1[:], fej[:, 0:2], lej[:, 0:2])
                    nc.vector.tensor_mul(e2[:], fej[:, 1022:1024],
                                         lej[:, 1022:1024])
                    nc.vector.tensor_mul(pay[j][:, 1:3], e1[:], maskrep[:, 0:2])
                    nc.vector.tensor_mul(pay[j][:, 3:5], e2[:], maskrep[:, 2:4])

                pay_in = dramp.tile([N, 5], F32, name="pay_in")
                pay_out = dramp.tile([N, 5], F32, name="pay_out")
                for j in range(NCH):
                    nc.sync.dma_start(pay_in[128 * j:128 * (j + 1), :], pay[j][:])
                nc.gpsimd.collective_compute(
                    "AllReduce", AL.add, replica_groups=[list(range(8))],
                    ins=[pay_in.opt()], outs=[pay_out.opt()])

                # s vectors + TCN matvec (k2t loaded into freed adj slots)
                k2 = [load("k2t", [128, 256], part=(128 * c, 128 * (c + 1)),
                           tag=f"t_adjb_({128 * (c % 8)}, {128 * (c % 8 + 1)})")
                      for c in range(24)]
                yps = mps.tile([1, 256], F32, name="yps", tag="yps")
                sfls = []
                for j in range(NCH):
                    red = smallp.tile([128, 5], F32, name=f"red{j}",
                                      tag=f"pay{j}", bufs=1)
                    nc.sync.dma_start(red[:], pay_out[128 * j:128 * (j + 1), :])
                    sfl = smallp.tile([128, 3], F32, name=f"sfl{j}",
                                      tag=f"sfl{j}", bufs=1)
                    t01 = smallp.tile([128, 1], F32, name=f"t01_{j}", tag="t01")
                    nc.vector.tensor_sub(sfl[:, 0:1], red[:, 0:1], red[:, 3:4])
                    nc.vector.tensor_sub(sfl[:, 0:1], sfl[:, 0:1], red[:, 4:5])
                    nc.vector.tensor_sub(t01[:], red[:, 0:1], red[:, 1:2])
                    nc.vector.tensor_sub(sfl[:, 1:2], t01[:], red[:, 4:5])
                    nc.vector.tensor_sub(sfl[:, 2:3], t01[:], red[:, 2:3])
                    sfls.append(sfl)
                for k in range(3):
                    for j in range(NCH):
                        ch = k * 8 + j
                        nc.tensor.matmul(yps[:], sfls[j][:, k:k + 1], k2[ch][:],
                                         start=(ch == 0), stop=(ch == 23))

                ysb = smallp.tile([128, 256], F32, name="ysb", tag="ysb")
                nc.vector.tensor_add(ysb[0:1, :], yps[:], tcnb[:])
                y_dram = dramp.tile([1, 256], F32, name="y_dram")
                nc.sync.dma_start(y_dram[:], ysb[0:1, :])
                yrep = smallp.tile([8, 256], F32, name="yrep", tag="yrep")
                nc.sync.dma_start(yrep[:], y_dram[:].partition_broadcast(8))
                ypad = smallp.tile([8, 256], F32, name="ypad", tag="ypad")
                nc.vector.tensor_scalar_mul(ypad[:], yrep[:], onehot[:])
                yar_in = dramp.tile([8, 256], F32, name="yar_in")
                yar_out = dramp.tile([8, 256], F32, name="yar_out")
                nc.sync.dma_start(yar_in[:], ypad[:])
                nc.gpsimd.collective_compute(
                    "AllReduce", AL.add, replica_groups=[list(range(8))],
                    ins=[yar_in.opt()], outs=[yar_out.opt()])
                yfull = smallp.tile([8, 256], F32, name="yfull", tag="yfull")
                nc.sync.dma_start(yfull[:], yar_out[:])
                ysq = smallp.tile([8, 256], F32, name="ysq", tag="ysq")
                ss8 = smallp.tile([8, 1], F32, name="ss8", tag="ss8")
                nc.scalar.activation(ysq[:], yfull[:], AF.Square,
                                     accum_out=ss8[:])
                sstot = smallp.tile([1, 1], F32, name="sstot", tag="sstot")
                nc.gpsimd.tensor_reduce(sstot[:], ss8[:],
                                        axis=mybir.AxisListType.C, op=AL.add)
                nc.scalar.activation(sstot[:], sstot[:], AF.Sqrt)
                nc.vector.reciprocal(sstot[:], sstot[:])
                invn8 = smallp.tile([8, 1], F32, name="invn8", tag="invn8")
                nc.gpsimd.partition_broadcast(invn8[:], sstot[:])
                yn = smallp.tile([8, 256], F32, name="yn", tag="yn")
                nc.scalar.activation(yn[:], yfull[:], AF.Copy, scale=invn8[:])
                nc.sync.dma_start(out_d.ap(), yn[:])

    nc.compile()
    _CACHE["nc"] = nc
    return nc


def _prep_inputs(batch_points, batch_descs, batch_adj, params):
    f32 = np.float32

    def A(x):
        return np.asarray(x, dtype=f32)

    pts = A(batch_points); descs = A(batch_descs); adjf = A(batch_adj)
    pe = params["pe"]; gcn = params["gcn"]
    eps = 1e-5
    pe_w = [A(w) for w in pe["W"]]
    pe_b = [A(b) for b in pe["b"]]
    folded = []
    for i in range(3):
        W, b = pe_w[i], pe_b[i]
        if i < 2:
            g, bt, m, v = [A(t) for t in pe["bn"][i]]
            gp = g / np.sqrt(v + eps)
            W = W * gp[None, :]
            b = b * gp + (bt - m * gp)
        folded.append((W, b))

    shared = {
        "pw1": folded[0][0], "pb1": folded[0][1][:, None],
        "pw2": folded[1][0], "pb2": folded[1][1][:, None],
        "pw3": folded[2][0], "pb3": folded[2][1][:, None],
    }
    for l, gk in ((1, "gat1"), (2, "gat2")):
        g = gcn[gk]
        W = A(g["W"]); a = A(g["a"]); Wo = A(g["Wo"]); ao = A(g["ao"])
        wall = np.transpose(W, (1, 0, 2)).reshape(NF, NF)
        ahat = np.zeros((NF, 16), f32)
        for h in range(NH):
            ahat[:, 2 * h] = W[h] @ a[h][:NHID]
            ahat[:, 2 * h + 1] = W[h] @ a[h][NHID:]
        aohat = np.stack([Wo @ ao[:NF], Wo @ ao[NF:]], axis=1)
        shared[f"wall{l}"] = wall.astype(BF)
        shared[f"ahat{l}"] = ahat.astype(BF)
        shared[f"wo{l}"] = Wo.astype(BF)
        shared[f"aohat{l}"] = aohat.astype(BF)
    for nm, wk, bk in (("tw", "tran1_W", "tran1_b"), ("f1", "fe1_W", "fe1_b"),
                      ("f2", "fe2_W", "fe2_b"), ("l1", "le1_W", "le1_b"),
                      ("l2", "le2_W", "le2_b")):
        wn = nm if nm == "tw" else nm + "w"
        bn = "tb" if nm == "tw" else nm + "b"
        shared[wn] = A(gcn[wk]).astype(BF)
        shared[bn] = A(gcn[bk])[:, None]

    K2 = np.transpose(A(params["tcn_K"]), (0, 2, 1)).reshape(TCN_OUT, 3 * NOUT)
    K2T = np.ascontiguousarray((K2 / float(L_OUT)).T)
    tcn_b = A(params["tcn_b"])

    in_maps = []
    for c in range(8):
        m = dict(shared)
        m["ptsT"] = np.ascontiguousarray(pts[c].T)
        m["descsT"] = np.ascontiguousarray(descs[c].T).astype(BF)
        m["adjb"] = adjf[c].astype(BF)
        em = np.zeros((1, 4), f32)
        if c == 0:
            em[0, 0] = em[0, 1] = 1.0
        if c == 7:
            em[0, 2] = em[0, 3] = 1.0
        m["edgemask"] = em
        oh = np.zeros((8, 1), f32); oh[c, 0] = 1.0
        m["onehot"] = oh
        m["k2t"] = np.ascontiguousarray(K2T[:, 256 * c:256 * (c + 1)])
        m["tcnb"] = np.ascontiguousarray(tcn_b[256 * c:256 * (c + 1)])[None, :]
        for k in list(m):
            if m[k].dtype == np.float64:
                m[k] = m[k].astype(np.float32)
        in_maps.append(m)
    return in_maps


def kernel(batch_points, batch_descs, batch_adj, params, _trace=False):
    nc = _build()
    in_maps = _prep_inputs(batch_points, batch_descs, batch_adj, params)
    res = run_bass_kernel_spmd(nc, in_maps, core_ids=list(range(8)),
                               trace=_trace)
    kernel.last_result = res
    return res.results[0]["out"].reshape(1, TCN_OUT).astype(np.float32)


# revision 19
# speedup vs baseline: 1.5736x; 1.0397x over previous
"""AirObject GNN kernel for 8 Trainium2 NeuronCores (Bass/Tile).

Data-parallel over the T=8 graphs (one graph per core). Feature-major
activations. Attention uses the exact identity
  exp(leaky_0.2(e)) = max(exp(0.2 e), exp(e)),  e[j,i] = f1[i]+f2[j]
which is rank-1 in exp space: Q = adj * max(a_j*b_i, c_j*d_i); per map
only 2 tensor_scalar + 2 tensor_tensor passes, spread over DVE/ACT/GPS.
Softmax denominator rides the PE matmul as a ones-column. The TCN+mean
collapses exactly to a matvec against sliding column-sums, so only
[1024,5] column-sum/edge data crosses cores (AllReduce), each core does
a 256-row slice of the final matvec, AllReduce-gathers y, l2norms.
"""
import numpy as np
import ml_dtypes

import concourse.bacc as bacc
import concourse.tile as tile
import concourse.mybir as mybir
from concourse.bass_utils import run_bass_kernel_spmd

F32 = mybir.dt.float32
BF16 = mybir.dt.bfloat16
AL = mybir.AluOpType
AF = mybir.ActivationFunctionType

T, N = 8, 1024
NF = 512
NHID, NH = 64, 8
NOUT = 1024
TCN_OUT, TCN_K = 2048, 3
L_OUT = T * N - TCN_K + 1
NCH = N // 128
BF = ml_dtypes.bfloat16

_CACHE = {}


def _engine_seq():
    pat = ["DVE", "ACT"]
    i = 0
    while True:
        yield pat[i % len(pat)]
        i += 1


def _build():
    if "nc" in _CACHE:
        return _CACHE["nc"]
    nc = bacc.Bacc("TRN2", debug=False, num_devices=8)
    D = {}

    def din(name, shape, dt):
        D[name] = nc.dram_tensor(name, shape, dt, kind="ExternalInput")

    din("ptsT", [2, N], F32)
    din("descsT", [256, N], BF16)
    din("adjb", [N, N], BF16)
    din("edgemask", [1, 4], F32)
    din("onehot", [8, 1], F32)
    din("k2t", [3 * NOUT, 256], F32)
    din("tcnb", [1, 256], F32)
    din("pw1", [2, 64], F32); din("pb1", [64, 1], F32)
    din("pw2", [64, 128], F32); din("pb2", [128, 1], F32)
    din("pw3", [128, 256], F32); din("pb3", [256, 1], F32)
    for l in (1, 2):
        din(f"wall{l}", [NF, NF], BF16)
        din(f"ahat{l}", [NF, 16], BF16)
        din(f"wo{l}", [NF, NF], BF16)
        din(f"aohat{l}", [NF, 2], BF16)
    din("tw", [NF, NF], BF16); din("tb", [NF, 1], F32)
    din("f1w", [NF, NF], BF16); din("f1b", [NF, 1], F32)
    din("f2w", [NF, NOUT], BF16); din("f2b", [NOUT, 1], F32)
    din("l1w", [NF, NF], BF16); din("l1b", [NF, 1], F32)
    din("l2w", [NF, NOUT], BF16); din("l2b", [NOUT, 1], F32)
    out_d = nc.dram_tensor("out", [8, 256], F32, kind="ExternalOutput")

    eng = _engine_seq()

    with tile.TileContext(nc) as tc:
        ctx_outer = [
            tc.tile_pool(name="pw", bufs=1),      # persistent inputs/weights
            tc.tile_pool(name="px", bufs=1),      # persistent activations
            tc.tile_pool(name="uvp", bufs=3),     # u/v/elu scratch
            tc.tile_pool(name="qp", bufs=3),      # q rotating (heads)
            tc.tile_pool(name="qmp", bufs=1),     # q persistent (out-map)
            tc.tile_pool(name="repp", bufs=2),    # broadcast rows
            tc.tile_pool(name="evp", bufs=1),     # whn/whno/xcat evacs
            tc.tile_pool(name="smallp", bufs=2),  # small scratch
            tc.tile_pool(name="rowp", bufs=1),    # f32 exp rows
            tc.tile_pool(name="dramp", bufs=1, space="DRAM"),
        ]
        import contextlib
        with contextlib.ExitStack() as ST:
            pw, px, uvp, qp, qmp, repp, evp, smallp, rowp, dramp = [
                ST.enter_context(c) for c in ctx_outer]

            _ldq = [nc.sync, nc.scalar]
            _ldi = [0]

            def load(name, shape=None, dt=None, part=None, tag=None, pool=pw):
                h = D[name]
                shape = shape or list(h.shape)
                t = pool.tile(shape, dt or h.dtype, name=f"t_{name}_{part}",
                              tag=tag or f"t_{name}_{part}")
                src = h.ap()
                if part is not None:
                    src = src[part[0]:part[1], :]
                _ldq[_ldi[0] % 2].dma_start(t[:], src)
                _ldi[0] += 1
                return t

            adj = [load("adjb", [128, N], part=(128 * j, 128 * (j + 1)))
                   for j in range(NCH)]
            x_fm = [load("descsT", [128, N], part=(128 * j, 128 * (j + 1)))
                    for j in range(2)]
            tcnb = load("tcnb")
            onehot = load("onehot")
            maskrep = pw.tile([128, 4], F32, name="maskrep")
            nc.sync.dma_start(maskrep[:], D["edgemask"].ap().partition_broadcast(128))
            ones1 = pw.tile([128, 1], F32, name="ones1")
            nc.vector.memset(ones1[:], 1.0)
            srow_scr = px.tile([128, N], F32, name="srow_scr")

            # ============ points encoder (fp32, feature-major) ============
            with tc.tile_pool(name="pes", bufs=1) as pes, \
                 tc.tile_pool(name="peps", bufs=2, space="PSUM") as peps:
                ptsT = load("ptsT", pool=pes)
                pw1 = load("pw1", pool=pes); pb1 = load("pb1", pool=pes)
                pw2 = load("pw2", pool=pes); pb2 = load("pb2", pool=pes)
                pw3 = load("pw3", pool=pes)
                pb3 = [load("pb3", [128, 1], part=(128 * j, 128 * (j + 1)),
                            pool=pes) for j in range(2)]
                o1 = pes.tile([64, N], F32, name="pe_o1")
                for h in range(2):
                    ps = peps.tile([64, 512], F32, name=f"pe1_{h}", tag="pe_ps")
                    nc.tensor.matmul(ps[:], pw1[:], ptsT[:, 512 * h:512 * (h + 1)],
                                     start=True, stop=True)
                    nc.scalar.activation(o1[:, 512 * h:512 * (h + 1)], ps[:],
                                         AF.Relu, bias=pb1[0:64, :])
                o2 = pes.tile([128, N], F32, name="pe_o2")
                for h in range(2):
                    ps = peps.tile([128, 512], F32, name=f"pe2_{h}", tag="pe_ps")
                    nc.tensor.matmul(ps[:], pw2[:], o1[:, 512 * h:512 * (h + 1)],
                                     start=True, stop=True)
                    nc.scalar.activation(o2[:, 512 * h:512 * (h + 1)], ps[:],
                                         AF.Relu, bias=pb2[:])
                ep = [pes.tile([128, N], F32, name=f"pe_ep{c}") for c in range(2)]
                epsq = pes.tile([128, N], F32, name="pe_sq", tag="pe_o1")
                sq_ps = [peps.tile([1, 512], F32, name=f"ssq{h}", tag="ssq_ps")
                         for h in range(2)]
                for c in range(2):
                    for h in range(2):
                        ps = peps.tile([128, 512], F32, name=f"pe3_{c}{h}",
                                       tag="pe_ps")
                        nc.tensor.matmul(ps[:], pw3[:, 128 * c:128 * (c + 1)],
                                         o2[:, 512 * h:512 * (h + 1)],
                                         start=True, stop=True)
                        sl = (slice(None), slice(512 * h, 512 * (h + 1)))
                        nc.scalar.activation(ep[c][sl], ps[:], AF.Identity,
                                             bias=pb3[c][:])
                        nc.scalar.activation(epsq[sl], ps[:], AF.Square,
                                             bias=pb3[c][:])
                        nc.tensor.matmul(sq_ps[h][:], ones1[:], epsq[sl],
                                         start=(c == 0), stop=(c == 1))
                for h in range(2):
                    nc.scalar.copy(srow_scr[0:1, 512 * h:512 * (h + 1)],
                                   sq_ps[h][:])
                ssq_dram = dramp.tile([1, N], F32, name="ssq_dram")
                nc.sync.dma_start(ssq_dram[:], srow_scr[0:1, :])
                ssq128 = pes.tile([128, 8], F32, name="ssq128")
                nc.sync.dma_start(ssq128[:],
                                  ssq_dram[:].rearrange("a (p c) -> (a p) c", c=8))
                nc.scalar.activation(ssq128[:], ssq128[:], AF.Sqrt)
                nc.vector.reciprocal(ssq128[:], ssq128[:])
                inv_dram = dramp.tile([1, N], F32, name="inv_dram")
                nc.sync.dma_start(inv_dram[:].rearrange("a (p c) -> (a p) c", c=8),
                                  ssq128[:])
                invrep = pes.tile([128, N], F32, name="invrep", tag="pe_o2")
                nc.sync.dma_start(invrep[:], inv_dram[:].partition_broadcast(128))
                for c in range(2):
                    epn = px.tile([128, N], BF16, name=f"x_ep{c}")
                    nc.vector.tensor_mul(epn[:], ep[c][:], invrep[:])
                    x_fm.append(epn)

            # ===================== GAT layers =====================
            def gat_layer(l, x_in, gtag):
                with tc.tile_pool(name=f"gw{l}", bufs=1) as gw, \
                     tc.tile_pool(name=f"gps{l}", bufs=2, space="PSUM") as gpsm, \
                     tc.tile_pool(name=f"gpsa{l}", bufs=2, space="PSUM") as gpsa:
                    wall = [load(f"wall{l}", [128, NF],
                                 part=(128 * k, 128 * (k + 1)),
                                 tag=f"wall{k}", pool=gw) for k in range(4)]
                    ahat = [load(f"ahat{l}", [128, 16],
                                 part=(128 * k, 128 * (k + 1)),
                                 tag=f"ahat{k}", pool=gw) for k in range(4)]
                    wo = [load(f"wo{l}", [64, NF], part=(64 * k, 64 * (k + 1)),
                               tag=f"wo{k}", pool=gw) for k in range(8)]
                    aoh = [load(f"aohat{l}", [64, 2], part=(64 * k, 64 * (k + 1)),
                                tag=f"aoh{k}", pool=gw) for k in range(8)]

                    # Wh node-major, [Wh_h | ones] 65-stride interleave
                    whn = []
                    for j in range(NCH):
                        t = evp.tile([128, 65 * NH], BF16, name=f"whn_{j}",
                                     tag=f"whn{j}")
                        ps = gpsm.tile([128, 512], F32, name=f"whps_{j}",
                                       tag="mm_ps")
                        for k in range(4):
                            nc.tensor.matmul(ps[:],
                                             x_in[k][:, 128 * j:128 * (j + 1)],
                                             wall[k][:], start=(k == 0),
                                             stop=(k == 3))
                        ot = t[:].rearrange("p (h c) -> p h c", c=65)
                        nc.scalar.copy(ot[:, :, 0:64],
                                       ps[:].rearrange("p (h c) -> p h c", c=64))
                        nc.vector.memset(ot[:, :, 64:65], 1.0)
                        whn.append(t)

                    # f vectors feature-major; exp rows; transposed scalars
                    fps = [gpsa.tile([16, 512], F32, name=f"fps_{h}", tag="f_ps")
                           for h in range(2)]
                    for h in range(2):
                        for k in range(4):
                            nc.tensor.matmul(fps[h][:], ahat[k][:],
                                             x_in[k][:, 512 * h:512 * (h + 1)],
                                             start=(k == 0), stop=(k == 3))
                    arow = rowp.tile([16, N], BF16, name="arow", tag="arow")
                    crow = rowp.tile([16, N], BF16, name="crow", tag="crow")
                    for h in range(2):
                        sl = (slice(None), slice(512 * h, 512 * (h + 1)))
                        nc.scalar.activation(arow[sl], fps[h][:], AF.Exp,
                                             scale=0.2)
                        nc.scalar.activation(crow[sl], fps[h][:], AF.Exp,
                                             scale=1.0)
                    ac_dram = dramp.tile([32, N], BF16, name=f"acd{l}", tag="acd")
                    nc.sync.dma_start(ac_dram[0:16, :], arow[:])
                    nc.sync.dma_start(ac_dram[16:32, :], crow[:])
                    aT, cT = [], []
                    for j in range(NCH):
                        fnm = gpsa.tile([128, 16], F32, name=f"fnm_{j}",
                                        tag="f_ps")
                        for k in range(4):
                            nc.tensor.matmul(fnm[:],
                                             x_in[k][:, 128 * j:128 * (j + 1)],
                                             ahat[k][:], start=(k == 0),
                                             stop=(k == 3))
                        at = rowp.tile([128, 16], F32, name=f"aT_{j}",
                                       tag=f"aT{j}")
                        ct = rowp.tile([128, 16], F32, name=f"cT_{j}",
                                       tag=f"cT{j}")
                        nc.scalar.activation(at[:], fnm[:], AF.Exp, scale=0.2)
                        nc.scalar.activation(ct[:], fnm[:], AF.Exp, scale=1.0)
                        aT.append(at)
                        cT.append(ct)

                    def make_q(j, brep, drep, a_col, c_col, qpool, qtag):
                        u = uvp.tile([128, N], BF16, name="u_t", tag="u_t")
                        v = uvp.tile([128, N], BF16, name="v_t", tag="v_t")
                        for (tt, rep, col) in ((u, brep, a_col), (v, drep, c_col)):
                            e = next(eng)
                            if e == "ACT":
                                nc.scalar.activation(tt[:], rep[:], AF.Copy,
                                                     scale=col)
                            elif e == "GPS":
                                nc.gpsimd.tensor_scalar_mul(tt[:], rep[:], col)
                            else:
                                nc.vector.tensor_scalar_mul(tt[:], rep[:], col)
                        q = qpool.tile([128, N], BF16, name="q_t", tag=qtag)
                        nc.vector.tensor_max(q[:], u[:], v[:])
                        nc.vector.tensor_mul(q[:], q[:], adj[j][:])
                        return q

                    # ---- heads ----
                    s_dram = dramp.tile([NH, N], F32, name=f"sdram{l}",
                                        tag="sdram")
                    xcat = []
                    for hh in range(NH):
                        brep = repp.tile([128, N], BF16, name="brep", tag="brep",
                                         bufs=3)
                        drep = repp.tile([128, N], BF16, name="drep", tag="drep",
                                         bufs=3)
                        nc.sync.dma_start(
                            brep[:],
                            ac_dram[2 * hh:2 * hh + 1, :].partition_broadcast(128))
                        nc.sync.dma_start(
                            drep[:],
                            ac_dram[16 + 2 * hh:16 + 2 * hh + 1, :]
                            .partition_broadcast(128))
                        hps = gpsa.tile([65, N], F32, name=f"hps_{hh}",
                                        tag="att_ps")
                        for j in range(NCH):
                            q = make_q(j, brep, drep,
                                       aT[j][:, 2 * hh + 1:2 * hh + 2],
                                       cT[j][:, 2 * hh + 1:2 * hh + 2],
                                       qp, "q_t")
                            for h in range(2):
                                nc.tensor.matmul(
                                    hps[:, 512 * h:512 * (h + 1)],
                                    whn[j][:, 65 * hh:65 * hh + 65],
                                    q[:, 512 * h:512 * (h + 1)],
                                    start=(j == 0), stop=(j == NCH - 1))
                        nc.scalar.copy(srow_scr[64:65, :], hps[64:65, :])
                        nc.sync.dma_start(s_dram[hh:hh + 1, :],
                                          srow_scr[64:65, :])
                        ev = evp.tile([64, N], BF16, name=f"hev_{hh}",
                                      tag=f"hev{hh}")
                        nc.scalar.copy(ev[:], hps[0:64, :])
                        xcat.append(ev)
                    s128 = smallp.tile([128, 64], F32, name="s128", tag="s128")
                    nc.sync.dma_start(
                        s128[:], s_dram[:].rearrange("h (g c) -> (h g) c", c=64))
                    nc.vector.reciprocal(s128[:], s128[:])
                    nc.sync.dma_start(
                        s_dram[:].rearrange("h (g c) -> (h g) c", c=64), s128[:])
                    for hh in range(NH):
                        sirep = repp.tile([128, N], F32, name="sirep",
                                          tag="sirep")
                        nc.sync.dma_start(
                            sirep[:],
                            s_dram[hh:hh + 1, :].partition_broadcast(128))
                        xh = xcat[hh]
                        nc.gpsimd.tensor_mul(xh[:], xh[:], sirep[0:64, :])
                        ex = uvp.tile([64, N], BF16, name="elu_e", tag="u_t")
                        nc.scalar.activation(ex[:], xh[:], AF.Exp)
                        nc.vector.tensor_scalar_add(ex[:], ex[:], -1.0)
                        nc.vector.tensor_scalar_min(ex[:], ex[:], 0.0)
                        nc.vector.tensor_max(xh[:], xh[:], ex[:])

                    # ---- output attention layer ----
                    whno = []
                    for j in range(NCH):
                        t = evp.tile([128, 513], BF16, name=f"whno_{j}",
                                     tag=f"whno{j}")
                        ps = gpsm.tile([128, 512], F32, name=f"wops_{j}",
                                       tag="mm_ps")
                        for k in range(8):
                            nc.tensor.matmul(ps[:],
                                             xcat[k][:, 128 * j:128 * (j + 1)],
                                             wo[k][:], start=(k == 0),
                                             stop=(k == 7))
                        nc.scalar.copy(t[:, 0:512], ps[:])
                        nc.vector.memset(t[:, 512:513], 1.0)
                        whno.append(t)
                    fops = [gpsa.tile([2, 512], F32, name=f"fo_{h}", tag="f_ps")
                            for h in range(2)]
                    for h in range(2):
                        for k in range(8):
                            nc.tensor.matmul(fops[h][:], aoh[k][:],
                                             xcat[k][:, 512 * h:512 * (h + 1)],
                                             start=(k == 0), stop=(k == 7))
                    aco_dram = dramp.tile([4, N], BF16, name=f"acod{l}",
                                          tag="acod")
                    aorow = rowp.tile([2, N], BF16, name="aorow", tag="aorow")
                    corow = rowp.tile([2, N], BF16, name="corow", tag="corow")
                    for h in range(2):
                        sl = (slice(None), slice(512 * h, 512 * (h + 1)))
                        nc.scalar.activation(aorow[sl], fops[h][:], AF.Exp,
                                             scale=0.2)
                        nc.scalar.activation(corow[sl], fops[h][:], AF.Exp,
                                             scale=1.0)
                    nc.sync.dma_start(aco_dram[0:2, :], aorow[:])
                    nc.sync.dma_start(aco_dram[2:4, :], corow[:])
                    aoT, coT = [], []
                    for j in range(NCH):
                        fonm = gpsa.tile([128, 2], F32, name=f"fonm_{j}",
                                         tag="f_ps")
                        for k in range(8):
                            nc.tensor.matmul(fonm[:],
                                             xcat[k][:, 128 * j:128 * (j + 1)],
                                             aoh[k][:], start=(k == 0),
                                             stop=(k == 7))
                        at = rowp.tile([128, 2], F32, name=f"aoT_{j}",
                                       tag=f"aoT{j}")
                        ct = rowp.tile([128, 2], F32, name=f"coT_{j}",
                                       tag=f"coT{j}")
                        nc.scalar.activation(at[:], fonm[:], AF.Exp, scale=0.2)
                        nc.scalar.activation(ct[:], fonm[:], AF.Exp, scale=1.0)
                        aoT.append(at)
                        coT.append(ct)

                    brep = repp.tile([128, N], BF16, name="brep", tag="brep",
                                     bufs=3)
                    drep = repp.tile([128, N], BF16, name="drep", tag="drep",
                                     bufs=3)
                    nc.sync.dma_start(brep[:],
                                      aco_dram[0:1, :].partition_broadcast(128))
                    nc.sync.dma_start(drep[:],
                                      aco_dram[2:3, :].partition_broadcast(128))
                    sps_o = [gpsa.tile([1, 512], F32, name=f"spso{h}", tag="f_ps")
                             for h in range(2)]
                    qmap = []
                    for j in range(NCH):
                        q = make_q(j, brep, drep, aoT[j][:, 1:2],
                                   coT[j][:, 1:2], qmp, f"qm{j}")
                        qmap.append(q)
                        for h in range(2):
                            nc.tensor.matmul(sps_o[h][:],
                                             whno[j][:, 512:513],
                                             q[:, 512 * h:512 * (h + 1)],
                                             start=(j == 0), stop=(j == NCH - 1))
                    for h in range(2):
                        nc.scalar.copy(srow_scr[0:1, 512 * h:512 * (h + 1)],
                                       sps_o[h][:])
                    so_dram = dramp.tile([1, N], F32, name=f"sod{l}", tag="sod")
                    nc.sync.dma_start(so_dram[:], srow_scr[0:1, :])
                    so128 = smallp.tile([128, 8], F32, name="so128", tag="so128")
                    nc.sync.dma_start(
                        so128[:], so_dram[:].rearrange("a (p c) -> (a p) c", c=8))
                    nc.vector.reciprocal(so128[:], so128[:])
                    nc.sync.dma_start(
                        so_dram[:].rearrange("a (p c) -> (a p) c", c=8), so128[:])
                    sorep = repp.tile([128, N], F32, name="sorep", tag="sirep")
                    nc.sync.dma_start(sorep[:],
                                      so_dram[:].partition_broadcast(128))
                    gout = []
                    ops_t = {}
                    for grp in range(2):
                        for m in (2 * grp, 2 * grp + 1):
                            ops_t[m] = gpsa.tile([128, N], F32, name=f"ops_{m}",
                                                 tag="att_ps")
                        for j in range(NCH):
                            for m in (2 * grp, 2 * grp + 1):
                                for h in range(2):
                                    nc.tensor.matmul(
                                        ops_t[m][:, 512 * h:512 * (h + 1)],
                                        whno[j][:, 128 * m:128 * (m + 1)],
                                        qmap[j][:, 512 * h:512 * (h + 1)],
                                        start=(j == 0), stop=(j == NCH - 1))
                    for m in range(4):
                        ops = ops_t[m]
                        g = px.tile([128, N], BF16, name=f"gout{l}_{m}",
                                    tag=f"{gtag}{m}")
                        nc.scalar.copy(g[:], ops[:])
                        nc.gpsimd.tensor_mul(g[:], g[:], sorep[:])
                        ex = uvp.tile([128, N], BF16, name="elu_o", tag="u_t")
                        nc.scalar.activation(ex[:], g[:], AF.Exp)
                        nc.vector.tensor_scalar_add(ex[:], ex[:], -1.0)
                        nc.vector.tensor_scalar_min(ex[:], ex[:], 0.0)
                        nc.vector.tensor_max(g[:], g[:], ex[:])
                        gout.append(g)
                    return gout

            g1 = gat_layer(1, x_fm, "goutA")
            g2 = gat_layer(2, g1, "goutB")

            # ============ MLPs + payload (feature-major) ============
            with tc.tile_pool(name="mw", bufs=1) as mw, \
                 tc.tile_pool(name="mps", bufs=4, space="PSUM") as mps:

                def loadw(name, n_out, k):
                    return load(name, [128, n_out], part=(128 * k, 128 * (k + 1)),
                                tag=f"mlpw{k}", pool=mw)

                def loadb(name, m):
                    return load(name, [128, 1], part=(128 * m, 128 * (m + 1)),
                                tag=f"mlpb{m % 4}_{name}", pool=mw)

                def mlp(x_in, wname, bname, n_out, xtag, pool):
                    wv = [loadw(wname, n_out, k) for k in range(4)]
                    bv = [loadb(bname, m) for m in range(n_out // 128)]
                    out = []
                    for m in range(n_out // 128):
                        t = pool.tile([128, N], BF16, name=f"o_{wname}_{m}",
                                      tag=f"{xtag}{m}")
                        for h in range(2):
                            ps = mps.tile([128, 512], F32,
                                          name=f"mp{wname}{m}{h}", tag="mm_ps")
                            for k in range(4):
                                nc.tensor.matmul(
                                    ps[:], wv[k][:, 128 * m:128 * (m + 1)],
                                    x_in[k][:, 512 * h:512 * (h + 1)],
                                    start=(k == 0), stop=(k == 3))
                            nc.scalar.activation(t[:, 512 * h:512 * (h + 1)],
                                                 ps[:], AF.Relu, bias=bv[m][:])
                        out.append(t)
                    return out

                tr = mlp(g2, "tw", "tb", NF, "goutA", px)
                f1o = mlp(tr, "f1w", "f1b", NF, "hev", evp)
                l1o_tiles = []
                for m in range(4):
                    wv = [loadw("l1w", NF, k) for k in range(4)]
                    bv = loadb("l1b", m)
                    t = evp.tile([128, N], BF16, name=f"o_l1w_{m}",
                                 tag=f"hev{4 + m}")
                    for h in range(2):
                        ps = mps.tile([128, 512], F32, name=f"mpl1{m}{h}",
                                      tag="mm_ps")
                        for k in range(4):
                            nc.tensor.matmul(
                                ps[:], wv[k][:, 128 * m:128 * (m + 1)],
                                tr[k][:, 512 * h:512 * (h + 1)],
                                start=(k == 0), stop=(k == 3))
                        nc.scalar.activation(t[:, 512 * h:512 * (h + 1)],
                                             ps[:], AF.Relu, bias=bv[:])
                    l1o_tiles.append(t)
                l1o = l1o_tiles

                f2wv = [loadw("f2w", NOUT, k) for k in range(4)]
                f2bv = [loadb("f2b", m) for m in range(8)]
                l2wv = [load("l2w", [128, NOUT], part=(128 * k, 128 * (k + 1)),
                             tag=f"mlpw2{k}", pool=mw) for k in range(4)]
                l2bv = [load("l2b", [128, 1], part=(128 * m, 128 * (m + 1)),
                             tag=f"mlpb2{m}", pool=mw) for m in range(8)]

                pay = [smallp.tile([128, 5], F32, name=f"pay{j}", tag=f"pay{j}",
                       bufs=1) for j in range(NCH)]
                for j in range(NCH):
                    fej = px.tile([128, N], BF16, name="fej", tag="fej", bufs=2)
                    lej = px.tile([128, N], BF16, name="lej", tag="lej", bufs=2)
                    nfs = px.tile([128, N], BF16, name="nf_scr", tag="nf_scr",
                                  bufs=2)
                    for (t, wv, bv, xi) in ((fej, f2wv, f2bv, f1o),
                                            (lej, l2wv, l2bv, l1o)):
                        for h in range(2):
                            ps = mps.tile([128, 512], F32, name=f"nfp{j}{h}",
                                          tag="mm_ps")
                            for k in range(4):
                                nc.tensor.matmul(
                                    ps[:], wv[k][:, 128 * j:128 * (j + 1)],
                                    xi[k][:, 512 * h:512 * (h + 1)],
                                    start=(k == 0), stop=(k == 3))
                            nc.scalar.activation(t[:, 512 * h:512 * (h + 1)],
                                                 ps[:], AF.Relu, bias=bv[j][:])
                    nc.vector.tensor_mul(nfs[:], fej[:], lej[:])
                    nc.scalar.activation(nfs[:], nfs[:], AF.Identity,
                                         accum_out=pay[j][:, 0:1])
                    e1 = smallp.tile([128, 2], F32, name="edge1", tag="edge1")
                    e2 = smallp.tile([128, 2], F32, name="edge2", tag="edge2")
                    nc.vector.tensor_mul(e1[:], fej[:, 0:2], lej[:, 0:2])
                    nc.vector.tensor_mul(e2[:], fej[:, 1022:1024],
                                         lej[:, 1022:1024])
                    nc.vector.tensor_mul(pay[j][:, 1:3], e1[:], maskrep[:, 0:2])
                    nc.vector.tensor_mul(pay[j][:, 3:5], e2[:], maskrep[:, 2:4])

                pay_in = dramp.tile([N, 5], F32, name="pay_in")
                pay_out = dramp.tile([N, 5], F32, name="pay_out")
                for j in range(NCH):
                    nc.sync.dma_start(pay_in[128 * j:128 * (j + 1), :], pay[j][:])
                    if j == 3:
                        nc.gpsimd.collective_compute(
                            "AllReduce", AL.add,
                            replica_groups=[list(range(8))],
                            ins=[pay_in[0:512, :].opt()],
                            outs=[pay_out[0:512, :].opt()])
                nc.gpsimd.collective_compute(
                    "AllReduce", AL.add, replica_groups=[list(range(8))],
                    ins=[pay_in[512:1024, :].opt()],
                    outs=[pay_out[512:1024, :].opt()])

                # s vectors + TCN matvec (k2t loaded into freed adj slots)
                k2 = [load("k2t", [128, 256], part=(128 * c, 128 * (c + 1)),
                           tag=f"t_adjb_({128 * (c % 8)}, {128 * (c % 8 + 1)})")
                      for c in range(24)]
                yps = mps.tile([1, 256], F32, name="yps", tag="yps")
                sfls = []
                for j in range(NCH):
                    red = smallp.tile([128, 5], F32, name=f"red{j}",
                                      tag=f"pay{j}", bufs=1)
                    nc.sync.dma_start(red[:], pay_out[128 * j:128 * (j + 1), :])
                    sfl = smallp.tile([128, 3], F32, name=f"sfl{j}",
                                      tag=f"sfl{j}", bufs=1)
                    t01 = smallp.tile([128, 1], F32, name=f"t01_{j}", tag="t01")
                    nc.vector.tensor_sub(sfl[:, 0:1], red[:, 0:1], red[:, 3:4])
                    nc.vector.tensor_sub(sfl[:, 0:1], sfl[:, 0:1], red[:, 4:5])
                    nc.vector.tensor_sub(t01[:], red[:, 0:1], red[:, 1:2])
                    nc.vector.tensor_sub(sfl[:, 1:2], t01[:], red[:, 4:5])
                    nc.vector.tensor_sub(sfl[:, 2:3], t01[:], red[:, 2:3])
                    sfls.append(sfl)
                for k in range(3):
                    for j in range(NCH):
                        ch = k * 8 + j
                        nc.tensor.matmul(yps[:], sfls[j][:, k:k + 1], k2[ch][:],
                                         start=(ch == 0), stop=(ch == 23))

                ysb = smallp.tile([128, 256], F32, name="ysb", tag="ysb")
                nc.vector.tensor_add(ysb[0:1, :], yps[:], tcnb[:])
                yrep = smallp.tile([8, 256], F32, name="yrep", tag="yrep")
                nc.gpsimd.partition_broadcast(yrep[:], ysb[0:1, :])
                ypad = smallp.tile([8, 256], F32, name="ypad", tag="ypad")
                nc.vector.tensor_scalar_mul(ypad[:], yrep[:], onehot[:])
                yar_in = dramp.tile([8, 256], F32, name="yar_in")
                yar_out = dramp.tile([8, 256], F32, name="yar_out")
                nc.sync.dma_start(yar_in[:], ypad[:])
                nc.gpsimd.collective_compute(
                    "AllReduce", AL.add, replica_groups=[list(range(8))],
                    ins=[yar_in.opt()], outs=[yar_out.opt()])
                yfull = smallp.tile([8, 256], F32, name="yfull", tag="yfull")
                nc.sync.dma_start(yfull[:], yar_out[:])
                ysq = smallp.tile([8, 256], F32, name="ysq", tag="ysq")
                ss8 = smallp.tile([8, 1], F32, name="ss8", tag="ss8")
                nc.scalar.activation(ysq[:], yfull[:], AF.Square,
                                     accum_out=ss8[:])
                sstot = smallp.tile([1, 1], F32, name="sstot", tag="sstot")
                nc.gpsimd.tensor_reduce(sstot[:], ss8[:],
                                        axis=mybir.AxisListType.C, op=AL.add)
                nc.scalar.activation(sstot[:], sstot[:], AF.Sqrt)
                nc.vector.reciprocal(sstot[:], sstot[:])
                invn8 = smallp.tile([8, 1], F32, name="invn8", tag="invn8")
                nc.gpsimd.partition_broadcast(invn8[:], sstot[:])
                yn = smallp.tile([8, 256], F32, name="yn", tag="yn")
                nc.scalar.activation(yn[:], yfull[:], AF.Copy, scale=invn8[:])
                nc.sync.dma_start(out_d.ap(), yn[:])

    nc.compile()
    _CACHE["nc"] = nc
    return nc


def _prep_inputs(batch_points, batch_descs, batch_adj, params):
    f32 = np.float32

    def A(x):
        return np.asarray(x, dtype=f32)

    pts = A(batch_points); descs = A(batch_descs); adjf = A(batch_adj)
    pe = params["pe"]; gcn = params["gcn"]
    eps = 1e-5
    pe_w = [A(w) for w in pe["W"]]
    pe_b = [A(b) for b in pe["b"]]
    folded = []
    for i in range(3):
        W, b = pe_w[i], pe_b[i]
        if i < 2:
            g, bt, m, v = [A(t) for t in pe["bn"][i]]
            gp = g / np.sqrt(v + eps)
            W = W * gp[None, :]
            b = b * gp + (bt - m * gp)
        folded.append((W, b))

    shared = {
        "pw1": folded[0][0], "pb1": folded[0][1][:, None],
        "pw2": folded[1][0], "pb2": folded[1][1][:, None],
        "pw3": folded[2][0], "pb3": folded[2][1][:, None],
    }
    for l, gk in ((1, "gat1"), (2, "gat2")):
        g = gcn[gk]
        W = A(g["W"]); a = A(g["a"]); Wo = A(g["Wo"]); ao = A(g["ao"])
        wall = np.transpose(W, (1, 0, 2)).reshape(NF, NF)
        ahat = np.zeros((NF, 16), f32)
        for h in range(NH):
            ahat[:, 2 * h] = W[h] @ a[h][:NHID]
            ahat[:, 2 * h + 1] = W[h] @ a[h][NHID:]
        aohat = np.stack([Wo @ ao[:NF], Wo @ ao[NF:]], axis=1)
        shared[f"wall{l}"] = wall.astype(BF)
        shared[f"ahat{l}"] = ahat.astype(BF)
        shared[f"wo{l}"] = Wo.astype(BF)
        shared[f"aohat{l}"] = aohat.astype(BF)
    for nm, wk, bk in (("tw", "tran1_W", "tran1_b"), ("f1", "fe1_W", "fe1_b"),
                      ("f2", "fe2_W", "fe2_b"), ("l1", "le1_W", "le1_b"),
                      ("l2", "le2_W", "le2_b")):
        wn = nm if nm == "tw" else nm + "w"
        bn = "tb" if nm == "tw" else nm + "b"
        shared[wn] = A(gcn[wk]).astype(BF)
        shared[bn] = A(gcn[bk])[:, None]

    K2 = np.transpose(A(params["tcn_K"]), (0, 2, 1)).reshape(TCN_OUT, 3 * NOUT)
    K2T = np.ascontiguousarray((K2 / float(L_OUT)).T)
    tcn_b = A(params["tcn_b"])

    in_maps = []
    for c in range(8):
        m = dict(shared)
        m["ptsT"] = np.ascontiguousarray(pts[c].T)
        m["descsT"] = np.ascontiguousarray(descs[c].T).astype(BF)
        m["adjb"] = adjf[c].astype(BF)
        em = np.zeros((1, 4), f32)
        if c == 0:
            em[0, 0] = em[0, 1] = 1.0
        if c == 7:
            em[0, 2] = em[0, 3] = 1.0
        m["edgemask"] = em
        oh = np.zeros((8, 1), f32); oh[c, 0] = 1.0
        m["onehot"] = oh
        m["k2t"] = np.ascontiguousarray(K2T[:, 256 * c:256 * (c + 1)])
        m["tcnb"] = np.ascontiguousarray(tcn_b[256 * c:256 * (c + 1)])[None, :]
        for k in list(m):
            if m[k].dtype == np.float64:
                m[k] = m[k].astype(np.float32)
        in_maps.append(m)
    return in_maps


def kernel(batch_points, batch_descs, batch_adj, params, _trace=False):
    nc = _build()
    in_maps = _prep_inputs(batch_points, batch_descs, batch_adj, params)
    res = run_bass_kernel_spmd(nc, in_maps, core_ids=list(range(8)),
                               trace=_trace)
    kernel.last_result = res
    return res.results[0]["out"].reshape(1, TCN_OUT).astype(np.float32)
